# revision 1
# baseline (speedup 1.0000x reference)
"""Trainium2 Bass kernel for a local-window-attention transformer block (v2).

Sharding: data-parallel over batch (one batch element per NeuronCore).

Key design vs v1:
- Sinusoidal positional encoding folded into the inputs on the host; the
  device receives x+pe both as fp8 (matmul operand) and as a bf16 residual
  copy pre-scaled by the out-projection PSUM scale.
- All intermediates stay in SBUF (no DRAM scratch round-trips).
- QKV + out-projection run as single-fp8 DoubleRow matmuls; FFN w1 runs as
  an fp8 hi/lo "x3" DoubleRow decomposition (hi*hi + lo*hi + hi*lo, which is
  numerically bf16-clean); FFN w2 stays bf16. LayerNorm statistics run as
  bf16 ones-vector matmuls.
- Attention computes transposed scores (K stationary) so the softmax sum is
  a ones-vector matmul; probabilities are normalized during the PV PSUM
  drain with a broadcast reciprocal row.
- LayerNorm is scale-invariant, so the residual chain is carried at the
  PSUM-native scale (eps pre-scaled on the host, no rescale ops).
"""
import numpy as np
import ml_dtypes

import concourse.bass as bass
import concourse.bacc as bacc
import concourse.mybir as mybir
import concourse.tile as tile
from concourse.bass import ts
from concourse.bass_utils import run_bass_kernel_spmd

F32 = mybir.dt.float32
BF16 = mybir.dt.bfloat16
FP8 = mybir.dt.float8e4
AF = mybir.ActivationFunctionType
OP = mybir.AluOpType
DR = mybir.MatmulPerfMode.DoubleRow
BF = ml_dtypes.bfloat16
E4 = ml_dtypes.float8_e4m3

B, T, D, W, H = 8, 2048, 1024, 16, 8
HD = D // H            # 128 = head dim = one partition chunk
FF = 4 * D             # 4096
NCH = D // 128         # 8 feature chunks
NHC = FF // 128        # 32 hidden chunks
EPS = 1e-5
ISQ = float(1.0 / np.sqrt(128.0))
MASKC = 340.0

SX = 16.0              # x+pe fp8 scale
SW = 64.0              # weight fp8 scale
SO = 32.0              # attention-out fp8 scale
SH = 32.0              # LN1-out fp8 scale
CS1 = SO * SW          # out-proj psum scale == residual-1 chain scale

SA = 512               # macro12 token slab
NSA = T // SA
SB = 256               # macro34 token slab
NSB = T // SB

_NC_CACHE = {}


def build_nc(flags):
    (zbv, zb1, zbo, zb2a1, za2) = flags
    nc = bacc.Bacc(None, target_bir_lowering=False)

    dram = {}
    # ---- per-core inputs ----
    dram["x8"] = nc.declare_dram_parameter("x8", [128, NCH, T], FP8,
                                           isOutput=False)
    dram["xs"] = nc.declare_dram_parameter("xs", [128, NCH, T], BF16,
                                           isOutput=False)
    # ---- shared weights ----
    for nm, sh, dt in (
            ("wq8", [128, NCH, D], FP8), ("wk8", [128, NCH, D], FP8),
            ("wv8", [128, NCH, D], FP8), ("wo8", [128, NCH, D], FP8),
            ("w1h", [128, NCH, FF], FP8), ("w1l", [128, NCH, FF], FP8),
            ("w2b", [128, NHC, D], BF16),
            ("bqp", [128, NCH], F32), ("bkp", [128, NCH], F32),
            ("bvS", [1, D], BF16), ("b1p", [128, NHC], F32),
            ("boS", [128, NCH], F32), ("bb2p", [128, NCH], F32),
            ("g1p", [128, NCH], F32), ("g2p", [128, NCH], F32),
            ("be2p", [128, NCH], F32),
            ("masku", [9, 128], BF16), ("maskw", [9, 512], BF16),
            ("onesrow", [1, 128], BF16), ("onescol", [128, 1], BF16),
            ("sorow", [1, 128], BF16)):
        dram[nm] = nc.declare_dram_parameter(nm, sh, dt, isOutput=False)

    dram["outb"] = nc.declare_dram_parameter("outb", [128, NCH, T], BF16,
                                             isOutput=True)

    with tile.TileContext(nc) as tc:
        _emit(nc, tc, flags, dram)
    nc.compile()
    return nc


def _emit(nc, tc, flags, dram):
    (zbv, zb1, zbo, zb2a1, za2) = flags
    open_pools = {}

    def popen(name, **kw):
        cm = tc.tile_pool(name=name, **kw)
        pool = cm.__enter__()
        open_pools[name] = cm
        return pool

    def pclose(name):
        open_pools.pop(name).__exit__(None, None, None)

    # ---- constants (live whole kernel) ----
    consts = popen("consts", bufs=1)

    pending_cdma = []

    def cdma(name, shape, dt):
        t = consts.tile(shape, dt, tag=f"c_{name}", name=f"c_{name}")
        pending_cdma.append((t, name))
        return t

    c_bq = cdma("bqp", [128, NCH], F32)
    c_bk = cdma("bkp", [128, NCH], F32)
    c_bv = cdma("bvS", [1, D], BF16)
    c_mu = cdma("masku", [9, 128], BF16)
    c_mw = cdma("maskw", [9, 512], BF16)
    c_or = cdma("onesrow", [1, 128], BF16)
    c_oc = cdma("onescol", [128, 1], BF16)
    c_so = cdma("sorow", [1, 128], BF16)
    c_eps1 = consts.tile([1, 1], F32, name="c_eps1")
    nc.vector.memset(c_eps1, EPS * CS1 * CS1)
    c_eps2 = consts.tile([1, 1], F32, name="c_eps2")
    nc.vector.memset(c_eps2, EPS)


    # attention output (fp8, full residency)
    osp = popen("osp", bufs=1)
    os8 = osp.tile([128, NCH, T], FP8, name="os8")

    # macro34 weights minus w2: space reserved up-front so their DMAs can
    # stream during macro12 instead of waiting for its pools to die.
    w1p = popen("w1p", bufs=1)
    wo_sb = w1p.tile([128, NCH, D], FP8, name="wo_sb")
    w1h_sb = w1p.tile([128, NCH, FF], FP8, name="w1h_sb")
    w1l_sb = w1p.tile([128, NCH, FF], FP8, name="w1l_sb")

    wqp = popen("wqp", bufs=1)
    wq_sb = wqp.tile([128, NCH, D], FP8, name="wq_sb")
    nc.sync.dma_start(out=wq_sb, in_=dram["wq8"][:, :, :])
    early = [p for p in pending_cdma if p[1] in ("bqp", "bkp")]
    for (t, name) in early:
        nc.sync.dma_start(out=t, in_=dram[name][:, :])
        pending_cdma.remove((t, name))
    wk_sb = wqp.tile([128, NCH, D], FP8, name="wk_sb")
    nc.sync.dma_start(out=wk_sb, in_=dram["wk8"][:, :, :])
    wv_sb = wqp.tile([128, NCH, D], FP8, name="wv_sb")
    nc.sync.dma_start(out=wv_sb, in_=dram["wv8"][:, :, :])
    for (t, name) in pending_cdma:
        nc.sync.dma_start(out=t, in_=dram[name][:, :])
    pending_cdma.clear()
    nc.sync.dma_start(out=wo_sb, in_=dram["wo8"][:, :, :])
    for q in range(4):
        nc.sync.dma_start(out=w1h_sb[:, 2 * q:2 * q + 2, :],
                          in_=dram["w1h"][:, 2 * q:2 * q + 2, :])
        nc.sync.dma_start(out=w1l_sb[:, 2 * q:2 * q + 2, :],
                          in_=dram["w1l"][:, 2 * q:2 * q + 2, :])

    # ================= macro 1+2: QKV + attention =================
    pa = popen("pa", bufs=2)
    psa = popen("psa", bufs=1, space="PSUM")

    def emit_qkv(s):
        tsl = ts(s, SA)
        x8t = pa.tile([128, NCH, SA], FP8, tag="x8t")
        nc.scalar.dma_start(out=x8t, in_=dram["x8"][:, :, tsl])

        qst = pa.tile([128, NCH, SA], BF16, tag="qst")
        kst = pa.tile([128, NCH, SA], BF16, tag="kst")
        for (w_sb, cbias, dst, on_act) in ((wq_sb, c_bq, qst, True),
                                           (wk_sb, c_bk, kst, False)):
            for h in range(H):
                ps = psa.tile([128, SA], F32, tag="sps", bufs=2, name="sps")
                for i in range(NCH // 2):
                    nc.tensor.matmul(ps, w_sb[:, 2 * i:2 * i + 2, ts(h, 128)],
                                     x8t[:, 2 * i:2 * i + 2, :],
                                     start=i == 0, stop=i == NCH // 2 - 1,
                                     perf_mode=DR)
                if on_act:
                    nc.scalar.activation(out=dst[:, h, :], in_=ps,
                                         func=AF.Identity,
                                         bias=cbias[:, h:h + 1],
                                         scale=1.0 / (SX * SW))
                else:
                    nc.vector.tensor_scalar(out=dst[:, h, :], in0=ps,
                                            scalar1=1.0 / (SX * SW),
                                            scalar2=cbias[:, h:h + 1],
                                            op0=OP.mult, op1=OP.add)

        # V: token-major out [tok, vout]
        vst = pa.tile([128, SA // 128, D], BF16, tag="vst")
        for tb in range(SA // 128):
            for nb in range(2):
                ps = psa.tile([128, 512], F32, tag="sps", bufs=2, name="spsv")
                nkp = NCH // 2
                for i in range(nkp):
                    last = (i == nkp - 1) and zbv
                    nc.tensor.matmul(ps, x8t[:, 2 * i:2 * i + 2, ts(tb, 128)],
                                     wv_sb[:, 2 * i:2 * i + 2, ts(nb, 512)],
                                     start=i == 0, stop=last, perf_mode=DR)
                if not zbv:
                    nc.tensor.matmul(ps, c_or, c_bv[:, ts(nb, 512)],
                                     start=False, stop=True)
                nc.scalar.activation(out=vst[:, tb, ts(nb, 512)], in_=ps,
                                     func=AF.Identity,
                                     scale=1.0 / (SX * SW))

        return qst, kst, vst

    def emit_attn(s, qkv):
        qst, kst, vst = qkv
        # attention per 128-token block (8 windows each, block-diagonal)
        for tb in range(SA // 128):
            et_ps = psa.tile([128, H, 128], F32, tag="att_ps", bufs=2,
                             name="et_ps")
            ets = pa.tile([128, H, 128], BF16, tag="ets", bufs=3)
            for hf in range(2):
                nc.tensor.matmul(et_ps[:, 4 * hf:4 * hf + 4, :],
                                 c_mu, c_mw, start=True, stop=False,
                                 skip_group_check=True)
                for hh in range(4):
                    h = 4 * hf + hh
                    nc.tensor.matmul(
                        et_ps[:, h, :],
                        kst[:, h, ts(tb, 128)], qst[:, h, ts(tb, 128)],
                        start=False, stop=hh == 3, skip_group_check=True)
                nc.scalar.activation(out=ets[:, 4 * hf:4 * hf + 4, :],
                                     in_=et_ps[:, 4 * hf:4 * hf + 4, :],
                                     func=AF.Exp, scale=ISQ)
            rb_ps = psa.tile([128, 2, 4, 128], F32, tag="rb_ps", bufs=1,
                             name="rb_ps")
            # z rows live on partition 0 of rb_ps until the broadcast matmul
            # overwrites them (tile deps serialize reciprocal before it).
            for hf in range(2):
                nc.tensor.matmul(rb_ps[0:1, hf, :, :], c_oc,
                                 ets[:, 4 * hf:4 * hf + 4, :],
                                 start=True, stop=True)
            rz = pa.tile([1, 2, 4, 128], BF16, tag="rz", bufs=3)
            with nc.allow_low_precision(reason="softmax renorm row in bf16"):
                nc.vector.reciprocal(out=rz, in_=rb_ps[0:1, :, :, :])
            for hf in range(2):
                nc.tensor.matmul(rb_ps[:, hf, :, :], c_so, rz[:, hf, :, :],
                                 start=True, stop=True)
            rbs = pa.tile([128, 2, 4, 128], BF16, tag="rbs", bufs=2)
            nc.scalar.copy(out=rbs, in_=rb_ps)
            o_ps = psa.tile([128, H, 128], F32, tag="att_ps", bufs=2,
                            name="o_ps")
            for h in range(H):
                nc.tensor.matmul(o_ps[:, h, :], vst[:, tb, ts(h, 128)],
                                 ets[:, h, :], start=h % 4 == 0,
                                 stop=h % 4 == 3)
            tok = ts(s * (SA // 128) + tb, 128)
            for hf in range(2):
                nc.vector.tensor_mul(
                    out=os8[:, 4 * hf:4 * hf + 4, tok],
                    in0=o_ps[:, 4 * hf:4 * hf + 4, :],
                    in1=rbs[:, hf, :, :])

    qkv_live = None
    for s in range(NSA):
        qkv_now = emit_qkv(s)
        if qkv_live is not None:
            emit_attn(s - 1, qkv_live)
        qkv_live = qkv_now
    emit_attn(NSA - 1, qkv_live)

    pclose("psa")
    pclose("pa")
    pclose("wqp")

    # ================= macro 3+4 weights (w2 streams at the boundary) ====
    w2p = popen("w2p", bufs=1)
    w2_sb = w2p.tile([128, NHC, D], BF16, name="w2_sb")
    for q in range(8):
        nc.sync.dma_start(out=w2_sb[:, 4 * q:4 * q + 4, :],
                          in_=dram["w2b"][:, 4 * q:4 * q + 4, :])
    c_b1 = consts.tile([128, NHC], F32, name="c_b1")
    nc.sync.dma_start(out=c_b1, in_=dram["b1p"][:, :])
    c_bo = cdma("boS", [128, NCH], F32)
    c_bb2 = cdma("bb2p", [128, NCH], F32)
    c_g1 = cdma("g1p", [128, NCH], F32)
    c_g2 = cdma("g2p", [128, NCH], F32)
    c_be2 = cdma("be2p", [128, NCH], F32)
    for (t, name) in pending_cdma:
        nc.sync.dma_start(out=t, in_=dram[name][:, :])
    pending_cdma.clear()

    pb = popen("pb", bufs=2)
    psb = popen("psb", bufs=1, space="PSUM")

    def half_ps(nm):
        return psb.tile([128, 4, SB], F32, tag="half_ps", bufs=3, name=nm)

    def emit_oproj(s):
        tsl = ts(s, SB)
        xst = pb.tile([128, NCH, SB], BF16, tag="xst", bufs=1)
        nc.scalar.dma_start(out=xst, in_=dram["xs"][:, :, tsl])

        hpre = pb.tile([128, NCH, SB], BF16, tag="hpre", bufs=1)
        for half in range(2):
            po = half_ps("po")
            for dq in range(4):
                dc = 4 * half + dq
                for i in range(NCH // 2):
                    nc.tensor.matmul(po[:, dq, :],
                                     wo_sb[:, 2 * i:2 * i + 2, ts(dc, 128)],
                                     os8[:, 2 * i:2 * i + 2, tsl],
                                     start=dq % 2 == 0 and i == 0,
                                     stop=dq % 2 == 1 and i == NCH // 2 - 1,
                                     perf_mode=DR)
            sl = slice(4 * half, 4 * half + 4)
            if zbo:
                nc.vector.tensor_add(out=hpre[:, sl, :], in0=po,
                                     in1=xst[:, sl, :])
            else:
                for dq in range(4):
                    dc = 4 * half + dq
                    nc.vector.scalar_tensor_tensor(
                        out=hpre[:, dc, :], in0=po[:, dq, :],
                        scalar=c_bo[:, dc:dc + 1], in1=xst[:, dc, :],
                        op0=OP.add, op1=OP.add)
        sq = pb.tile([128, NCH, SB], BF16, tag="sq", bufs=1)
        nc.scalar.activation(out=sq, in_=hpre, func=AF.Square)
        return {"hpre": hpre, "sq": sq}

    def emit_ln1(s, st):
        hpre, sq = st["hpre"], st["sq"]
        st_ps = psb.tile([1, 2, SB], F32, tag="st_ps", bufs=1, name="st_ps")
        for dc in range(NCH):
            nc.tensor.matmul(st_ps[:, 0, :], c_oc, hpre[:, dc, :],
                             start=dc == 0, stop=False)
            nc.tensor.matmul(st_ps[:, 1, :], c_oc, sq[:, dc, :],
                             start=False, stop=dc == NCH - 1)
        rbm = _ln_stats(nc, pb, st_ps, c_eps1, SB, "")
        bc_ps = psb.tile([128, 2, SB], F32, tag="bc_ps", bufs=1, name="bc_ps")
        nc.tensor.matmul(bc_ps[:, :, :], c_or, rbm, start=True, stop=True)
        bcs = pb.tile([128, 2, SB], BF16, tag="bcs", bufs=1)
        nc.scalar.copy(out=bcs, in_=bc_ps)
        ys = pb.tile([128, NCH, SB], BF16, tag="ys", bufs=2, name="ys")
        for dc in range(NCH):
            nc.vector.tensor_mul(out=ys[:, dc, :], in0=hpre[:, dc, :],
                                 in1=bcs[:, 0, :])
        for dc in range(NCH):
            nc.vector.tensor_sub(out=ys[:, dc, :], in0=ys[:, dc, :],
                                 in1=bcs[:, 1, :])
        y8 = pb.tile([128, NCH, SB], FP8, tag="y8", bufs=1)
        nc.scalar.activation(out=y8, in_=ys, func=AF.Identity, scale=SH)
        y8l = pb.tile([128, NCH, SB], FP8, tag="y8l", bufs=1)
        nc.vector.scalar_tensor_tensor(out=y8l, in0=ys, scalar=SH, in1=y8,
                                       op0=OP.mult, op1=OP.subtract)
        st.update(ys=ys, y8=y8, y8l=y8l)

    def emit_w1(s, st):
        y8, y8l = st["y8"], st["y8l"]
        hid = pb.tile([128, NHC, SB], BF16, tag="hid", bufs=1)
        for g in range(NHC // 4):
            w1ps = half_ps("w1ps")
            for hh in range(4):
                hc = 4 * g + hh
                first = hh % 2 == 0
                for (xa, wa) in ((y8, w1h_sb), (y8l, w1h_sb), (y8, w1l_sb)):
                    for i in range(NCH // 2):
                        nc.tensor.matmul(
                            w1ps[:, hh, :],
                            wa[:, 2 * i:2 * i + 2, ts(hc, 128)],
                            xa[:, 2 * i:2 * i + 2, :],
                            start=first,
                            stop=(hh % 2 == 1 and xa is y8 and wa is w1l_sb
                                  and i == NCH // 2 - 1),
                            perf_mode=DR)
                        first = False
            if zb1:
                nc.scalar.activation(out=hid[:, 4 * g:4 * g + 4, :],
                                     in_=w1ps, func=AF.Gelu,
                                     scale=1.0 / (SH * SW))
            else:
                for hh in range(4):
                    hc = 4 * g + hh
                    nc.scalar.activation(out=hid[:, hc, :],
                                         in_=w1ps[:, hh, :], func=AF.Gelu,
                                         bias=c_b1[:, hc:hc + 1],
                                         scale=1.0 / (SH * SW))
        st["hid"] = hid

    def emit_w2_half(s, st, half):
        hid, ys = st["hid"], st["ys"]
        if half == 0:
            st["h2"] = pb.tile([128, NCH, SB], BF16, tag="h2", bufs=2, name="h2")
        h2 = st["h2"]
        w2ps = half_ps("w2ps")
        for dq in range(4):
            dc = 4 * half + dq
            for hc in range(NHC):
                nc.tensor.matmul(w2ps[:, dq, :],
                                 w2_sb[:, hc, ts(dc, 128)],
                                 hid[:, hc, :],
                                 start=dq % 2 == 0 and hc == 0,
                                 stop=dq % 2 == 1 and hc == NHC - 1)
        sl = slice(4 * half, 4 * half + 4)
        if zb2a1:
            nc.vector.tensor_add(out=h2[:, sl, :], in0=w2ps,
                                 in1=ys[:, sl, :])
        else:
            for dq in range(4):
                dc = 4 * half + dq
                yg = pb.tile([128, SB], BF16, tag="yg", bufs=2)
                nc.vector.tensor_scalar(out=yg, in0=ys[:, dc, :],
                                        scalar1=c_g1[:, dc:dc + 1],
                                        scalar2=c_bb2[:, dc:dc + 1],
                                        op0=OP.mult, op1=OP.add)
                nc.vector.tensor_add(out=h2[:, dc, :], in0=w2ps[:, dq, :],
                                     in1=yg)
        if half == 1:
            sq2 = pb.tile([128, NCH, SB], BF16, tag="sq", bufs=1, name="sq2")
            nc.scalar.activation(out=sq2, in_=h2, func=AF.Square)
            st["sq2"] = sq2

    def emit_ln2(s, st):
        h2, sq2 = st["h2"], st["sq2"]
        tsl = ts(s, SB)
        st2_ps = psb.tile([1, 2, SB], F32, tag="st_ps", bufs=1, name="st2_ps")
        for dc in range(NCH):
            nc.tensor.matmul(st2_ps[:, 0, :], c_oc, h2[:, dc, :],
                             start=dc == 0, stop=False)
            nc.tensor.matmul(st2_ps[:, 1, :], c_oc, sq2[:, dc, :],
                             start=False, stop=dc == NCH - 1)
        rbm2 = _ln_stats(nc, pb, st2_ps, c_eps2, SB, "")
        bc2_ps = psb.tile([128, 2, SB], F32, tag="bc_ps", bufs=1,
                          name="bc2_ps")
        nc.tensor.matmul(bc2_ps[:, :, :], c_or, rbm2, start=True, stop=True)
        bc2s = pb.tile([128, 2, SB], BF16, tag="bcs", bufs=1, name="bc2s")
        nc.scalar.copy(out=bc2s, in_=bc2_ps)
        yout = pb.tile([128, NCH, SB], BF16, tag="xst", bufs=1, name="yout")
        for dc in range(NCH):
            nc.vector.tensor_mul(out=yout[:, dc, :], in0=h2[:, dc, :],
                                 in1=bc2s[:, 0, :])
        for dc in range(NCH):
            nc.vector.tensor_sub(out=yout[:, dc, :], in0=yout[:, dc, :],
                                 in1=bc2s[:, 1, :])
            if not za2:
                nc.vector.tensor_scalar(out=yout[:, dc, :],
                                        in0=yout[:, dc, :],
                                        scalar1=c_g2[:, dc:dc + 1],
                                        scalar2=c_be2[:, dc:dc + 1],
                                        op0=OP.mult, op1=OP.add)
        nc.scalar.dma_start(out=dram["outb"][:, :, tsl], in_=yout)

    states = {0: emit_oproj(0)}
    emit_ln1(0, states[0])
    states[1] = emit_oproj(1)
    for s in range(NSB):
        st = states[s]
        emit_w1(s, st)
        if s + 1 < NSB and s + 1 not in states:
            states[s + 1] = emit_oproj(s + 1)
        emit_w2_half(s, st, 0)
        if s >= 1:
            emit_ln2(s - 1, states.pop(s - 1))
        if s + 1 < NSB:
            emit_ln1(s + 1, states[s + 1])
        emit_w2_half(s, st, 1)
    emit_ln2(NSB - 1, states.pop(NSB - 1))

    pclose("psb")
    pclose("pb")
    pclose("w2p")
    pclose("w1p")
    pclose("osp")
    pclose("consts")


def _ln_stats(nc, pool, st_ps, eps_t, TW, tag):
    """stat psum [1, 2, TW] (sum, sumsq) -> rstd, bm rows (bf16)."""
    mean = pool.tile([1, TW], BF16, tag=f"mean{tag}", bufs=1)
    nc.scalar.activation(out=mean, in_=st_ps[:, 0, :], func=AF.Identity,
                         scale=1.0 / D)
    msq = pool.tile([1, TW], BF16, tag=f"msq{tag}", bufs=1)
    nc.vector.tensor_mul(out=msq, in0=mean, in1=mean)
    var = pool.tile([1, TW], BF16, tag=f"var{tag}", bufs=1)
    nc.vector.scalar_tensor_tensor(out=var, in0=st_ps[:, 1, :],
                                   scalar=1.0 / D, in1=msq,
                                   op0=OP.mult, op1=OP.subtract)
    sd = pool.tile([1, TW], BF16, tag=f"msq{tag}", bufs=1, name="sd")
    nc.scalar.activation(out=sd, in_=var, func=AF.Sqrt, bias=eps_t, scale=1.0)
    rbm = pool.tile([1, 2, TW], BF16, tag=f"rbm{tag}", bufs=1, name="rbm")
    with nc.allow_low_precision(reason="LN broadcast rows in bf16"):
        nc.vector.reciprocal(out=rbm[:, 0, :], in_=sd)
    nc.vector.tensor_mul(out=rbm[:, 1, :], in0=mean, in1=rbm[:, 0, :])
    return rbm


# ======================= host side =======================

def _prep_shared(w_qkv, b_qkv, w_out, b_out, w1, b1, w2, b2,
                 g1, beta1, g2, beta2):
    wq, wk, wv = w_qkv[0:D], w_qkv[D:2 * D], w_qkv[2 * D:3 * D]
    bq, bk, bv = b_qkv[0:D], b_qkv[D:2 * D], b_qkv[2 * D:3 * D]

    def pmaj(v, n):
        return np.ascontiguousarray(
            np.asarray(v, np.float32).reshape(n, 128).T)

    def chunk8(wT, nk):
        # [K, M] -> [128, nk, M] (K = nk*128, chunk-major along K)
        return np.ascontiguousarray(
            wT.reshape(nk, 128, wT.shape[1]).transpose(1, 0, 2))

    wqT = np.ascontiguousarray(np.asarray(wq, np.float32).T)
    wkT = np.ascontiguousarray(np.asarray(wk, np.float32).T)
    wvT = np.ascontiguousarray(np.asarray(wv, np.float32).T)
    woT = np.ascontiguousarray(np.asarray(w_out, np.float32).T)
    w1g = np.asarray(w1, np.float32) * np.asarray(g1, np.float32)[None, :]
    w1T = np.ascontiguousarray(w1g.T)          # [D, FF]
    w2T = np.ascontiguousarray(np.asarray(w2, np.float32).T)  # [FF, D]

    w1s = w1T * SW
    w1hT = w1s.astype(E4)
    w1lT = (w1s - w1hT.astype(np.float32)).astype(E4)

    b1f = np.asarray(b1, np.float32) + w1g @ np.asarray(beta1, np.float32)
    bb2 = np.asarray(b2, np.float32) + np.asarray(beta1, np.float32)

    mu = np.zeros((9, 128), np.float32)
    mw = np.zeros((9, 128), np.float32)
    for w in range(8):
        mu[w, w * 16:(w + 1) * 16] = MASKC
        mw[w, w * 16:(w + 1) * 16] = 1.0
    mu[8, :] = -MASKC
    mw[8, :] = 1.0
    mwr = np.tile(mw, (1, 4))

    shared = {
        "wq8": chunk8(wqT * SW, NCH).astype(E4),
        "wk8": chunk8(wkT * SW, NCH).astype(E4),
        "wv8": chunk8(wvT * SW, NCH).astype(E4),
        "wo8": chunk8(woT * SW, NCH).astype(E4),
        "w1h": chunk8(w1hT.astype(np.float32), NCH).astype(E4),
        "w1l": chunk8(w1lT.astype(np.float32), NCH).astype(E4),
        "w2b": chunk8(w2T, NHC).astype(BF),
        "bqp": pmaj(bq, NCH), "bkp": pmaj(bk, NCH),
        "bvS": (np.asarray(bv, np.float32) * SX * SW).reshape(1, D).astype(BF),
        "b1p": pmaj(b1f, NHC),
        "boS": pmaj(np.asarray(b_out, np.float32) * CS1, NCH),
        "bb2p": pmaj(bb2, NCH),
        "g1p": pmaj(g1, NCH),
        "g2p": pmaj(g2, NCH), "be2p": pmaj(beta2, NCH),
        "masku": mu.astype(BF), "maskw": mwr.astype(BF),
        "onesrow": np.ones((1, 128), np.float32).astype(BF),
        "onescol": np.ones((128, 1), np.float32).astype(BF),
        "sorow": np.full((1, 128), SO, np.float32).astype(BF),
    }
    flags = (
        bool(np.all(np.asarray(bv) == 0)),                       # zbv
        bool(np.all(b1f == 0)),                                  # zb1
        bool(np.all(np.asarray(b_out) == 0)),                    # zbo
        bool(np.all(bb2 == 0)
             and np.all(np.asarray(g1, np.float32) == 1.0)),     # zb2a1
        bool(np.all(np.asarray(beta2) == 0)
             and np.all(np.asarray(g2, np.float32) == 1.0)),     # za2
    )
    return shared, flags


def make_in_maps(inputs):
    ff = np.asarray(inputs["frame_features"], np.float32)
    fi = np.asarray(inputs["frame_indices"])
    shared, flags = _prep_shared(
        np.asarray(inputs["w_qkv"]), np.asarray(inputs["b_qkv"]),
        np.asarray(inputs["w_out"]), np.asarray(inputs["b_out"]),
        np.asarray(inputs["w1"]), np.asarray(inputs["b1"]),
        np.asarray(inputs["w2"]), np.asarray(inputs["b2"]),
        np.asarray(inputs["g1"]), np.asarray(inputs["beta1"]),
        np.asarray(inputs["g2"]), np.asarray(inputs["beta2"]))

    div = np.exp(np.float32(-np.log(10000.0))
                 * np.arange(0, D, 2, dtype=np.float32) / np.float32(D))
    in_maps = []
    for b in range(B):
        pos = np.asarray(fi[b], np.float32)[:, None]
        ang = pos * div[None, :]
        pe = np.empty((T, D), np.float32)
        pe[:, 0::2] = np.sin(ang)
        pe[:, 1::2] = np.cos(ang)
        xpe = ff[b] + pe                       # [T, D]
        xpeT = np.ascontiguousarray(xpe.T)     # [D, T]
        x8 = np.ascontiguousarray(
            (xpeT * SX).reshape(NCH, 128, T).transpose(1, 0, 2)).astype(E4)
        xsc = np.ascontiguousarray(
            (xpeT * CS1).reshape(NCH, 128, T).transpose(1, 0, 2)).astype(BF)
        m = dict(shared)
        m["x8"] = x8
        m["xs"] = xsc
        in_maps.append(m)
    return in_maps, flags


def get_nc(flags=(True, True, True, True, True)):
    if flags not in _NC_CACHE:
        _NC_CACHE[flags] = build_nc(flags)
    return _NC_CACHE[flags]


def kernel(**inputs) -> np.ndarray:
    in_maps, flags = make_in_maps(inputs)
    nc = get_nc(flags)
    res = run_bass_kernel_spmd(nc, in_maps, core_ids=list(range(B)))
    outs = []
    for r in res.results:
        ob = np.asarray(r["outb"])             # [128, NCH, T] bf16
        oT = ob.transpose(1, 0, 2).reshape(D, T)
        outs.append(oT.T.astype(np.float32))
    return np.ascontiguousarray(np.stack(outs))



# revision 7
# speedup vs baseline: 1.1449x; 1.1449x over previous
"""Trainium2 Bass kernel for a local-window-attention transformer block (v3).

Sharding: data-parallel over batch (one batch element per NeuronCore).

v3 vs v2:
- fp16 replaces bf16 everywhere on the residual/attention path (same engine
  cost, ~8x finer mantissa), which buys back enough accuracy budget to cut
  tensor-engine work:
- w1 runs as fp8 "x2b" (y8/y8l hi-lo activations against a single fp8 w1h;
  no w1l tensor), w2 as fp8 "x3" (h8/h8l hi-lo against w2 hi + h8 against
  w2 lo). Both were 1.0-cycle/row bf16 or x3 fp8 before; net PE work in the
  FFN phase drops ~30%.
- attention output is kept in fp16 (no fp8 quantization), and the out
  projection runs as a plain fp16 matmul with unscaled fp16 weights; the
  residual chain is scale-free (no CS1 prescaling).
- w2h streams during the attention phase (SBUF freed by dropping w1l).
"""
import numpy as np
import ml_dtypes

import concourse.bass as bass
import concourse.bacc as bacc
import concourse.mybir as mybir
import concourse.tile as tile
from concourse.bass import ts
from concourse.bass_utils import run_bass_kernel_spmd

F32 = mybir.dt.float32
F16 = mybir.dt.float16
FP8 = mybir.dt.float8e4
AF = mybir.ActivationFunctionType
OP = mybir.AluOpType
DR = mybir.MatmulPerfMode.DoubleRow
NF = np.float16
E4 = ml_dtypes.float8_e4m3

B, T, D, W, H = 8, 2048, 1024, 16, 8
HD = D // H            # 128 = head dim = one partition chunk
FF = 4 * D             # 4096
NCH = D // 128         # 8 feature chunks
NHC = FF // 128        # 32 hidden chunks
EPS = 1e-5
ISQ = float(1.0 / np.sqrt(128.0))
MASKC = 340.0

SX = 16.0              # x+pe fp8 scale
SW = 64.0              # weight fp8 scale
SH = 32.0              # LN1-out / gelu-out fp8 scale

SA = 512               # macro12 token slab
NSA = T // SA
SB = 256               # macro34 token slab
NSB = T // SB

_NC_CACHE = {}


def build_nc(flags):
    (zbv, zb1, zbo, zb2a1, za2) = flags
    nc = bacc.Bacc(None, target_bir_lowering=False)

    dram = {}
    # ---- per-core inputs ----
    dram["x8"] = nc.declare_dram_parameter("x8", [128, NCH, T], FP8,
                                           isOutput=False)
    dram["xs"] = nc.declare_dram_parameter("xs", [128, NCH, T], F16,
                                           isOutput=False)
    # ---- shared weights ----
    for nm, sh, dt in (
            ("wq8", [128, NCH, D], FP8), ("wk8", [128, NCH, D], FP8),
            ("wv8", [128, NCH, D], FP8), ("wo16", [128, NCH, D], F16),
            ("w1h", [128, NCH, FF], FP8),
            ("w2h", [128, NHC, D], FP8), ("w2l", [128, NHC, D], FP8),
            ("bqp", [128, NCH], F32), ("bkp", [128, NCH], F32),
            ("bvS", [1, D], F16), ("b1p", [128, NHC], F32),
            ("boS", [128, NCH], F32), ("bb2p", [128, NCH], F32),
            ("g1p", [128, NCH], F32), ("g2p", [128, NCH], F32),
            ("be2p", [128, NCH], F32),
            ("masku", [9, 128], F16), ("maskw", [9, 512], F16),
            ("onesrow", [1, 128], F16), ("onescol", [128, 1], F16)):
        dram[nm] = nc.declare_dram_parameter(nm, sh, dt, isOutput=False)

    dram["outb"] = nc.declare_dram_parameter("outb", [128, NCH, T], F16,
                                             isOutput=True)

    with tile.TileContext(nc) as tc:
        _emit(nc, tc, flags, dram)
    nc.compile()
    return nc


def _emit(nc, tc, flags, dram):
    (zbv, zb1, zbo, zb2a1, za2) = flags
    open_pools = {}

    def popen(name, **kw):
        cm = tc.tile_pool(name=name, **kw)
        pool = cm.__enter__()
        open_pools[name] = cm
        return pool

    def pclose(name):
        open_pools.pop(name).__exit__(None, None, None)

    # ---- constants (live whole kernel) ----
    consts = popen("consts", bufs=1)

    pending_cdma = []

    def cdma(name, shape, dt):
        t = consts.tile(shape, dt, tag=f"c_{name}", name=f"c_{name}")
        pending_cdma.append((t, name))
        return t

    c_bq = cdma("bqp", [128, NCH], F32)
    c_bk = cdma("bkp", [128, NCH], F32)
    c_bv = None if zbv else cdma("bvS", [1, D], F16)
    c_mu = cdma("masku", [9, 128], F16)
    c_mw = cdma("maskw", [9, 512], F16)
    c_or = cdma("onesrow", [1, 128], F16)
    c_oc = cdma("onescol", [128, 1], F16)
    c_eps = consts.tile([1, 1], F32, name="c_eps")
    nc.vector.memset(c_eps, EPS)

    # attention output (fp16, full residency)
    osp = popen("osp", bufs=1)
    os16 = osp.tile([128, NCH, T], F16, name="os16")

    # macro34 weights minus w2l: space reserved up-front so their DMAs can
    # stream during macro12 instead of waiting for its pools to die.
    w1p = popen("w1p", bufs=1)
    wo_sb = w1p.tile([128, NCH, D], F16, name="wo_sb")
    w1h_sb = w1p.tile([128, NCH, FF], FP8, name="w1h_sb")
    w2h_sb = w1p.tile([128, NHC, D], FP8, name="w2h_sb")

    wqp = popen("wqp", bufs=1)
    wq_sb = wqp.tile([128, NCH, D], FP8, name="wq_sb")
    nc.sync.dma_start(out=wq_sb, in_=dram["wq8"][:, :, :])
    early = [p for p in pending_cdma if p[1] in ("bqp", "bkp")]
    for (t, name) in early:
        nc.sync.dma_start(out=t, in_=dram[name][:, :])
        pending_cdma.remove((t, name))
    wk_sb = wqp.tile([128, NCH, D], FP8, name="wk_sb")
    nc.sync.dma_start(out=wk_sb, in_=dram["wk8"][:, :, :])
    wv_sb = wqp.tile([128, NCH, D], FP8, name="wv_sb")
    nc.sync.dma_start(out=wv_sb, in_=dram["wv8"][:, :, :])
    for (t, name) in pending_cdma:
        nc.sync.dma_start(out=t, in_=dram[name][:, :])
    pending_cdma.clear()
    nc.sync.dma_start(out=wo_sb, in_=dram["wo16"][:, :, :])
    for q in range(4):
        nc.sync.dma_start(out=w1h_sb[:, 2 * q:2 * q + 2, :],
                          in_=dram["w1h"][:, 2 * q:2 * q + 2, :])
    # stream w2h during macro12 (fills DMA idle; ready before macro34)
    for q in range(8):
        nc.sync.dma_start(out=w2h_sb[:, 4 * q:4 * q + 4, :],
                          in_=dram["w2h"][:, 4 * q:4 * q + 4, :])

    # ================= macro 1+2: QKV + attention =================
    pa = popen("pa", bufs=2)
    psa = popen("psa", bufs=1, space="PSUM")

    def emit_qkv(s):
        tsl = ts(s, SA)
        x8t = pa.tile([128, NCH, SA], FP8, tag="x8t")
        nc.scalar.dma_start(out=x8t, in_=dram["x8"][:, :, tsl])

        qst = pa.tile([128, NCH, SA], F16, tag="qst")
        kst = pa.tile([128, NCH, SA], F16, tag="kst")
        for (w_sb, cbias, dst, on_act) in ((wq_sb, c_bq, qst, True),
                                           (wk_sb, c_bk, kst, False)):
            for h in range(H):
                ps = psa.tile([128, SA], F32, tag="sps", bufs=2, name="sps")
                for i in range(NCH // 2):
                    nc.tensor.matmul(ps, w_sb[:, 2 * i:2 * i + 2, ts(h, 128)],
                                     x8t[:, 2 * i:2 * i + 2, :],
                                     start=i == 0, stop=i == NCH // 2 - 1,
                                     perf_mode=DR)
                if on_act:
                    nc.scalar.activation(out=dst[:, h, :], in_=ps,
                                         func=AF.Identity,
                                         bias=cbias[:, h:h + 1],
                                         scale=1.0 / (SX * SW))
                else:
                    nc.vector.tensor_scalar(out=dst[:, h, :], in0=ps,
                                            scalar1=1.0 / (SX * SW),
                                            scalar2=cbias[:, h:h + 1],
                                            op0=OP.mult, op1=OP.add)

        # V: token-major out [tok, vout]
        vst = pa.tile([128, SA // 128, D], F16, tag="vst")
        for tb in range(SA // 128):
            for nb in range(2):
                ps = psa.tile([128, 512], F32, tag="sps", bufs=2, name="spsv")
                nkp = NCH // 2
                for i in range(nkp):
                    last = (i == nkp - 1) and zbv
                    nc.tensor.matmul(ps, x8t[:, 2 * i:2 * i + 2, ts(tb, 128)],
                                     wv_sb[:, 2 * i:2 * i + 2, ts(nb, 512)],
                                     start=i == 0, stop=last, perf_mode=DR)
                if not zbv:
                    nc.tensor.matmul(ps, c_or, c_bv[:, ts(nb, 512)],
                                     start=False, stop=True)
                nc.vector.tensor_scalar(out=vst[:, tb, ts(nb, 512)],
                                        in0=ps, scalar1=1.0 / (SX * SW),
                                        scalar2=None, op0=OP.mult)

        return qst, kst, vst

    def emit_attn(s, qkv):
        qst, kst, vst = qkv
        # attention per 128-token block (8 windows each, block-diagonal)
        for tb in range(SA // 128):
            et_ps = psa.tile([128, H, 128], F32, tag="att_ps", bufs=2,
                             name="et_ps")
            ets = pa.tile([128, H, 128], F16, tag="ets", bufs=2)
            for hf in range(2):
                nc.tensor.matmul(et_ps[:, 4 * hf:4 * hf + 4, :],
                                 c_mu, c_mw, start=True, stop=False,
                                 skip_group_check=True)
                for hh in range(4):
                    h = 4 * hf + hh
                    nc.tensor.matmul(
                        et_ps[:, h, :],
                        kst[:, h, ts(tb, 128)], qst[:, h, ts(tb, 128)],
                        start=False, stop=hh == 3, skip_group_check=True)
                nc.scalar.activation(out=ets[:, 4 * hf:4 * hf + 4, :],
                                     in_=et_ps[:, 4 * hf:4 * hf + 4, :],
                                     func=AF.Exp, scale=ISQ)
            rb_ps = psa.tile([128, 2, 4, 128], F32, tag="rb_ps", bufs=1,
                             name="rb_ps")
            # z rows live on partition 0 of rb_ps until the broadcast matmul
            # overwrites them (tile deps serialize reciprocal before it).
            for hf in range(2):
                nc.tensor.matmul(rb_ps[0:1, hf, :, :], c_oc,
                                 ets[:, 4 * hf:4 * hf + 4, :],
                                 start=True, stop=True)
            rz = pa.tile([1, 2, 4, 128], F16, tag="rz", bufs=2)
            with nc.allow_low_precision(reason="softmax renorm row in fp16"):
                nc.vector.reciprocal(out=rz, in_=rb_ps[0:1, :, :, :])
            for hf in range(2):
                nc.tensor.matmul(rb_ps[:, hf, :, :], c_or, rz[:, hf, :, :],
                                 start=True, stop=True)
            rbs = pa.tile([128, 2, 4, 128], F16, tag="rbs", bufs=2)
            nc.scalar.copy(out=rbs, in_=rb_ps)
            o_ps = psa.tile([128, H, 128], F32, tag="att_ps", bufs=2,
                            name="o_ps")
            for h in range(H):
                nc.tensor.matmul(o_ps[:, h, :], vst[:, tb, ts(h, 128)],
                                 ets[:, h, :], start=h % 4 == 0,
                                 stop=h % 4 == 3)
            tok = ts(s * (SA // 128) + tb, 128)
            for hf in range(2):
                nc.vector.tensor_mul(
                    out=os16[:, 4 * hf:4 * hf + 4, tok],
                    in0=o_ps[:, 4 * hf:4 * hf + 4, :],
                    in1=rbs[:, hf, :, :])

    qkv_live = None
    for s in range(NSA):
        qkv_now = emit_qkv(s)
        if qkv_live is not None:
            emit_attn(s - 1, qkv_live)
        qkv_live = qkv_now
    emit_attn(NSA - 1, qkv_live)

    pclose("psa")
    pclose("pa")
    pclose("wqp")

    # ================= macro 3+4 (w2l streams at the boundary) ====
    w2p = popen("w2p", bufs=1)
    w2l_sb = w2p.tile([128, NHC, D], FP8, name="w2l_sb")
    for q in range(8):
        nc.sync.dma_start(out=w2l_sb[:, 4 * q:4 * q + 4, :],
                          in_=dram["w2l"][:, 4 * q:4 * q + 4, :])
    c_b1 = consts.tile([128, NHC], F32, name="c_b1")
    nc.sync.dma_start(out=c_b1, in_=dram["b1p"][:, :])
    c_bo = cdma("boS", [128, NCH], F32)
    c_bb2 = cdma("bb2p", [128, NCH], F32)
    c_g1 = cdma("g1p", [128, NCH], F32)
    c_g2 = cdma("g2p", [128, NCH], F32)
    c_be2 = cdma("be2p", [128, NCH], F32)
    for (t, name) in pending_cdma:
        nc.sync.dma_start(out=t, in_=dram[name][:, :])
    pending_cdma.clear()

    pb = popen("pb", bufs=2)
    psb = popen("psb", bufs=1, space="PSUM")

    def half_ps(nm):
        return psb.tile([128, 4, SB], F32, tag="half_ps", bufs=3, name=nm)

    def emit_oproj(s):
        tsl = ts(s, SB)
        xst = pb.tile([128, NCH, SB], F16, tag="xst", bufs=2)
        nc.scalar.dma_start(out=xst, in_=dram["xs"][:, :, tsl])

        hpre = pb.tile([128, NCH, SB], F16, tag="hpre", bufs=1)
        for half in range(2):
            po = half_ps("po")
            for dq in range(4):
                dc = 4 * half + dq
                for i in range(NCH):
                    nc.tensor.matmul(po[:, dq, :],
                                     wo_sb[:, i, ts(dc, 128)],
                                     os16[:, i, tsl],
                                     start=dq % 2 == 0 and i == 0,
                                     stop=dq % 2 == 1 and i == NCH - 1)
            sl = slice(4 * half, 4 * half + 4)
            if zbo:
                nc.vector.tensor_add(out=hpre[:, sl, :], in0=po,
                                     in1=xst[:, sl, :])
            else:
                for dq in range(4):
                    dc = 4 * half + dq
                    nc.vector.scalar_tensor_tensor(
                        out=hpre[:, dc, :], in0=po[:, dq, :],
                        scalar=c_bo[:, dc:dc + 1], in1=xst[:, dc, :],
                        op0=OP.add, op1=OP.add)
        return {"hpre": hpre}

    def emit_ln1(s, st):
        hpre = st["hpre"]
        sq = pb.tile([128, NCH, SB], F16, tag="sq", bufs=1)
        nc.scalar.activation(out=sq, in_=hpre, func=AF.Square)
        st_ps = psb.tile([1, 2, SB], F32, tag="st_ps", bufs=1, name="st_ps")
        for dc in range(NCH):
            nc.tensor.matmul(st_ps[:, 0, :], c_oc, hpre[:, dc, :],
                             start=dc == 0, stop=False)
            nc.tensor.matmul(st_ps[:, 1, :], c_oc, sq[:, dc, :],
                             start=False, stop=dc == NCH - 1)
        rbm = _ln_stats(nc, pb, st_ps, c_eps, SB, "")
        bc_ps = psb.tile([128, 2, SB], F32, tag="bc_ps", bufs=1, name="bc_ps")
        nc.tensor.matmul(bc_ps[:, :, :], c_or, rbm, start=True, stop=True)
        bcs = pb.tile([128, 2, SB], F16, tag="bcs", bufs=1)
        nc.scalar.copy(out=bcs, in_=bc_ps)
        ys = pb.tile([128, NCH, SB], F16, tag="ys", bufs=2, name="ys")
        for dc in range(NCH):
            nc.vector.tensor_mul(out=ys[:, dc, :], in0=hpre[:, dc, :],
                                 in1=bcs[:, 0, :])
        for dc in range(NCH):
            nc.vector.tensor_sub(out=ys[:, dc, :], in0=ys[:, dc, :],
                                 in1=bcs[:, 1, :])
        y8 = pb.tile([128, NCH, SB], FP8, tag="y8", bufs=1)
        nc.scalar.activation(out=y8, in_=ys, func=AF.Identity, scale=SH)
        y8l = pb.tile([128, NCH, SB], FP8, tag="y8l", bufs=1)
        nc.vector.scalar_tensor_tensor(out=y8l, in0=ys, scalar=SH, in1=y8,
                                       op0=OP.mult, op1=OP.subtract)
        st.update(ys=ys, y8=y8, y8l=y8l)

    def emit_w1(s, st):
        y8, y8l = st["y8"], st["y8l"]
        h8 = pb.tile([128, NHC, SB], FP8, tag="h8", bufs=1)
        h8l = pb.tile([128, NHC, SB], FP8, tag="h8l", bufs=1)
        for g in range(NHC // 4):
            w1ps = half_ps("w1ps")
            for hh in range(4):
                hc = 4 * g + hh
                first = hh % 2 == 0
                for xa in (y8, y8l):
                    for i in range(NCH // 2):
                        nc.tensor.matmul(
                            w1ps[:, hh, :],
                            w1h_sb[:, 2 * i:2 * i + 2, ts(hc, 128)],
                            xa[:, 2 * i:2 * i + 2, :],
                            start=first,
                            stop=(hh % 2 == 1 and xa is y8l
                                  and i == NCH // 2 - 1),
                            perf_mode=DR)
                        first = False
            gsl = slice(4 * g, 4 * g + 4)
            g16 = pb.tile([128, 4, SB], F16, tag="g16", bufs=2)
            if zb1:
                nc.scalar.activation(out=g16, in_=w1ps, func=AF.Gelu,
                                     scale=1.0 / (SH * SW))
            else:
                for hh in range(4):
                    hc = 4 * g + hh
                    nc.scalar.activation(out=g16[:, hh, :],
                                         in_=w1ps[:, hh, :], func=AF.Gelu,
                                         bias=c_b1[:, hc:hc + 1],
                                         scale=1.0 / (SH * SW))
            nc.vector.tensor_scalar(out=h8[:, gsl, :], in0=g16,
                                    scalar1=SH, scalar2=None, op0=OP.mult)
            nc.vector.scalar_tensor_tensor(out=h8l[:, gsl, :], in0=g16,
                                           scalar=SH, in1=h8[:, gsl, :],
                                           op0=OP.mult, op1=OP.subtract)
        st["h8"] = h8
        st["h8l"] = h8l

    def emit_w2_half(s, st, half):
        h8, h8l, ys = st["h8"], st["h8l"], st["ys"]
        if half == 0:
            st["h2"] = pb.tile([128, NCH, SB], F16, tag="h2", bufs=2,
                               name="h2")
        h2 = st["h2"]
        w2ps = half_ps("w2ps")
        for dq in range(4):
            dc = 4 * half + dq
            first = dq % 2 == 0
            for (xa, wa) in ((h8, w2h_sb), (h8l, w2h_sb), (h8, w2l_sb)):
                for i in range(NHC // 2):
                    nc.tensor.matmul(
                        w2ps[:, dq, :],
                        wa[:, 2 * i:2 * i + 2, ts(dc, 128)],
                        xa[:, 2 * i:2 * i + 2, :],
                        start=first,
                        stop=(dq % 2 == 1 and xa is h8 and wa is w2l_sb
                              and i == NHC // 2 - 1),
                        perf_mode=DR)
                    first = False
        sl = slice(4 * half, 4 * half + 4)
        if zb2a1:
            nc.vector.scalar_tensor_tensor(
                out=h2[:, sl, :], in0=w2ps, scalar=1.0 / (SH * SW),
                in1=ys[:, sl, :], op0=OP.mult, op1=OP.add)
        else:
            for dq in range(4):
                dc = 4 * half + dq
                yg = pb.tile([128, SB], F16, tag="yg", bufs=2)
                nc.vector.tensor_scalar(out=yg, in0=ys[:, dc, :],
                                        scalar1=c_g1[:, dc:dc + 1],
                                        scalar2=c_bb2[:, dc:dc + 1],
                                        op0=OP.mult, op1=OP.add)
                nc.vector.scalar_tensor_tensor(
                    out=h2[:, dc, :], in0=w2ps[:, dq, :],
                    scalar=1.0 / (SH * SW), in1=yg,
                    op0=OP.mult, op1=OP.add)


    def emit_ln2(s, st):
        h2 = st["h2"]
        sq2 = pb.tile([128, NCH, SB], F16, tag="sq", bufs=1, name="sq2")
        nc.scalar.activation(out=sq2, in_=h2, func=AF.Square)
        tsl = ts(s, SB)
        st2_ps = psb.tile([1, 2, SB], F32, tag="st_ps", bufs=1, name="st2_ps")
        for dc in range(NCH):
            nc.tensor.matmul(st2_ps[:, 0, :], c_oc, h2[:, dc, :],
                             start=dc == 0, stop=False)
            nc.tensor.matmul(st2_ps[:, 1, :], c_oc, sq2[:, dc, :],
                             start=False, stop=dc == NCH - 1)
        rbm2 = _ln_stats(nc, pb, st2_ps, c_eps, SB, "")
        bc2_ps = psb.tile([128, 2, SB], F32, tag="bc_ps", bufs=1,
                          name="bc2_ps")
        nc.tensor.matmul(bc2_ps[:, :, :], c_or, rbm2, start=True, stop=True)
        bc2s = pb.tile([128, 2, SB], F16, tag="bcs", bufs=1, name="bc2s")
        nc.scalar.copy(out=bc2s, in_=bc2_ps)
        yout = pb.tile([128, NCH, SB], F16, tag="xst", bufs=2, name="yout")
        for dc in range(NCH):
            nc.vector.tensor_mul(out=yout[:, dc, :], in0=h2[:, dc, :],
                                 in1=bc2s[:, 0, :])
        for dc in range(NCH):
            nc.vector.tensor_sub(out=yout[:, dc, :], in0=yout[:, dc, :],
                                 in1=bc2s[:, 1, :])
            if not za2:
                nc.vector.tensor_scalar(out=yout[:, dc, :],
                                        in0=yout[:, dc, :],
                                        scalar1=c_g2[:, dc:dc + 1],
                                        scalar2=c_be2[:, dc:dc + 1],
                                        op0=OP.mult, op1=OP.add)
        nc.scalar.dma_start(out=dram["outb"][:, :, tsl], in_=yout)

    states = {0: emit_oproj(0)}
    emit_ln1(0, states[0])
    for s in range(NSB):
        st = states[s]
        if s + 1 < NSB:
            states[s + 1] = emit_oproj(s + 1)
        emit_w1(s, st)
        if s + 1 < NSB:
            emit_ln1(s + 1, states[s + 1])
        emit_w2_half(s, st, 0)
        emit_w2_half(s, st, 1)
        if s >= 1:
            emit_ln2(s - 1, states.pop(s - 1))
    emit_ln2(NSB - 1, states.pop(NSB - 1))

    pclose("psb")
    pclose("pb")
    pclose("w2p")
    pclose("w1p")
    pclose("osp")
    pclose("consts")


def _ln_stats(nc, pool, st_ps, eps_t, TW, tag):
    """stat psum [1, 2, TW] (sum, sumsq) -> rstd, bm rows (fp16)."""
    mean = pool.tile([1, TW], F16, tag=f"mean{tag}", bufs=1)
    nc.scalar.activation(out=mean, in_=st_ps[:, 0, :], func=AF.Identity,
                         scale=1.0 / D)
    msq = pool.tile([1, TW], F16, tag=f"msq{tag}", bufs=1)
    nc.vector.tensor_mul(out=msq, in0=mean, in1=mean)
    var = pool.tile([1, TW], F16, tag=f"var{tag}", bufs=1)
    nc.vector.scalar_tensor_tensor(out=var, in0=st_ps[:, 1, :],
                                   scalar=1.0 / D, in1=msq,
                                   op0=OP.mult, op1=OP.subtract)
    sd = pool.tile([1, TW], F16, tag=f"msq{tag}", bufs=1, name="sd")
    nc.scalar.activation(out=sd, in_=var, func=AF.Sqrt, bias=eps_t, scale=1.0)
    rbm = pool.tile([1, 2, TW], F16, tag=f"rbm{tag}", bufs=1, name="rbm")
    with nc.allow_low_precision(reason="LN broadcast rows in fp16"):
        nc.vector.reciprocal(out=rbm[:, 0, :], in_=sd)
    nc.vector.tensor_mul(out=rbm[:, 1, :], in0=mean, in1=rbm[:, 0, :])
    return rbm


# ======================= host side =======================

def _prep_shared(w_qkv, b_qkv, w_out, b_out, w1, b1, w2, b2,
                 g1, beta1, g2, beta2):
    wq, wk, wv = w_qkv[0:D], w_qkv[D:2 * D], w_qkv[2 * D:3 * D]
    bq, bk, bv = b_qkv[0:D], b_qkv[D:2 * D], b_qkv[2 * D:3 * D]

    def pmaj(v, n):
        return np.ascontiguousarray(
            np.asarray(v, np.float32).reshape(n, 128).T)

    def chunk8(wT, nk):
        # [K, M] -> [128, nk, M] (K = nk*128, chunk-major along K)
        return np.ascontiguousarray(
            wT.reshape(nk, 128, wT.shape[1]).transpose(1, 0, 2))

    wqT = np.ascontiguousarray(np.asarray(wq, np.float32).T)
    wkT = np.ascontiguousarray(np.asarray(wk, np.float32).T)
    wvT = np.ascontiguousarray(np.asarray(wv, np.float32).T)
    woT = np.ascontiguousarray(np.asarray(w_out, np.float32).T)
    w1g = np.asarray(w1, np.float32) * np.asarray(g1, np.float32)[None, :]
    w1T = np.ascontiguousarray(w1g.T)          # [D, FF]
    w2T = np.ascontiguousarray(np.asarray(w2, np.float32).T)  # [FF, D]

    w2s = w2T * SW
    w2hT = w2s.astype(E4)
    w2lT = (w2s - w2hT.astype(np.float32)).astype(E4)

    b1f = np.asarray(b1, np.float32) + w1g @ np.asarray(beta1, np.float32)
    bb2 = np.asarray(b2, np.float32) + np.asarray(beta1, np.float32)

    mu = np.zeros((9, 128), np.float32)
    mw = np.zeros((9, 128), np.float32)
    for w in range(8):
        mu[w, w * 16:(w + 1) * 16] = MASKC
        mw[w, w * 16:(w + 1) * 16] = 1.0
    mu[8, :] = -MASKC
    mw[8, :] = 1.0
    mwr = np.tile(mw, (1, 4))

    shared = {
        "wq8": chunk8(wqT * SW, NCH).astype(E4),
        "wk8": chunk8(wkT * SW, NCH).astype(E4),
        "wv8": chunk8(wvT * SW, NCH).astype(E4),
        "wo16": chunk8(woT, NCH).astype(NF),
        "w1h": chunk8(w1T * SW, NCH).astype(E4),
        "w2h": chunk8(w2hT.astype(np.float32), NHC).astype(E4),
        "w2l": chunk8(w2lT.astype(np.float32), NHC).astype(E4),
        "bqp": pmaj(bq, NCH), "bkp": pmaj(bk, NCH),
        "bvS": (np.asarray(bv, np.float32) * SX * SW).reshape(1, D).astype(NF),
        "b1p": pmaj(b1f, NHC),
        "boS": pmaj(np.asarray(b_out, np.float32), NCH),
        "bb2p": pmaj(bb2, NCH),
        "g1p": pmaj(g1, NCH),
        "g2p": pmaj(g2, NCH), "be2p": pmaj(beta2, NCH),
        "masku": mu.astype(NF), "maskw": mwr.astype(NF),
        "onesrow": np.ones((1, 128), np.float32).astype(NF),
        "onescol": np.ones((128, 1), np.float32).astype(NF),
    }
    flags = (
        bool(np.all(np.asarray(bv) == 0)),                       # zbv
        bool(np.all(b1f == 0)),                                  # zb1
        bool(np.all(np.asarray(b_out) == 0)),                    # zbo
        bool(np.all(bb2 == 0)
             and np.all(np.asarray(g1, np.float32) == 1.0)),     # zb2a1
        bool(np.all(np.asarray(beta2) == 0)
             and np.all(np.asarray(g2, np.float32) == 1.0)),     # za2
    )
    return shared, flags


def make_in_maps(inputs):
    ff = np.asarray(inputs["frame_features"], np.float32)
    fi = np.asarray(inputs["frame_indices"])
    shared, flags = _prep_shared(
        np.asarray(inputs["w_qkv"]), np.asarray(inputs["b_qkv"]),
        np.asarray(inputs["w_out"]), np.asarray(inputs["b_out"]),
        np.asarray(inputs["w1"]), np.asarray(inputs["b1"]),
        np.asarray(inputs["w2"]), np.asarray(inputs["b2"]),
        np.asarray(inputs["g1"]), np.asarray(inputs["beta1"]),
        np.asarray(inputs["g2"]), np.asarray(inputs["beta2"]))

    div = np.exp(np.float32(-np.log(10000.0))
                 * np.arange(0, D, 2, dtype=np.float32) / np.float32(D))
    in_maps = []
    for b in range(B):
        pos = np.asarray(fi[b], np.float32)[:, None]
        ang = pos * div[None, :]
        pe = np.empty((T, D), np.float32)
        pe[:, 0::2] = np.sin(ang)
        pe[:, 1::2] = np.cos(ang)
        xpe = ff[b] + pe                       # [T, D]
        xpeT = np.ascontiguousarray(xpe.T)     # [D, T]
        x8 = np.ascontiguousarray(
            (xpeT * SX).reshape(NCH, 128, T).transpose(1, 0, 2)).astype(E4)
        xsc = np.ascontiguousarray(
            xpeT.reshape(NCH, 128, T).transpose(1, 0, 2)).astype(NF)
        m = dict(shared)
        m["x8"] = x8
        m["xs"] = xsc
        in_maps.append(m)
    return in_maps, flags


def get_nc(flags=(True, True, True, True, True)):
    if flags not in _NC_CACHE:
        _NC_CACHE[flags] = build_nc(flags)
    return _NC_CACHE[flags]


def kernel(**inputs) -> np.ndarray:
    in_maps, flags = make_in_maps(inputs)
    nc = get_nc(flags)
    res = run_bass_kernel_spmd(nc, in_maps, core_ids=list(range(B)))
    outs = []
    for r in res.results:
        ob = np.asarray(r["outb"])             # [128, NCH, T] fp16
        oT = ob.transpose(1, 0, 2).reshape(D, T)
        outs.append(oT.T.astype(np.float32))
    return np.ascontiguousarray(np.stack(outs))


# revision 27
# speedup vs baseline: 1.2253x; 1.0702x over previous
"""Trainium2 Bass kernel for a local-window-attention transformer block (v3).

Sharding: data-parallel over batch (one batch element per NeuronCore).

v3 vs v2:
- fp16 replaces bf16 everywhere on the residual/attention path (same engine
  cost, ~8x finer mantissa), which buys back enough accuracy budget to cut
  tensor-engine work:
- w1 runs as fp8 "x2b" (y8/y8l hi-lo activations against a single fp8 w1h;
  no w1l tensor), w2 as fp8 "x3" (h8/h8l hi-lo against w2 hi + h8 against
  w2 lo). Both were 1.0-cycle/row bf16 or x3 fp8 before; net PE work in the
  FFN phase drops ~30%.
- attention output is kept in fp16 (no fp8 quantization), and the out
  projection runs as a plain fp16 matmul with unscaled fp16 weights; the
  residual chain is scale-free (no CS1 prescaling).
- w2h streams during the attention phase (SBUF freed by dropping w1l).
"""
import numpy as np
import ml_dtypes

import concourse.bass as bass
import concourse.bacc as bacc
import concourse.mybir as mybir
import concourse.tile as tile
from concourse.bass import ts
from concourse.bass_utils import run_bass_kernel_spmd

F32 = mybir.dt.float32
F16 = mybir.dt.float16
FP8 = mybir.dt.float8e4
AF = mybir.ActivationFunctionType
OP = mybir.AluOpType
DR = mybir.MatmulPerfMode.DoubleRow
NF = np.float16
E4 = ml_dtypes.float8_e4m3

B, T, D, W, H = 8, 2048, 1024, 16, 8
HD = D // H            # 128 = head dim = one partition chunk
FF = 4 * D             # 4096
NCH = D // 128         # 8 feature chunks
NHC = FF // 128        # 32 hidden chunks
EPS = 1e-5
ISQ = float(1.0 / np.sqrt(128.0))
MASKC = 340.0

SX = 16.0              # x+pe fp8 scale
SW = 64.0              # weight fp8 scale
SH = 32.0              # LN1-out / gelu-out fp8 scale
SO = 32.0              # attention-out fp8 requant scale

SA = 512               # macro12 token slab
NSA = T // SA
SB = 256               # macro34 token slab
NSB = T // SB

_NC_CACHE = {}


def build_nc(flags):
    (zbv, zb1, zbo, zb2a1, za2) = flags
    nc = bacc.Bacc(None, target_bir_lowering=False)

    dram = {}
    # ---- per-core inputs ----
    dram["x8"] = nc.declare_dram_parameter("x8", [128, NCH, T], FP8,
                                           isOutput=False)
    dram["xs"] = nc.declare_dram_parameter("xs", [128, NCH, T], F16,
                                           isOutput=False)
    # ---- shared weights ----
    for nm, sh, dt in (
            ("wq8", [128, NCH, D], FP8), ("wk8", [128, NCH, D], FP8),
            ("wv8", [128, NCH, D], FP8), ("wo8", [128, NCH, D], FP8),
            ("w1h", [128, NCH, FF], FP8),
            ("w2h", [128, NHC, D], FP8), ("w2l", [128, NHC, D], FP8),
            ("bqp", [128, NCH], F32), ("bkp", [128, NCH], F32),
            ("bvS", [1, D], F16), ("b1p", [128, NHC], F32),
            ("boS", [128, NCH], F32), ("bb2p", [128, NCH], F32),
            ("g1p", [128, NCH], F32), ("g2p", [128, NCH], F32),
            ("be2p", [128, NCH], F32),
            ("masku", [9, 128], F16), ("maskw", [9, 512], F16),
            ("onesrow", [1, 128], F16), ("onescol", [128, 1], F16)):
        dram[nm] = nc.declare_dram_parameter(nm, sh, dt, isOutput=False)

    dram["outb"] = nc.declare_dram_parameter("outb", [128, NCH, T], F16,
                                             isOutput=True)

    with tile.TileContext(nc) as tc:
        _emit(nc, tc, flags, dram)
    nc.compile()
    return nc


def _emit(nc, tc, flags, dram):
    (zbv, zb1, zbo, zb2a1, za2) = flags
    open_pools = {}

    def popen(name, **kw):
        cm = tc.tile_pool(name=name, **kw)
        pool = cm.__enter__()
        open_pools[name] = cm
        return pool

    def pclose(name):
        open_pools.pop(name).__exit__(None, None, None)

    # ---- constants (live whole kernel) ----
    consts = popen("consts", bufs=1)

    pending_cdma = []

    def cdma(name, shape, dt):
        t = consts.tile(shape, dt, tag=f"c_{name}", name=f"c_{name}")
        pending_cdma.append((t, name))
        return t

    c_bq = cdma("bqp", [128, NCH], F32)
    c_bk = cdma("bkp", [128, NCH], F32)
    c_bv = None if zbv else cdma("bvS", [1, D], F16)
    c_mu = cdma("masku", [9, 128], F16)
    c_mw = cdma("maskw", [9, 512], F16)
    c_or = cdma("onesrow", [1, 128], F16)
    c_oc = cdma("onescol", [128, 1], F16)
    c_eps = consts.tile([1, 1], F32, name="c_eps")
    nc.vector.memset(c_eps, EPS)

    # attention output (fp16, full residency)
    osp = popen("osp", bufs=1)
    os16 = osp.tile([128, NCH, T], F16, name="os16")

    # macro34 weights minus w2l: space reserved up-front so their DMAs can
    # stream during macro12 instead of waiting for its pools to die.
    w1p = popen("w1p", bufs=1)
    wo_sb = w1p.tile([128, NCH, D], FP8, name="wo_sb")
    w1h_sb = w1p.tile([128, NCH, FF], FP8, name="w1h_sb")
    w2h_sb = w1p.tile([128, NHC, D], FP8, name="w2h_sb")

    wqp = popen("wqp", bufs=1)
    wq_sb = wqp.tile([128, NCH, D], FP8, name="wq_sb")
    nc.sync.dma_start(out=wq_sb, in_=dram["wq8"][:, :, :])
    early = [p for p in pending_cdma if p[1] in ("bqp", "bkp")]
    for (t, name) in early:
        nc.sync.dma_start(out=t, in_=dram[name][:, :])
        pending_cdma.remove((t, name))
    wk_sb = wqp.tile([128, NCH, D], FP8, name="wk_sb")
    nc.sync.dma_start(out=wk_sb, in_=dram["wk8"][:, :, :])
    wv_sb = wqp.tile([128, NCH, D], FP8, name="wv_sb")
    nc.sync.dma_start(out=wv_sb, in_=dram["wv8"][:, :, :])
    for (t, name) in pending_cdma:
        nc.sync.dma_start(out=t, in_=dram[name][:, :])
    pending_cdma.clear()
    nc.sync.dma_start(out=wo_sb, in_=dram["wo8"][:, :, :])
    for q in range(4):
        nc.sync.dma_start(out=w1h_sb[:, 2 * q:2 * q + 2, :],
                          in_=dram["w1h"][:, 2 * q:2 * q + 2, :])
    # stream w2h during macro12 (fills DMA idle; ready before macro34)
    for q in range(8):
        nc.sync.dma_start(out=w2h_sb[:, 4 * q:4 * q + 4, :],
                          in_=dram["w2h"][:, 4 * q:4 * q + 4, :])

    # ================= macro 1+2: QKV + attention =================
    pa = popen("pa", bufs=2)
    psa = popen("psa", bufs=1, space="PSUM")

    def emit_qkv(s):
        tsl = ts(s, SA)
        x8t = pa.tile([128, NCH, SA], FP8, tag="x8t")
        nc.scalar.dma_start(out=x8t, in_=dram["x8"][:, :, tsl])

        qst = pa.tile([128, NCH, SA], F16, tag="qst")
        kst = pa.tile([128, NCH, SA], F16, tag="kst")
        for (w_sb, cbias, dst, on_act) in ((wq_sb, c_bq, qst, True),
                                           (wk_sb, c_bk, kst, False)):
            for h in range(H):
                ps = psa.tile([128, SA], F32, tag="sps", bufs=2, name="sps")
                for i in range(NCH // 2):
                    nc.tensor.matmul(ps, w_sb[:, 2 * i:2 * i + 2, ts(h, 128)],
                                     x8t[:, 2 * i:2 * i + 2, :],
                                     start=i == 0, stop=i == NCH // 2 - 1,
                                     perf_mode=DR)
                if on_act:
                    nc.scalar.activation(out=dst[:, h, :], in_=ps,
                                         func=AF.Identity,
                                         bias=cbias[:, h:h + 1],
                                         scale=1.0 / (SX * SW))
                else:
                    nc.vector.tensor_scalar(out=dst[:, h, :], in0=ps,
                                            scalar1=1.0 / (SX * SW),
                                            scalar2=cbias[:, h:h + 1],
                                            op0=OP.mult, op1=OP.add)

        # V: token-major out [tok, vout]
        vst = pa.tile([128, SA // 128, D], F16, tag="vst")
        for tb in range(SA // 128):
            for nb in range(2):
                ps = psa.tile([128, 512], F32, tag="sps", bufs=2, name="spsv")
                nkp = NCH // 2
                for i in range(nkp):
                    last = (i == nkp - 1) and zbv
                    nc.tensor.matmul(ps, x8t[:, 2 * i:2 * i + 2, ts(tb, 128)],
                                     wv_sb[:, 2 * i:2 * i + 2, ts(nb, 512)],
                                     start=i == 0, stop=last, perf_mode=DR)
                if not zbv:
                    nc.tensor.matmul(ps, c_or, c_bv[:, ts(nb, 512)],
                                     start=False, stop=True)
                nc.vector.tensor_scalar(out=vst[:, tb, ts(nb, 512)],
                                        in0=ps, scalar1=1.0 / (SX * SW),
                                        scalar2=None, op0=OP.mult)

        return qst, kst, vst

    def emit_attn(s, qkv):
        qst, kst, vst = qkv
        # attention per (128-token block, 4-head group): every pipeline
        # stage has its own 1-bank PSUM tag with bufs=2, so the
        # zsum -> reciprocal -> broadcast -> PV chains of consecutive
        # groups overlap instead of serializing on one PSUM buffer.
        for tb in range(SA // 128):
            for hf in range(2):
                hs = slice(4 * hf, 4 * hf + 4)
                et_ps = psa.tile([128, 4, 128], F32, tag="et_ps", bufs=2,
                                 name="et_ps")
                ets = pa.tile([128, 4, 128], F16, tag="ets", bufs=3)
                nc.tensor.matmul(et_ps, c_mu, c_mw, start=True, stop=False,
                                 skip_group_check=True)
                for hh in range(4):
                    h = 4 * hf + hh
                    nc.tensor.matmul(
                        et_ps[:, hh, :],
                        kst[:, h, ts(tb, 128)], qst[:, h, ts(tb, 128)],
                        start=False, stop=hh == 3, skip_group_check=True)
                nc.scalar.activation(out=ets, in_=et_ps, func=AF.Exp,
                                     scale=ISQ)
                rb_ps = psa.tile([128, 4, 128], F32, tag="rb_ps", bufs=2,
                                 name="rb_ps")
                # z row lives on partition 0 of rb_ps until the broadcast
                # matmul overwrites it (tile deps serialize the reciprocal
                # before it).
                nc.tensor.matmul(rb_ps[0:1, :, :], c_oc, ets,
                                 start=True, stop=True)
                rz = pa.tile([1, 4, 128], F16, tag="rz", bufs=2)
                with nc.allow_low_precision(reason="softmax renorm in fp16"):
                    nc.vector.reciprocal(out=rz, in_=rb_ps[0:1, :, :])
                nc.tensor.matmul(rb_ps, c_or, rz, start=True, stop=True)
                rbs = pa.tile([128, 4, 128], F16, tag="rbs", bufs=2)
                nc.scalar.copy(out=rbs, in_=rb_ps)
                o_ps = psa.tile([128, 4, 128], F32, tag="o_ps", bufs=2,
                                name="o_ps")
                for hh in range(4):
                    h = 4 * hf + hh
                    nc.tensor.matmul(o_ps[:, hh, :], vst[:, tb, ts(h, 128)],
                                     ets[:, hh, :], start=hh == 0,
                                     stop=hh == 3)
                tok = ts(s * (SA // 128) + tb, 128)
                nc.vector.tensor_mul(out=os16[:, hs, tok], in0=o_ps,
                                     in1=rbs)

    qkv_live = None
    for s in range(NSA):
        qkv_now = emit_qkv(s)
        if qkv_live is not None:
            emit_attn(s - 1, qkv_live)
        qkv_live = qkv_now
    emit_attn(NSA - 1, qkv_live)

    pclose("psa")
    pclose("pa")
    pclose("wqp")

    # ================= macro 3+4 (w2l streams at the boundary) ====
    w2p = popen("w2p", bufs=1)
    w2l_sb = w2p.tile([128, NHC, D], FP8, name="w2l_sb")
    for q in range(8):
        nc.sync.dma_start(out=w2l_sb[:, 4 * q:4 * q + 4, :],
                          in_=dram["w2l"][:, 4 * q:4 * q + 4, :])
    c_b1 = consts.tile([128, NHC], F32, name="c_b1")
    nc.sync.dma_start(out=c_b1, in_=dram["b1p"][:, :])
    c_bo = cdma("boS", [128, NCH], F32)
    c_bb2 = cdma("bb2p", [128, NCH], F32)
    c_g1 = cdma("g1p", [128, NCH], F32)
    c_g2 = cdma("g2p", [128, NCH], F32)
    c_be2 = cdma("be2p", [128, NCH], F32)
    for (t, name) in pending_cdma:
        nc.sync.dma_start(out=t, in_=dram[name][:, :])
    pending_cdma.clear()

    pb = popen("pb", bufs=2)
    psb = popen("psb", bufs=1, space="PSUM")

    def half_ps(nm):
        return psb.tile([128, 4, SB], F32, tag="half_ps", bufs=3, name=nm)

    def emit_requant(s):
        tsl = ts(s, SB)
        xst = pb.tile([128, NCH, SB], F16, tag="xst", bufs=2)
        nc.scalar.dma_start(out=xst, in_=dram["xs"][:, :, tsl])
        os8 = pb.tile([128, NCH, SB], FP8, tag="os8", bufs=2)
        nc.scalar.activation(out=os8, in_=os16[:, :, tsl], func=AF.Identity,
                             scale=SO)
        os8l = pb.tile([128, NCH, SB], FP8, tag="os8l", bufs=2)
        nc.vector.scalar_tensor_tensor(out=os8l, in0=os16[:, :, tsl],
                                       scalar=SO, in1=os8,
                                       op0=OP.mult, op1=OP.subtract)
        return {"xst": xst, "os8": os8, "os8l": os8l}

    def emit_oproj(s, st):
        tsl = ts(s, SB)
        xst, os8, os8l = st["xst"], st["os8"], st["os8l"]

        hpre = pb.tile([128, NCH, SB], F16, tag="hpre", bufs=1)
        for half in range(2):
            po = half_ps("po")
            for dq in range(4):
                dc = 4 * half + dq
                first = dq % 2 == 0
                for xa in (os8, os8l):
                    for i in range(NCH // 2):
                        nc.tensor.matmul(
                            po[:, dq, :],
                            wo_sb[:, 2 * i:2 * i + 2, ts(dc, 128)],
                            xa[:, 2 * i:2 * i + 2, :],
                            start=first,
                            stop=(dq % 2 == 1 and xa is os8l
                                  and i == NCH // 2 - 1),
                            perf_mode=DR)
                        first = False
            sl = slice(4 * half, 4 * half + 4)
            if zbo:
                nc.vector.scalar_tensor_tensor(
                    out=hpre[:, sl, :], in0=po, scalar=1.0 / (SO * SW),
                    in1=xst[:, sl, :], op0=OP.mult, op1=OP.add)
            else:
                for dq in range(4):
                    dc = 4 * half + dq
                    yo = pb.tile([128, SB], F16, tag="yg", bufs=2, name="yo")
                    nc.vector.tensor_scalar(out=yo, in0=xst[:, dc, :],
                                            scalar1=1.0,
                                            scalar2=c_bo[:, dc:dc + 1],
                                            op0=OP.mult, op1=OP.add)
                    nc.vector.scalar_tensor_tensor(
                        out=hpre[:, dc, :], in0=po[:, dq, :],
                        scalar=1.0 / (SO * SW), in1=yo,
                        op0=OP.mult, op1=OP.add)
        return {"hpre": hpre}

    def emit_ln1(s, st):
        hpre = st["hpre"]
        sq = pb.tile([128, NCH, SB], F16, tag="sq", bufs=1)
        nc.scalar.activation(out=sq, in_=hpre, func=AF.Square)
        st_ps = psb.tile([1, 2, SB], F32, tag="st_ps", bufs=1, name="st_ps")
        for dc in range(NCH):
            nc.tensor.matmul(st_ps[:, 0, :], c_oc, hpre[:, dc, :],
                             start=dc == 0, stop=False)
            nc.tensor.matmul(st_ps[:, 1, :], c_oc, sq[:, dc, :],
                             start=False, stop=dc == NCH - 1)
        rbm = _ln_stats(nc, pb, st_ps, c_eps, SB, "")
        bc_ps = psb.tile([128, 2, SB], F32, tag="bc_ps", bufs=1, name="bc_ps")
        nc.tensor.matmul(bc_ps[:, :, :], c_or, rbm, start=True, stop=True)
        bcs = pb.tile([128, 2, SB], F16, tag="bcs", bufs=1)
        nc.scalar.copy(out=bcs, in_=bc_ps)
        ys = pb.tile([128, NCH, SB], F16, tag="ys", bufs=2, name="ys")
        for dc in range(NCH):
            nc.vector.tensor_mul(out=ys[:, dc, :], in0=hpre[:, dc, :],
                                 in1=bcs[:, 0, :])
        for dc in range(NCH):
            nc.vector.tensor_sub(out=ys[:, dc, :], in0=ys[:, dc, :],
                                 in1=bcs[:, 1, :])
        y8 = pb.tile([128, NCH, SB], FP8, tag="y8", bufs=1)
        nc.scalar.activation(out=y8, in_=ys, func=AF.Identity, scale=SH)
        y8l = pb.tile([128, NCH, SB], FP8, tag="y8l", bufs=1)
        nc.vector.scalar_tensor_tensor(out=y8l, in0=ys, scalar=SH, in1=y8,
                                       op0=OP.mult, op1=OP.subtract)
        st.update(ys=ys, y8=y8, y8l=y8l)

    def emit_w1(s, st):
        y8, y8l = st["y8"], st["y8l"]
        h8 = pb.tile([128, NHC, SB], FP8, tag="h8", bufs=1)
        h8l = pb.tile([128, NHC, SB], FP8, tag="h8l", bufs=1)
        for g in range(NHC // 4):
            w1ps = half_ps("w1ps")
            for hh in range(4):
                hc = 4 * g + hh
                first = hh % 2 == 0
                for xa in (y8, y8l):
                    for i in range(NCH // 2):
                        nc.tensor.matmul(
                            w1ps[:, hh, :],
                            w1h_sb[:, 2 * i:2 * i + 2, ts(hc, 128)],
                            xa[:, 2 * i:2 * i + 2, :],
                            start=first,
                            stop=(hh % 2 == 1 and xa is y8l
                                  and i == NCH // 2 - 1),
                            perf_mode=DR)
                        first = False
            gsl = slice(4 * g, 4 * g + 4)
            g16 = pb.tile([128, 4, SB], F16, tag="g16", bufs=2)
            if zb1:
                nc.scalar.activation(out=g16, in_=w1ps, func=AF.Gelu,
                                     scale=1.0 / (SH * SW))
            else:
                for hh in range(4):
                    hc = 4 * g + hh
                    nc.scalar.activation(out=g16[:, hh, :],
                                         in_=w1ps[:, hh, :], func=AF.Gelu,
                                         bias=c_b1[:, hc:hc + 1],
                                         scale=1.0 / (SH * SW))
            nc.vector.tensor_scalar(out=h8[:, gsl, :], in0=g16,
                                    scalar1=SH, scalar2=None, op0=OP.mult)
            nc.vector.scalar_tensor_tensor(out=h8l[:, gsl, :], in0=g16,
                                           scalar=SH, in1=h8[:, gsl, :],
                                           op0=OP.mult, op1=OP.subtract)
        st["h8"] = h8
        st["h8l"] = h8l

    def emit_w2_half(s, st, half):
        h8, h8l, ys = st["h8"], st["h8l"], st["ys"]
        if half == 0:
            st["h2"] = pb.tile([128, NCH, SB], F16, tag="h2", bufs=2,
                               name="h2")
        h2 = st["h2"]
        w2ps = half_ps("w2ps")
        for dp in range(2):
            first = True
            for (xa, wa) in ((h8, w2h_sb), (h8l, w2h_sb), (h8, w2l_sb)):
                for dq in (2 * dp, 2 * dp + 1):
                    dc = 4 * half + dq
                    for i in range(NHC // 2):
                        nc.tensor.matmul(
                            w2ps[:, dq, :],
                            wa[:, 2 * i:2 * i + 2, ts(dc, 128)],
                            xa[:, 2 * i:2 * i + 2, :],
                            start=first,
                            stop=(dq == 2 * dp + 1 and wa is w2l_sb
                                  and i == NHC // 2 - 1),
                            perf_mode=DR)
                        first = False
        sl = slice(4 * half, 4 * half + 4)
        if zb2a1:
            nc.vector.scalar_tensor_tensor(
                out=h2[:, sl, :], in0=w2ps, scalar=1.0 / (SH * SW),
                in1=ys[:, sl, :], op0=OP.mult, op1=OP.add)
        else:
            for dq in range(4):
                dc = 4 * half + dq
                yg = pb.tile([128, SB], F16, tag="yg", bufs=2)
                nc.vector.tensor_scalar(out=yg, in0=ys[:, dc, :],
                                        scalar1=c_g1[:, dc:dc + 1],
                                        scalar2=c_bb2[:, dc:dc + 1],
                                        op0=OP.mult, op1=OP.add)
                nc.vector.scalar_tensor_tensor(
                    out=h2[:, dc, :], in0=w2ps[:, dq, :],
                    scalar=1.0 / (SH * SW), in1=yg,
                    op0=OP.mult, op1=OP.add)


    def emit_ln2(s, st, halves=1):
        h2 = st["h2"]
        sq2 = pb.tile([128, NCH, SB], F16, tag="sq", bufs=1, name="sq2")
        nc.scalar.activation(out=sq2, in_=h2, func=AF.Square)
        hw = SB // halves
        for hx in range(halves):
            hsl = slice(hx * hw, (hx + 1) * hw)
            tsl = slice(s * SB + hx * hw, s * SB + (hx + 1) * hw)
            st2_ps = psb.tile([1, 2, SB], F32, tag="st_ps", bufs=1,
                              name="st2_ps")
            for dc in range(NCH):
                nc.tensor.matmul(st2_ps[:, 0, :hw], c_oc, h2[:, dc, hsl],
                                 start=dc == 0, stop=False)
                nc.tensor.matmul(st2_ps[:, 1, :hw], c_oc, sq2[:, dc, hsl],
                                 start=False, stop=dc == NCH - 1)
            rbm2 = _ln_stats(nc, pb, st2_ps[:, :, :hw], c_eps, hw, "")
            bc2_ps = psb.tile([128, 2, SB], F32, tag="bc_ps", bufs=1,
                              name="bc2_ps")
            nc.tensor.matmul(bc2_ps[:, :, :hw], c_or, rbm2,
                             start=True, stop=True)
            bc2s = pb.tile([128, 2, SB], F16, tag="bcs", bufs=1, name="bc2s")
            nc.scalar.copy(out=bc2s[:, :, :hw], in_=bc2_ps[:, :, :hw])
            yout = pb.tile([128, NCH, SB], F16, tag="xst", bufs=2,
                           name="yout")
            for dc in range(NCH):
                nc.vector.tensor_mul(out=yout[:, dc, :hw],
                                     in0=h2[:, dc, hsl],
                                     in1=bc2s[:, 0, :hw])
            for dc in range(NCH):
                nc.vector.tensor_sub(out=yout[:, dc, :hw],
                                     in0=yout[:, dc, :hw],
                                     in1=bc2s[:, 1, :hw])
                if not za2:
                    nc.vector.tensor_scalar(out=yout[:, dc, :hw],
                                            in0=yout[:, dc, :hw],
                                            scalar1=c_g2[:, dc:dc + 1],
                                            scalar2=c_be2[:, dc:dc + 1],
                                            op0=OP.mult, op1=OP.add)
            nc.scalar.dma_start(out=dram["outb"][:, :, tsl],
                                in_=yout[:, :, :hw])

    rq = {0: emit_requant(0)}
    states = {0: emit_oproj(0, rq.pop(0))}
    emit_ln1(0, states[0])
    rq[1] = emit_requant(1)
    for s in range(NSB):
        st = states[s]
        if s + 1 < NSB:
            states[s + 1] = emit_oproj(s + 1, rq.pop(s + 1))
        if s + 2 < NSB:
            rq[s + 2] = emit_requant(s + 2)
        emit_w1(s, st)
        if s + 1 < NSB:
            emit_ln1(s + 1, states[s + 1])
        emit_w2_half(s, st, 0)
        emit_w2_half(s, st, 1)
        if s >= 1:
            emit_ln2(s - 1, states.pop(s - 1))
    emit_ln2(NSB - 1, states.pop(NSB - 1), halves=2)

    pclose("psb")
    pclose("pb")
    pclose("w2p")
    pclose("w1p")
    pclose("osp")
    pclose("consts")


def _ln_stats(nc, pool, st_ps, eps_t, TW, tag):
    """stat psum [1, 2, TW] (sum, sumsq) -> rstd, bm rows (fp16)."""
    mean = pool.tile([1, TW], F16, tag=f"mean{tag}", bufs=1)
    nc.scalar.activation(out=mean, in_=st_ps[:, 0, :], func=AF.Identity,
                         scale=1.0 / D)
    msq = pool.tile([1, TW], F16, tag=f"msq{tag}", bufs=1)
    nc.vector.tensor_mul(out=msq, in0=mean, in1=mean)
    var = pool.tile([1, TW], F16, tag=f"var{tag}", bufs=1)
    nc.vector.scalar_tensor_tensor(out=var, in0=st_ps[:, 1, :],
                                   scalar=1.0 / D, in1=msq,
                                   op0=OP.mult, op1=OP.subtract)
    sd = pool.tile([1, TW], F16, tag=f"msq{tag}", bufs=1, name="sd")
    nc.scalar.activation(out=sd, in_=var, func=AF.Sqrt, bias=eps_t, scale=1.0)
    rbm = pool.tile([1, 2, TW], F16, tag=f"rbm{tag}", bufs=1, name="rbm")
    with nc.allow_low_precision(reason="LN broadcast rows in fp16"):
        nc.vector.reciprocal(out=rbm[:, 0, :], in_=sd)
    nc.vector.tensor_mul(out=rbm[:, 1, :], in0=mean, in1=rbm[:, 0, :])
    return rbm


# ======================= host side =======================

def _prep_shared(w_qkv, b_qkv, w_out, b_out, w1, b1, w2, b2,
                 g1, beta1, g2, beta2):
    wq, wk, wv = w_qkv[0:D], w_qkv[D:2 * D], w_qkv[2 * D:3 * D]
    bq, bk, bv = b_qkv[0:D], b_qkv[D:2 * D], b_qkv[2 * D:3 * D]

    def pmaj(v, n):
        return np.ascontiguousarray(
            np.asarray(v, np.float32).reshape(n, 128).T)

    def chunk8(wT, nk):
        # [K, M] -> [128, nk, M] (K = nk*128, chunk-major along K)
        return np.ascontiguousarray(
            wT.reshape(nk, 128, wT.shape[1]).transpose(1, 0, 2))

    wqT = np.ascontiguousarray(np.asarray(wq, np.float32).T)
    wkT = np.ascontiguousarray(np.asarray(wk, np.float32).T)
    wvT = np.ascontiguousarray(np.asarray(wv, np.float32).T)
    woT = np.ascontiguousarray(np.asarray(w_out, np.float32).T)
    w1g = np.asarray(w1, np.float32) * np.asarray(g1, np.float32)[None, :]
    w1T = np.ascontiguousarray(w1g.T)          # [D, FF]
    w2T = np.ascontiguousarray(np.asarray(w2, np.float32).T)  # [FF, D]

    w2s = w2T * SW
    w2hT = w2s.astype(E4)
    w2lT = (w2s - w2hT.astype(np.float32)).astype(E4)

    b1f = np.asarray(b1, np.float32) + w1g @ np.asarray(beta1, np.float32)
    bb2 = np.asarray(b2, np.float32) + np.asarray(beta1, np.float32)

    mu = np.zeros((9, 128), np.float32)
    mw = np.zeros((9, 128), np.float32)
    for w in range(8):
        mu[w, w * 16:(w + 1) * 16] = MASKC
        mw[w, w * 16:(w + 1) * 16] = 1.0
    mu[8, :] = -MASKC
    mw[8, :] = 1.0
    mwr = np.tile(mw, (1, 4))

    shared = {
        "wq8": chunk8(wqT * SW, NCH).astype(E4),
        "wk8": chunk8(wkT * SW, NCH).astype(E4),
        "wv8": chunk8(wvT * SW, NCH).astype(E4),
        "wo8": chunk8(woT * SW, NCH).astype(E4),
        "w1h": chunk8(w1T * SW, NCH).astype(E4),
        "w2h": chunk8(w2hT.astype(np.float32), NHC).astype(E4),
        "w2l": chunk8(w2lT.astype(np.float32), NHC).astype(E4),
        "bqp": pmaj(bq, NCH), "bkp": pmaj(bk, NCH),
        "bvS": (np.asarray(bv, np.float32) * SX * SW).reshape(1, D).astype(NF),
        "b1p": pmaj(b1f, NHC),
        "boS": pmaj(np.asarray(b_out, np.float32), NCH),
        "bb2p": pmaj(bb2, NCH),
        "g1p": pmaj(g1, NCH),
        "g2p": pmaj(g2, NCH), "be2p": pmaj(beta2, NCH),
        "masku": mu.astype(NF), "maskw": mwr.astype(NF),
        "onesrow": np.ones((1, 128), np.float32).astype(NF),
        "onescol": np.ones((128, 1), np.float32).astype(NF),
    }
    flags = (
        bool(np.all(np.asarray(bv) == 0)),                       # zbv
        bool(np.all(b1f == 0)),                                  # zb1
        bool(np.all(np.asarray(b_out) == 0)),                    # zbo
        bool(np.all(bb2 == 0)
             and np.all(np.asarray(g1, np.float32) == 1.0)),     # zb2a1
        bool(np.all(np.asarray(beta2) == 0)
             and np.all(np.asarray(g2, np.float32) == 1.0)),     # za2
    )
    return shared, flags


def make_in_maps(inputs):
    ff = np.asarray(inputs["frame_features"], np.float32)
    fi = np.asarray(inputs["frame_indices"])
    shared, flags = _prep_shared(
        np.asarray(inputs["w_qkv"]), np.asarray(inputs["b_qkv"]),
        np.asarray(inputs["w_out"]), np.asarray(inputs["b_out"]),
        np.asarray(inputs["w1"]), np.asarray(inputs["b1"]),
        np.asarray(inputs["w2"]), np.asarray(inputs["b2"]),
        np.asarray(inputs["g1"]), np.asarray(inputs["beta1"]),
        np.asarray(inputs["g2"]), np.asarray(inputs["beta2"]))

    div = np.exp(np.float32(-np.log(10000.0))
                 * np.arange(0, D, 2, dtype=np.float32) / np.float32(D))
    in_maps = []
    for b in range(B):
        pos = np.asarray(fi[b], np.float32)[:, None]
        ang = pos * div[None, :]
        pe = np.empty((T, D), np.float32)
        pe[:, 0::2] = np.sin(ang)
        pe[:, 1::2] = np.cos(ang)
        xpe = ff[b] + pe                       # [T, D]
        xpeT = np.ascontiguousarray(xpe.T)     # [D, T]
        x8 = np.ascontiguousarray(
            (xpeT * SX).reshape(NCH, 128, T).transpose(1, 0, 2)).astype(E4)
        xsc = np.ascontiguousarray(
            xpeT.reshape(NCH, 128, T).transpose(1, 0, 2)).astype(NF)
        m = dict(shared)
        m["x8"] = x8
        m["xs"] = xsc
        in_maps.append(m)
    return in_maps, flags


def get_nc(flags=(True, True, True, True, True)):
    if flags not in _NC_CACHE:
        _NC_CACHE[flags] = build_nc(flags)
    return _NC_CACHE[flags]


def kernel(**inputs) -> np.ndarray:
    in_maps, flags = make_in_maps(inputs)
    nc = get_nc(flags)
    res = run_bass_kernel_spmd(nc, in_maps, core_ids=list(range(B)))
    outs = []
    for r in res.results:
        ob = np.asarray(r["outb"])             # [128, NCH, T] fp16
        oT = ob.transpose(1, 0, 2).reshape(D, T)
        outs.append(oT.T.astype(np.float32))
    return np.ascontiguousarray(np.stack(outs))


# revision 34
# speedup vs baseline: 1.2524x; 1.0222x over previous
"""Trainium2 Bass kernel for a local-window-attention transformer block (v4).

Sharding: data-parallel over batch (one batch element per NeuronCore).

v4 vs v2 (604us -> 482us TimelineSim per core):
- fp16 replaces bf16 on the whole residual/attention path (same engine
  cost, ~8x finer mantissa), buying accuracy budget to cut PE work:
  * w1 runs as fp8 "x2b": y8/y8l hi-lo activations against a single fp8
    w1h (w1l is gone entirely, freeing 32KB/partition of SBUF),
  * w2 runs as fp8 "x3": h8/h8l hi-lo against w2h + h8 against w2l,
  * the out-projection moving operand is the fp16 attention output
    requantized per-slab to fp8 hi/lo (os8/os8l) against fp8 wo,
  all DoubleRow at 0.5 cycles/row instead of 1.0 bf16.
- scale-free residual chain (no CS1 prescaling of xs/eps).
- attention runs per (128-token block, 4-head group) with every PSUM stage
  (scores, zsum/broadcast, PV) on its own 1-bank double-buffered tag, so
  the exp -> zsum -> reciprocal -> broadcast -> PV chains of consecutive
  groups overlap.
- macro34 is software-pipelined as: oproj(s+1) | requant(s+2) | w1(s) |
  ln1(s+1) | w2(s) | ln2(s-1), which gives each LayerNorm's long
  Act/DVE chain a full w2 phase to hide behind; sq/sq2 are computed at
  their LN sites so one buffer suffices.
- w2h streams during the attention phase (space freed by dropping w1l);
  only w2l loads at the phase boundary, and w2 accumulation orders the
  w2l-reads last to cover its DMA.
- QKV/attention PSUM drains alternate between Act and DVE per head so
  neither engine is the macro12 bottleneck.
"""
import numpy as np
import ml_dtypes

import concourse.bass as bass
import concourse.bacc as bacc
import concourse.mybir as mybir
import concourse.tile as tile
from concourse.bass import ts
from concourse.bass_utils import run_bass_kernel_spmd

F32 = mybir.dt.float32
F16 = mybir.dt.float16
FP8 = mybir.dt.float8e4
AF = mybir.ActivationFunctionType
OP = mybir.AluOpType
DR = mybir.MatmulPerfMode.DoubleRow
NF = np.float16
E4 = ml_dtypes.float8_e4m3

B, T, D, W, H = 8, 2048, 1024, 16, 8
HD = D // H            # 128 = head dim = one partition chunk
FF = 4 * D             # 4096
NCH = D // 128         # 8 feature chunks
NHC = FF // 128        # 32 hidden chunks
EPS = 1e-5
ISQ = float(1.0 / np.sqrt(128.0))
MASKC = 340.0

SX = 16.0              # x+pe fp8 scale
SW = 64.0              # weight fp8 scale
SH = 32.0              # LN1-out / gelu-out fp8 scale
SO = 32.0              # attention-out fp8 requant scale

SA = 512               # macro12 token slab
NSA = T // SA
SB = 256               # macro34 token slab
NSB = T // SB

_NC_CACHE = {}


def build_nc(flags):
    (zbv, zb1, zbo, zb2a1, za2) = flags
    nc = bacc.Bacc(None, target_bir_lowering=False)

    dram = {}
    # ---- per-core inputs ----
    dram["x8"] = nc.declare_dram_parameter("x8", [128, NCH, T], FP8,
                                           isOutput=False)
    dram["xs"] = nc.declare_dram_parameter("xs", [128, NCH, T], F16,
                                           isOutput=False)
    # ---- shared weights ----
    for nm, sh, dt in (
            ("wq8", [128, NCH, D], FP8), ("wk8", [128, NCH, D], FP8),
            ("wv8", [128, NCH, D], FP8), ("wo8", [128, NCH, D], FP8),
            ("w1h", [128, NCH, FF], FP8),
            ("w2h", [128, NHC, D], FP8), ("w2l", [128, NHC, D], FP8),
            ("bqp", [128, NCH], F32), ("bkp", [128, NCH], F32),
            ("bvS", [1, D], F16), ("b1p", [128, NHC], F32),
            ("boS", [128, NCH], F32), ("bb2p", [128, NCH], F32),
            ("g1p", [128, NCH], F32), ("g2p", [128, NCH], F32),
            ("be2p", [128, NCH], F32),
            ("masku", [9, 128], F16), ("maskw", [9, 512], F16),
            ("onesrow", [1, 128], F16), ("onescol", [128, 1], F16)):
        dram[nm] = nc.declare_dram_parameter(nm, sh, dt, isOutput=False)

    dram["outb"] = nc.declare_dram_parameter("outb", [128, NCH, T], F16,
                                             isOutput=True)

    with tile.TileContext(nc) as tc:
        _emit(nc, tc, flags, dram)
    nc.compile()
    return nc


def _emit(nc, tc, flags, dram):
    (zbv, zb1, zbo, zb2a1, za2) = flags
    open_pools = {}

    def popen(name, **kw):
        cm = tc.tile_pool(name=name, **kw)
        pool = cm.__enter__()
        open_pools[name] = cm
        return pool

    def pclose(name):
        open_pools.pop(name).__exit__(None, None, None)

    # ---- constants (live whole kernel) ----
    consts = popen("consts", bufs=1)

    pending_cdma = []

    def cdma(name, shape, dt):
        t = consts.tile(shape, dt, tag=f"c_{name}", name=f"c_{name}")
        pending_cdma.append((t, name))
        return t

    c_bq = cdma("bqp", [128, NCH], F32)
    c_bk = cdma("bkp", [128, NCH], F32)
    c_bv = None if zbv else cdma("bvS", [1, D], F16)
    c_mu = cdma("masku", [9, 128], F16)
    c_mw = cdma("maskw", [9, 512], F16)
    c_or = cdma("onesrow", [1, 128], F16)
    c_oc = cdma("onescol", [128, 1], F16)
    c_eps = consts.tile([1, 1], F32, name="c_eps")
    nc.vector.memset(c_eps, EPS)

    # attention output (fp16, full residency)
    osp = popen("osp", bufs=1)
    os16 = osp.tile([128, NCH, T], F16, name="os16")

    # macro34 weights minus w2l: space reserved up-front so their DMAs can
    # stream during macro12 instead of waiting for its pools to die.
    w1p = popen("w1p", bufs=1)
    wo_sb = w1p.tile([128, NCH, D], FP8, name="wo_sb")
    w1h_sb = w1p.tile([128, NCH, FF], FP8, name="w1h_sb")
    w2h_sb = w1p.tile([128, NHC, D], FP8, name="w2h_sb")

    wqp = popen("wqp", bufs=1)
    wq_sb = wqp.tile([128, NCH, D], FP8, name="wq_sb")
    nc.sync.dma_start(out=wq_sb, in_=dram["wq8"][:, :, :])
    early = [p for p in pending_cdma if p[1] in ("bqp", "bkp")]
    for (t, name) in early:
        nc.sync.dma_start(out=t, in_=dram[name][:, :])
        pending_cdma.remove((t, name))
    wk_sb = wqp.tile([128, NCH, D], FP8, name="wk_sb")
    nc.sync.dma_start(out=wk_sb, in_=dram["wk8"][:, :, :])
    wv_sb = wqp.tile([128, NCH, D], FP8, name="wv_sb")
    nc.sync.dma_start(out=wv_sb, in_=dram["wv8"][:, :, :])
    for (t, name) in pending_cdma:
        nc.sync.dma_start(out=t, in_=dram[name][:, :])
    pending_cdma.clear()
    nc.sync.dma_start(out=wo_sb, in_=dram["wo8"][:, :, :])
    for q in range(4):
        nc.sync.dma_start(out=w1h_sb[:, 2 * q:2 * q + 2, :],
                          in_=dram["w1h"][:, 2 * q:2 * q + 2, :])
    # stream w2h during macro12 (fills DMA idle; ready before macro34)
    for q in range(8):
        nc.sync.dma_start(out=w2h_sb[:, 4 * q:4 * q + 4, :],
                          in_=dram["w2h"][:, 4 * q:4 * q + 4, :])

    # ================= macro 1+2: QKV + attention =================
    pa = popen("pa", bufs=2)
    psa = popen("psa", bufs=1, space="PSUM")

    def emit_qkv(s):
        tsl = ts(s, SA)
        x8t = pa.tile([128, NCH, SA], FP8, tag="x8t")
        nc.scalar.dma_start(out=x8t, in_=dram["x8"][:, :, tsl])

        qst = pa.tile([128, NCH, SA], F16, tag="qst")
        kst = pa.tile([128, NCH, SA], F16, tag="kst")
        for (w_sb, cbias, dst, on_act) in ((wq_sb, c_bq, qst, True),
                                           (wk_sb, c_bk, kst, False)):
            for h in range(H):
                ps = psa.tile([128, SA], F32, tag="sps", bufs=2, name="sps")
                for i in range(NCH // 2):
                    nc.tensor.matmul(ps, w_sb[:, 2 * i:2 * i + 2, ts(h, 128)],
                                     x8t[:, 2 * i:2 * i + 2, :],
                                     start=i == 0, stop=i == NCH // 2 - 1,
                                     perf_mode=DR)
                if h % 2 == 0:
                    nc.scalar.activation(out=dst[:, h, :], in_=ps,
                                         func=AF.Identity,
                                         bias=cbias[:, h:h + 1],
                                         scale=1.0 / (SX * SW))
                else:
                    nc.vector.tensor_scalar(out=dst[:, h, :], in0=ps,
                                            scalar1=1.0 / (SX * SW),
                                            scalar2=cbias[:, h:h + 1],
                                            op0=OP.mult, op1=OP.add)

        # V: token-major out [tok, vout]
        vst = pa.tile([128, SA // 128, D], F16, tag="vst")
        for tb in range(SA // 128):
            for nb in range(2):
                ps = psa.tile([128, 512], F32, tag="sps", bufs=2, name="spsv")
                nkp = NCH // 2
                for i in range(nkp):
                    last = (i == nkp - 1) and zbv
                    nc.tensor.matmul(ps, x8t[:, 2 * i:2 * i + 2, ts(tb, 128)],
                                     wv_sb[:, 2 * i:2 * i + 2, ts(nb, 512)],
                                     start=i == 0, stop=last, perf_mode=DR)
                if not zbv:
                    nc.tensor.matmul(ps, c_or, c_bv[:, ts(nb, 512)],
                                     start=False, stop=True)
                if nb == 0:
                    nc.scalar.activation(out=vst[:, tb, ts(nb, 512)],
                                         in_=ps, func=AF.Identity,
                                         scale=1.0 / (SX * SW))
                else:
                    nc.vector.tensor_scalar(out=vst[:, tb, ts(nb, 512)],
                                            in0=ps, scalar1=1.0 / (SX * SW),
                                            scalar2=None, op0=OP.mult)

        return qst, kst, vst

    def emit_attn(s, qkv):
        qst, kst, vst = qkv
        # attention per (128-token block, 4-head group): every pipeline
        # stage has its own 1-bank PSUM tag with bufs=2, so the
        # zsum -> reciprocal -> broadcast -> PV chains of consecutive
        # groups overlap instead of serializing on one PSUM buffer.
        for tb in range(SA // 128):
            for hf in range(2):
                hs = slice(4 * hf, 4 * hf + 4)
                et_ps = psa.tile([128, 4, 128], F32, tag="et_ps", bufs=2,
                                 name="et_ps")
                ets = pa.tile([128, 4, 128], F16, tag="ets", bufs=3)
                nc.tensor.matmul(et_ps, c_mu, c_mw, start=True, stop=False,
                                 skip_group_check=True)
                for hh in range(4):
                    h = 4 * hf + hh
                    nc.tensor.matmul(
                        et_ps[:, hh, :],
                        kst[:, h, ts(tb, 128)], qst[:, h, ts(tb, 128)],
                        start=False, stop=hh == 3, skip_group_check=True)
                nc.scalar.activation(out=ets, in_=et_ps, func=AF.Exp,
                                     scale=ISQ)
                rb_ps = psa.tile([128, 4, 128], F32, tag="rb_ps", bufs=2,
                                 name="rb_ps")
                # z row lives on partition 0 of rb_ps until the broadcast
                # matmul overwrites it (tile deps serialize the reciprocal
                # before it).
                nc.tensor.matmul(rb_ps[0:1, :, :], c_oc, ets,
                                 start=True, stop=True)
                rz = pa.tile([1, 4, 128], F16, tag="rz", bufs=2)
                with nc.allow_low_precision(reason="softmax renorm in fp16"):
                    nc.vector.reciprocal(out=rz, in_=rb_ps[0:1, :, :])
                nc.tensor.matmul(rb_ps, c_or, rz, start=True, stop=True)
                rbs = pa.tile([128, 4, 128], F16, tag="rbs", bufs=2)
                nc.scalar.copy(out=rbs, in_=rb_ps)
                o_ps = psa.tile([128, 4, 128], F32, tag="o_ps", bufs=2,
                                name="o_ps")
                for hh in range(4):
                    h = 4 * hf + hh
                    nc.tensor.matmul(o_ps[:, hh, :], vst[:, tb, ts(h, 128)],
                                     ets[:, hh, :], start=hh == 0,
                                     stop=hh == 3)
                tok = ts(s * (SA // 128) + tb, 128)
                nc.vector.tensor_mul(out=os16[:, hs, tok], in0=o_ps,
                                     in1=rbs)

    qkv_live = None
    for s in range(NSA):
        qkv_now = emit_qkv(s)
        if qkv_live is not None:
            emit_attn(s - 1, qkv_live)
        qkv_live = qkv_now
    emit_attn(NSA - 1, qkv_live)

    pclose("psa")
    pclose("pa")
    pclose("wqp")

    # ================= macro 3+4 (w2l streams at the boundary) ====
    w2p = popen("w2p", bufs=1)
    w2l_sb = w2p.tile([128, NHC, D], FP8, name="w2l_sb")
    for q in range(8):
        nc.sync.dma_start(out=w2l_sb[:, 4 * q:4 * q + 4, :],
                          in_=dram["w2l"][:, 4 * q:4 * q + 4, :])
    c_b1 = consts.tile([128, NHC], F32, name="c_b1")
    nc.sync.dma_start(out=c_b1, in_=dram["b1p"][:, :])
    c_bo = cdma("boS", [128, NCH], F32)
    c_bb2 = cdma("bb2p", [128, NCH], F32)
    c_g1 = cdma("g1p", [128, NCH], F32)
    c_g2 = cdma("g2p", [128, NCH], F32)
    c_be2 = cdma("be2p", [128, NCH], F32)
    for (t, name) in pending_cdma:
        nc.sync.dma_start(out=t, in_=dram[name][:, :])
    pending_cdma.clear()

    pb = popen("pb", bufs=2)
    psb = popen("psb", bufs=1, space="PSUM")

    def half_ps(nm):
        return psb.tile([128, 4, SB], F32, tag="half_ps", bufs=3, name=nm)

    def emit_requant(s):
        tsl = ts(s, SB)
        xst = pb.tile([128, NCH, SB], F16, tag="xst", bufs=2)
        nc.scalar.dma_start(out=xst, in_=dram["xs"][:, :, tsl])
        os8 = pb.tile([128, NCH, SB], FP8, tag="os8", bufs=2)
        nc.scalar.activation(out=os8, in_=os16[:, :, tsl], func=AF.Identity,
                             scale=SO)
        os8l = pb.tile([128, NCH, SB], FP8, tag="os8l", bufs=2)
        nc.vector.scalar_tensor_tensor(out=os8l, in0=os16[:, :, tsl],
                                       scalar=SO, in1=os8,
                                       op0=OP.mult, op1=OP.subtract)
        return {"xst": xst, "os8": os8, "os8l": os8l}

    def emit_oproj(s, st):
        tsl = ts(s, SB)
        xst, os8, os8l = st["xst"], st["os8"], st["os8l"]

        hpre = pb.tile([128, NCH, SB], F16, tag="hpre", bufs=1)
        for half in range(2):
            po = half_ps("po")
            for dq in range(4):
                dc = 4 * half + dq
                first = dq % 2 == 0
                for xa in (os8, os8l):
                    for i in range(NCH // 2):
                        nc.tensor.matmul(
                            po[:, dq, :],
                            wo_sb[:, 2 * i:2 * i + 2, ts(dc, 128)],
                            xa[:, 2 * i:2 * i + 2, :],
                            start=first,
                            stop=(dq % 2 == 1 and xa is os8l
                                  and i == NCH // 2 - 1),
                            perf_mode=DR)
                        first = False
            sl = slice(4 * half, 4 * half + 4)
            if zbo:
                nc.vector.scalar_tensor_tensor(
                    out=hpre[:, sl, :], in0=po, scalar=1.0 / (SO * SW),
                    in1=xst[:, sl, :], op0=OP.mult, op1=OP.add)
            else:
                for dq in range(4):
                    dc = 4 * half + dq
                    yo = pb.tile([128, SB], F16, tag="yg", bufs=2, name="yo")
                    nc.vector.tensor_scalar(out=yo, in0=xst[:, dc, :],
                                            scalar1=1.0,
                                            scalar2=c_bo[:, dc:dc + 1],
                                            op0=OP.mult, op1=OP.add)
                    nc.vector.scalar_tensor_tensor(
                        out=hpre[:, dc, :], in0=po[:, dq, :],
                        scalar=1.0 / (SO * SW), in1=yo,
                        op0=OP.mult, op1=OP.add)
        return {"hpre": hpre}

    def emit_ln1(s, st):
        hpre = st["hpre"]
        sq = pb.tile([128, NCH, SB], F16, tag="sq", bufs=1)
        nc.scalar.activation(out=sq, in_=hpre, func=AF.Square)
        st_ps = psb.tile([1, 2, SB], F32, tag="st_ps", bufs=1, name="st_ps")
        for dc in range(NCH):
            nc.tensor.matmul(st_ps[:, 0, :], c_oc, hpre[:, dc, :],
                             start=dc == 0, stop=False)
            nc.tensor.matmul(st_ps[:, 1, :], c_oc, sq[:, dc, :],
                             start=False, stop=dc == NCH - 1)
        rbm = _ln_stats(nc, pb, st_ps, c_eps, SB, "")
        bc_ps = psb.tile([128, 2, SB], F32, tag="bc_ps", bufs=1, name="bc_ps")
        nc.tensor.matmul(bc_ps[:, :, :], c_or, rbm, start=True, stop=True)
        bcs = pb.tile([128, 2, SB], F16, tag="bcs", bufs=2)
        nc.scalar.copy(out=bcs, in_=bc_ps)
        ys = pb.tile([128, NCH, SB], F16, tag="ys", bufs=2, name="ys")
        for dc in range(NCH):
            nc.vector.tensor_mul(out=ys[:, dc, :], in0=hpre[:, dc, :],
                                 in1=bcs[:, 0, :])
        for dc in range(NCH):
            nc.vector.tensor_sub(out=ys[:, dc, :], in0=ys[:, dc, :],
                                 in1=bcs[:, 1, :])
        y8 = pb.tile([128, NCH, SB], FP8, tag="y8", bufs=1)
        nc.scalar.activation(out=y8, in_=ys, func=AF.Identity, scale=SH)
        y8l = pb.tile([128, NCH, SB], FP8, tag="y8l", bufs=1)
        nc.vector.scalar_tensor_tensor(out=y8l, in0=ys, scalar=SH, in1=y8,
                                       op0=OP.mult, op1=OP.subtract)
        st.update(ys=ys, y8=y8, y8l=y8l)

    def emit_w1(s, st):
        y8, y8l = st["y8"], st["y8l"]
        h8 = pb.tile([128, NHC, SB], FP8, tag="h8", bufs=1)
        h8l = pb.tile([128, NHC, SB], FP8, tag="h8l", bufs=1)
        for g in range(NHC // 4):
            w1ps = half_ps("w1ps")
            for hh in range(4):
                hc = 4 * g + hh
                first = hh % 2 == 0
                for xa in (y8, y8l):
                    for i in range(NCH // 2):
                        nc.tensor.matmul(
                            w1ps[:, hh, :],
                            w1h_sb[:, 2 * i:2 * i + 2, ts(hc, 128)],
                            xa[:, 2 * i:2 * i + 2, :],
                            start=first,
                            stop=(hh % 2 == 1 and xa is y8l
                                  and i == NCH // 2 - 1),
                            perf_mode=DR)
                        first = False
            gsl = slice(4 * g, 4 * g + 4)
            g16 = pb.tile([128, 4, SB], F16, tag="g16", bufs=2)
            if zb1:
                nc.scalar.activation(out=g16, in_=w1ps, func=AF.Gelu,
                                     scale=1.0 / (SH * SW))
            else:
                for hh in range(4):
                    hc = 4 * g + hh
                    nc.scalar.activation(out=g16[:, hh, :],
                                         in_=w1ps[:, hh, :], func=AF.Gelu,
                                         bias=c_b1[:, hc:hc + 1],
                                         scale=1.0 / (SH * SW))
            nc.vector.tensor_scalar(out=h8[:, gsl, :], in0=g16,
                                    scalar1=SH, scalar2=None, op0=OP.mult)
            nc.vector.scalar_tensor_tensor(out=h8l[:, gsl, :], in0=g16,
                                           scalar=SH, in1=h8[:, gsl, :],
                                           op0=OP.mult, op1=OP.subtract)
        st["h8"] = h8
        st["h8l"] = h8l

    def emit_w2_half(s, st, half):
        h8, h8l, ys = st["h8"], st["h8l"], st["ys"]
        if half == 0:
            st["h2"] = pb.tile([128, NCH, SB], F16, tag="h2", bufs=2,
                               name="h2")
        h2 = st["h2"]
        w2ps = half_ps("w2ps")
        for dp in range(2):
            first = True
            for (xa, wa) in ((h8, w2h_sb), (h8l, w2h_sb), (h8, w2l_sb)):
                for dq in (2 * dp, 2 * dp + 1):
                    dc = 4 * half + dq
                    for i in range(NHC // 2):
                        nc.tensor.matmul(
                            w2ps[:, dq, :],
                            wa[:, 2 * i:2 * i + 2, ts(dc, 128)],
                            xa[:, 2 * i:2 * i + 2, :],
                            start=first,
                            stop=(dq == 2 * dp + 1 and wa is w2l_sb
                                  and i == NHC // 2 - 1),
                            perf_mode=DR)
                        first = False
        sl = slice(4 * half, 4 * half + 4)
        if zb2a1:
            nc.vector.scalar_tensor_tensor(
                out=h2[:, sl, :], in0=w2ps, scalar=1.0 / (SH * SW),
                in1=ys[:, sl, :], op0=OP.mult, op1=OP.add)
        else:
            for dq in range(4):
                dc = 4 * half + dq
                yg = pb.tile([128, SB], F16, tag="yg", bufs=2)
                nc.vector.tensor_scalar(out=yg, in0=ys[:, dc, :],
                                        scalar1=c_g1[:, dc:dc + 1],
                                        scalar2=c_bb2[:, dc:dc + 1],
                                        op0=OP.mult, op1=OP.add)
                nc.vector.scalar_tensor_tensor(
                    out=h2[:, dc, :], in0=w2ps[:, dq, :],
                    scalar=1.0 / (SH * SW), in1=yg,
                    op0=OP.mult, op1=OP.add)


    def emit_ln2(s, st):
        h2 = st["h2"]
        sq2 = pb.tile([128, NCH, SB], F16, tag="sq", bufs=1, name="sq2")
        nc.scalar.activation(out=sq2, in_=h2, func=AF.Square)
        tsl = ts(s, SB)
        st2_ps = psb.tile([1, 2, SB], F32, tag="st_ps", bufs=1, name="st2_ps")
        for dc in range(NCH):
            nc.tensor.matmul(st2_ps[:, 0, :], c_oc, h2[:, dc, :],
                             start=dc == 0, stop=False)
            nc.tensor.matmul(st2_ps[:, 1, :], c_oc, sq2[:, dc, :],
                             start=False, stop=dc == NCH - 1)
        rbm2 = _ln_stats(nc, pb, st2_ps, c_eps, SB, "")
        bc2_ps = psb.tile([128, 2, SB], F32, tag="bc_ps", bufs=1,
                          name="bc2_ps")
        nc.tensor.matmul(bc2_ps[:, :, :], c_or, rbm2, start=True, stop=True)
        bc2s = pb.tile([128, 2, SB], F16, tag="bcs", bufs=2, name="bc2s")
        nc.scalar.copy(out=bc2s, in_=bc2_ps)
        yout = pb.tile([128, NCH, SB], F16, tag="xst", bufs=2, name="yout")
        for dc in range(NCH):
            nc.vector.tensor_mul(out=yout[:, dc, :], in0=h2[:, dc, :],
                                 in1=bc2s[:, 0, :])
        for dc in range(NCH):
            nc.vector.tensor_sub(out=yout[:, dc, :], in0=yout[:, dc, :],
                                 in1=bc2s[:, 1, :])
            if not za2:
                nc.vector.tensor_scalar(out=yout[:, dc, :],
                                        in0=yout[:, dc, :],
                                        scalar1=c_g2[:, dc:dc + 1],
                                        scalar2=c_be2[:, dc:dc + 1],
                                        op0=OP.mult, op1=OP.add)
        nc.scalar.dma_start(out=dram["outb"][:, :, tsl], in_=yout)

    rq = {0: emit_requant(0)}
    states = {0: emit_oproj(0, rq.pop(0))}
    emit_ln1(0, states[0])
    rq[1] = emit_requant(1)
    for s in range(NSB):
        st = states[s]
        if s + 1 < NSB:
            states[s + 1] = emit_oproj(s + 1, rq.pop(s + 1))
        if s + 2 < NSB:
            rq[s + 2] = emit_requant(s + 2)
        emit_w1(s, st)
        if s + 1 < NSB:
            emit_ln1(s + 1, states[s + 1])
        emit_w2_half(s, st, 0)
        emit_w2_half(s, st, 1)
        if s >= 1:
            emit_ln2(s - 1, states.pop(s - 1))
    emit_ln2(NSB - 1, states.pop(NSB - 1))

    pclose("psb")
    pclose("pb")
    pclose("w2p")
    pclose("w1p")
    pclose("osp")
    pclose("consts")


def _ln_stats(nc, pool, st_ps, eps_t, TW, tag):
    """stat psum [1, 2, TW] (sum, sumsq) -> rstd, bm rows (fp16)."""
    mean = pool.tile([1, TW], F16, tag=f"mean{tag}", bufs=1)
    nc.scalar.activation(out=mean, in_=st_ps[:, 0, :], func=AF.Identity,
                         scale=1.0 / D)
    msq = pool.tile([1, TW], F16, tag=f"msq{tag}", bufs=1)
    nc.vector.tensor_mul(out=msq, in0=mean, in1=mean)
    var = pool.tile([1, TW], F16, tag=f"var{tag}", bufs=1)
    nc.vector.scalar_tensor_tensor(out=var, in0=st_ps[:, 1, :],
                                   scalar=1.0 / D, in1=msq,
                                   op0=OP.mult, op1=OP.subtract)
    sd = pool.tile([1, TW], F16, tag=f"msq{tag}", bufs=1, name="sd")
    nc.scalar.activation(out=sd, in_=var, func=AF.Sqrt, bias=eps_t, scale=1.0)
    rbm = pool.tile([1, 2, TW], F16, tag=f"rbm{tag}", bufs=1, name="rbm")
    with nc.allow_low_precision(reason="LN broadcast rows in fp16"):
        nc.vector.reciprocal(out=rbm[:, 0, :], in_=sd)
    nc.vector.tensor_mul(out=rbm[:, 1, :], in0=mean, in1=rbm[:, 0, :])
    return rbm


# ======================= host side =======================

def _prep_shared(w_qkv, b_qkv, w_out, b_out, w1, b1, w2, b2,
                 g1, beta1, g2, beta2):
    wq, wk, wv = w_qkv[0:D], w_qkv[D:2 * D], w_qkv[2 * D:3 * D]
    bq, bk, bv = b_qkv[0:D], b_qkv[D:2 * D], b_qkv[2 * D:3 * D]

    def pmaj(v, n):
        return np.ascontiguousarray(
            np.asarray(v, np.float32).reshape(n, 128).T)

    def chunk8(wT, nk):
        # [K, M] -> [128, nk, M] (K = nk*128, chunk-major along K)
        return np.ascontiguousarray(
            wT.reshape(nk, 128, wT.shape[1]).transpose(1, 0, 2))

    wqT = np.ascontiguousarray(np.asarray(wq, np.float32).T)
    wkT = np.ascontiguousarray(np.asarray(wk, np.float32).T)
    wvT = np.ascontiguousarray(np.asarray(wv, np.float32).T)
    woT = np.ascontiguousarray(np.asarray(w_out, np.float32).T)
    w1g = np.asarray(w1, np.float32) * np.asarray(g1, np.float32)[None, :]
    w1T = np.ascontiguousarray(w1g.T)          # [D, FF]
    w2T = np.ascontiguousarray(np.asarray(w2, np.float32).T)  # [FF, D]

    w2s = w2T * SW
    w2hT = w2s.astype(E4)
    w2lT = (w2s - w2hT.astype(np.float32)).astype(E4)

    b1f = np.asarray(b1, np.float32) + w1g @ np.asarray(beta1, np.float32)
    bb2 = np.asarray(b2, np.float32) + np.asarray(beta1, np.float32)

    mu = np.zeros((9, 128), np.float32)
    mw = np.zeros((9, 128), np.float32)
    for w in range(8):
        mu[w, w * 16:(w + 1) * 16] = MASKC
        mw[w, w * 16:(w + 1) * 16] = 1.0
    mu[8, :] = -MASKC
    mw[8, :] = 1.0
    mwr = np.tile(mw, (1, 4))

    shared = {
        "wq8": chunk8(wqT * SW, NCH).astype(E4),
        "wk8": chunk8(wkT * SW, NCH).astype(E4),
        "wv8": chunk8(wvT * SW, NCH).astype(E4),
        "wo8": chunk8(woT * SW, NCH).astype(E4),
        "w1h": chunk8(w1T * SW, NCH).astype(E4),
        "w2h": chunk8(w2hT.astype(np.float32), NHC).astype(E4),
        "w2l": chunk8(w2lT.astype(np.float32), NHC).astype(E4),
        "bqp": pmaj(bq, NCH), "bkp": pmaj(bk, NCH),
        "bvS": (np.asarray(bv, np.float32) * SX * SW).reshape(1, D).astype(NF),
        "b1p": pmaj(b1f, NHC),
        "boS": pmaj(np.asarray(b_out, np.float32), NCH),
        "bb2p": pmaj(bb2, NCH),
        "g1p": pmaj(g1, NCH),
        "g2p": pmaj(g2, NCH), "be2p": pmaj(beta2, NCH),
        "masku": mu.astype(NF), "maskw": mwr.astype(NF),
        "onesrow": np.ones((1, 128), np.float32).astype(NF),
        "onescol": np.ones((128, 1), np.float32).astype(NF),
    }
    flags = (
        bool(np.all(np.asarray(bv) == 0)),                       # zbv
        bool(np.all(b1f == 0)),                                  # zb1
        bool(np.all(np.asarray(b_out) == 0)),                    # zbo
        bool(np.all(bb2 == 0)
             and np.all(np.asarray(g1, np.float32) == 1.0)),     # zb2a1
        bool(np.all(np.asarray(beta2) == 0)
             and np.all(np.asarray(g2, np.float32) == 1.0)),     # za2
    )
    return shared, flags


def make_in_maps(inputs):
    ff = np.asarray(inputs["frame_features"], np.float32)
    fi = np.asarray(inputs["frame_indices"])
    shared, flags = _prep_shared(
        np.asarray(inputs["w_qkv"]), np.asarray(inputs["b_qkv"]),
        np.asarray(inputs["w_out"]), np.asarray(inputs["b_out"]),
        np.asarray(inputs["w1"]), np.asarray(inputs["b1"]),
        np.asarray(inputs["w2"]), np.asarray(inputs["b2"]),
        np.asarray(inputs["g1"]), np.asarray(inputs["beta1"]),
        np.asarray(inputs["g2"]), np.asarray(inputs["beta2"]))

    div = np.exp(np.float32(-np.log(10000.0))
                 * np.arange(0, D, 2, dtype=np.float32) / np.float32(D))
    in_maps = []
    for b in range(B):
        pos = np.asarray(fi[b], np.float32)[:, None]
        ang = pos * div[None, :]
        pe = np.empty((T, D), np.float32)
        pe[:, 0::2] = np.sin(ang)
        pe[:, 1::2] = np.cos(ang)
        xpe = ff[b] + pe                       # [T, D]
        xpeT = np.ascontiguousarray(xpe.T)     # [D, T]
        x8 = np.ascontiguousarray(
            (xpeT * SX).reshape(NCH, 128, T).transpose(1, 0, 2)).astype(E4)
        xsc = np.ascontiguousarray(
            xpeT.reshape(NCH, 128, T).transpose(1, 0, 2)).astype(NF)
        m = dict(shared)
        m["x8"] = x8
        m["xs"] = xsc
        in_maps.append(m)
    return in_maps, flags


def get_nc(flags=(True, True, True, True, True)):
    if flags not in _NC_CACHE:
        _NC_CACHE[flags] = build_nc(flags)
    return _NC_CACHE[flags]


def kernel(**inputs) -> np.ndarray:
    in_maps, flags = make_in_maps(inputs)
    nc = get_nc(flags)
    res = run_bass_kernel_spmd(nc, in_maps, core_ids=list(range(B)))
    outs = []
    for r in res.results:
        ob = np.asarray(r["outb"])             # [128, NCH, T] fp16
        oT = ob.transpose(1, 0, 2).reshape(D, T)
        outs.append(oT.T.astype(np.float32))
    return np.ascontiguousarray(np.stack(outs))


# revision 35
# speedup vs baseline: 1.2541x; 1.0013x over previous
"""Trainium2 Bass kernel for a local-window-attention transformer block (v4).

Sharding: data-parallel over batch (one batch element per NeuronCore).

v4 vs v2 (604us -> 482us TimelineSim per core):
- fp16 replaces bf16 on the whole residual/attention path (same engine
  cost, ~8x finer mantissa), buying accuracy budget to cut PE work:
  * w1 runs as fp8 "x2b": y8/y8l hi-lo activations against a single fp8
    w1h (w1l is gone entirely, freeing 32KB/partition of SBUF),
  * w2 runs as fp8 "x3": h8/h8l hi-lo against w2h + h8 against w2l,
  * the out-projection moving operand is the fp16 attention output
    requantized per-slab to fp8 hi/lo (os8/os8l) against fp8 wo,
  all DoubleRow at 0.5 cycles/row instead of 1.0 bf16.
- scale-free residual chain (no CS1 prescaling of xs/eps).
- attention runs per (128-token block, 4-head group) with every PSUM stage
  (scores, zsum/broadcast, PV) on its own 1-bank double-buffered tag, so
  the exp -> zsum -> reciprocal -> broadcast -> PV chains of consecutive
  groups overlap.
- macro34 is software-pipelined as: oproj(s+1) | requant(s+2) | w1(s) |
  ln1(s+1) | w2(s) | ln2(s-1), which gives each LayerNorm's long
  Act/DVE chain a full w2 phase to hide behind; sq/sq2 are computed at
  their LN sites so one buffer suffices.
- w2h streams during the attention phase (space freed by dropping w1l);
  only w2l loads at the phase boundary, and w2 accumulation orders the
  w2l-reads last to cover its DMA.
- QKV/attention PSUM drains alternate between Act and DVE per head so
  neither engine is the macro12 bottleneck.
"""
import numpy as np
import ml_dtypes

import concourse.bass as bass
import concourse.bacc as bacc
import concourse.mybir as mybir
import concourse.tile as tile
from concourse.bass import ts
from concourse.bass_utils import run_bass_kernel_spmd

F32 = mybir.dt.float32
F16 = mybir.dt.float16
FP8 = mybir.dt.float8e4
AF = mybir.ActivationFunctionType
OP = mybir.AluOpType
DR = mybir.MatmulPerfMode.DoubleRow
NF = np.float16
E4 = ml_dtypes.float8_e4m3

B, T, D, W, H = 8, 2048, 1024, 16, 8
HD = D // H            # 128 = head dim = one partition chunk
FF = 4 * D             # 4096
NCH = D // 128         # 8 feature chunks
NHC = FF // 128        # 32 hidden chunks
EPS = 1e-5
ISQ = float(1.0 / np.sqrt(128.0))
MASKC = 340.0

SX = 16.0              # x+pe fp8 scale
SW = 64.0              # weight fp8 scale
SH = 32.0              # LN1-out / gelu-out fp8 scale
SO = 32.0              # attention-out fp8 requant scale

SA = 512               # macro12 token slab
NSA = T // SA
SB = 256               # macro34 token slab
NSB = T // SB

_NC_CACHE = {}


def build_nc(flags):
    (zbv, zb1, zbo, zb2a1, za2) = flags
    nc = bacc.Bacc(None, target_bir_lowering=False)

    dram = {}
    # ---- per-core inputs ----
    dram["x8"] = nc.declare_dram_parameter("x8", [128, NCH, T], FP8,
                                           isOutput=False)
    dram["xs"] = nc.declare_dram_parameter("xs", [128, NCH, T], F16,
                                           isOutput=False)
    # ---- shared weights ----
    for nm, sh, dt in (
            ("wq8", [128, NCH, D], FP8), ("wk8", [128, NCH, D], FP8),
            ("wv8", [128, NCH, D], FP8), ("wo8", [128, NCH, D], FP8),
            ("w1h", [128, NCH, FF], FP8),
            ("w2h", [128, NHC, D], FP8), ("w2l", [128, NHC, D], FP8),
            ("bqp", [128, NCH], F32), ("bkp", [128, NCH], F32),
            ("bvS", [1, D], F16), ("b1p", [128, NHC], F32),
            ("boS", [128, NCH], F32), ("bb2p", [128, NCH], F32),
            ("g1p", [128, NCH], F32), ("g2p", [128, NCH], F32),
            ("be2p", [128, NCH], F32),
            ("masku", [9, 128], F16), ("maskw", [9, 512], F16),
            ("onesrow", [1, 128], F16), ("onescol", [128, 1], F16)):
        dram[nm] = nc.declare_dram_parameter(nm, sh, dt, isOutput=False)

    dram["outb"] = nc.declare_dram_parameter("outb", [128, NCH, T], F16,
                                             isOutput=True)

    with tile.TileContext(nc) as tc:
        _emit(nc, tc, flags, dram)
    nc.compile()
    return nc


def _emit(nc, tc, flags, dram):
    (zbv, zb1, zbo, zb2a1, za2) = flags
    open_pools = {}

    def popen(name, **kw):
        cm = tc.tile_pool(name=name, **kw)
        pool = cm.__enter__()
        open_pools[name] = cm
        return pool

    def pclose(name):
        open_pools.pop(name).__exit__(None, None, None)

    # ---- constants (live whole kernel) ----
    consts = popen("consts", bufs=1)

    pending_cdma = []

    def cdma(name, shape, dt):
        t = consts.tile(shape, dt, tag=f"c_{name}", name=f"c_{name}")
        pending_cdma.append((t, name))
        return t

    c_bq = cdma("bqp", [128, NCH], F32)
    c_bk = cdma("bkp", [128, NCH], F32)
    c_bv = None if zbv else cdma("bvS", [1, D], F16)
    c_mu = cdma("masku", [9, 128], F16)
    c_mw = cdma("maskw", [9, 512], F16)
    c_or = cdma("onesrow", [1, 128], F16)
    c_oc = cdma("onescol", [128, 1], F16)
    c_eps = consts.tile([1, 1], F32, name="c_eps")
    nc.vector.memset(c_eps, EPS)

    # attention output (fp16, full residency)
    osp = popen("osp", bufs=1)
    os16 = osp.tile([128, NCH, T], F16, name="os16")

    # macro34 weights minus w2l: space reserved up-front so their DMAs can
    # stream during macro12 instead of waiting for its pools to die.
    w1p = popen("w1p", bufs=1)
    wo_sb = w1p.tile([128, NCH, D], FP8, name="wo_sb")
    w1h_sb = w1p.tile([128, NCH, FF], FP8, name="w1h_sb")
    w2h_sb = w1p.tile([128, NHC, D], FP8, name="w2h_sb")

    wqp = popen("wqp", bufs=1)
    wq_sb = wqp.tile([128, NCH, D], FP8, name="wq_sb")
    nc.sync.dma_start(out=wq_sb, in_=dram["wq8"][:, :, :])
    early = [p for p in pending_cdma if p[1] in ("bqp", "bkp")]
    for (t, name) in early:
        nc.sync.dma_start(out=t, in_=dram[name][:, :])
        pending_cdma.remove((t, name))
    wk_sb = wqp.tile([128, NCH, D], FP8, name="wk_sb")
    nc.sync.dma_start(out=wk_sb, in_=dram["wk8"][:, :, :])
    wv_sb = wqp.tile([128, NCH, D], FP8, name="wv_sb")
    nc.sync.dma_start(out=wv_sb, in_=dram["wv8"][:, :, :])
    for (t, name) in pending_cdma:
        nc.sync.dma_start(out=t, in_=dram[name][:, :])
    pending_cdma.clear()
    nc.sync.dma_start(out=wo_sb, in_=dram["wo8"][:, :, :])
    for q in range(4):
        nc.sync.dma_start(out=w1h_sb[:, 2 * q:2 * q + 2, :],
                          in_=dram["w1h"][:, 2 * q:2 * q + 2, :])
    # stream w2h during macro12 (fills DMA idle; ready before macro34)
    for q in range(8):
        nc.sync.dma_start(out=w2h_sb[:, 4 * q:4 * q + 4, :],
                          in_=dram["w2h"][:, 4 * q:4 * q + 4, :])

    # ================= macro 1+2: QKV + attention =================
    pa = popen("pa", bufs=2)
    psa = popen("psa", bufs=1, space="PSUM")

    def emit_qkv_pieces(s):
        tsl = ts(s, SA)
        x8t = pa.tile([128, NCH, SA], FP8, tag="x8t")
        nc.scalar.dma_start(out=x8t, in_=dram["x8"][:, :, tsl])

        qst = pa.tile([128, NCH, SA], F16, tag="qst")
        kst = pa.tile([128, NCH, SA], F16, tag="kst")
        vst = pa.tile([128, SA // 128, D], F16, tag="vst")
        pieces = []

        def qk_head(w_sb, cbias, dst, h):
            ps = psa.tile([128, SA], F32, tag="sps", bufs=2, name="sps")
            for i in range(NCH // 2):
                nc.tensor.matmul(ps, w_sb[:, 2 * i:2 * i + 2, ts(h, 128)],
                                 x8t[:, 2 * i:2 * i + 2, :],
                                 start=i == 0, stop=i == NCH // 2 - 1,
                                 perf_mode=DR)
            if h % 2 == 0:
                nc.scalar.activation(out=dst[:, h, :], in_=ps,
                                     func=AF.Identity,
                                     bias=cbias[:, h:h + 1],
                                     scale=1.0 / (SX * SW))
            else:
                nc.vector.tensor_scalar(out=dst[:, h, :], in0=ps,
                                        scalar1=1.0 / (SX * SW),
                                        scalar2=cbias[:, h:h + 1],
                                        op0=OP.mult, op1=OP.add)

        def v_piece(tb, nb):
            ps = psa.tile([128, 512], F32, tag="sps", bufs=2, name="spsv")
            nkp = NCH // 2
            for i in range(nkp):
                last = (i == nkp - 1) and zbv
                nc.tensor.matmul(ps, x8t[:, 2 * i:2 * i + 2, ts(tb, 128)],
                                 wv_sb[:, 2 * i:2 * i + 2, ts(nb, 512)],
                                 start=i == 0, stop=last, perf_mode=DR)
            if not zbv:
                nc.tensor.matmul(ps, c_or, c_bv[:, ts(nb, 512)],
                                 start=False, stop=True)
            if nb == 0:
                nc.scalar.activation(out=vst[:, tb, ts(nb, 512)],
                                     in_=ps, func=AF.Identity,
                                     scale=1.0 / (SX * SW))
            else:
                nc.vector.tensor_scalar(out=vst[:, tb, ts(nb, 512)],
                                        in0=ps, scalar1=1.0 / (SX * SW),
                                        scalar2=None, op0=OP.mult)

        for (w_sb, cbias, dst) in ((wq_sb, c_bq, qst), (wk_sb, c_bk, kst)):
            for h in range(H):
                pieces.append(lambda w=w_sb, c=cbias, d=dst, hh=h:
                              qk_head(w, c, d, hh))
        for tb in range(SA // 128):
            for nb in range(2):
                pieces.append(lambda t=tb, n=nb: v_piece(t, n))
        return (qst, kst, vst), pieces

    def emit_attn_groups(s, qkv):
        qst, kst, vst = qkv

        def group(tb, hf):
            hs = slice(4 * hf, 4 * hf + 4)
            et_ps = psa.tile([128, 4, 128], F32, tag="et_ps", bufs=2,
                             name="et_ps")
            ets = pa.tile([128, 4, 128], F16, tag="ets", bufs=4)
            nc.tensor.matmul(et_ps, c_mu, c_mw, start=True, stop=False,
                             skip_group_check=True)
            for hh in range(4):
                h = 4 * hf + hh
                nc.tensor.matmul(
                    et_ps[:, hh, :],
                    kst[:, h, ts(tb, 128)], qst[:, h, ts(tb, 128)],
                    start=False, stop=hh == 3, skip_group_check=True)
            nc.scalar.activation(out=ets, in_=et_ps, func=AF.Exp,
                                 scale=ISQ)
            rb_ps = psa.tile([128, 4, 128], F32, tag="rb_ps", bufs=2,
                             name="rb_ps")
            # z row lives on partition 0 of rb_ps until the broadcast
            # matmul overwrites it (tile deps serialize the reciprocal
            # before it).
            nc.tensor.matmul(rb_ps[0:1, :, :], c_oc, ets,
                             start=True, stop=True)
            rz = pa.tile([1, 4, 128], F16, tag="rz", bufs=4)
            with nc.allow_low_precision(reason="softmax renorm in fp16"):
                nc.vector.reciprocal(out=rz, in_=rb_ps[0:1, :, :])
            nc.tensor.matmul(rb_ps, c_or, rz, start=True, stop=True)
            rbs = pa.tile([128, 4, 128], F16, tag="rbs", bufs=4)
            nc.scalar.copy(out=rbs, in_=rb_ps)
            o_ps = psa.tile([128, 4, 128], F32, tag="o_ps", bufs=2,
                            name="o_ps")
            for hh in range(4):
                h = 4 * hf + hh
                nc.tensor.matmul(o_ps[:, hh, :], vst[:, tb, ts(h, 128)],
                                 ets[:, hh, :], start=hh == 0,
                                 stop=hh == 3)
            tok = ts(s * (SA // 128) + tb, 128)
            nc.vector.tensor_mul(out=os16[:, hs, tok], in0=o_ps,
                                 in1=rbs)

        return [lambda t=tb, h=hf: group(t, h)
                for tb in range(SA // 128) for hf in range(2)]

    # interleave attn(s-1) groups between qkv(s) pieces so the PE always
    # has projection matmuls to run while the softmax chains resolve.
    qkv_live = None
    for s in range(NSA):
        qkv_now, pieces = emit_qkv_pieces(s)
        groups = emit_attn_groups(s - 1, qkv_live) if qkv_live else []
        gi = iter(groups)
        for i, pc in enumerate(pieces):
            pc()
            if i % 3 == 2:
                g = next(gi, None)
                if g:
                    g()
        for g in gi:
            g()
        qkv_live = qkv_now
    for g in emit_attn_groups(NSA - 1, qkv_live):
        g()

    pclose("psa")
    pclose("pa")
    pclose("wqp")

    # ================= macro 3+4 (w2l streams at the boundary) ====
    w2p = popen("w2p", bufs=1)
    w2l_sb = w2p.tile([128, NHC, D], FP8, name="w2l_sb")
    for q in range(8):
        nc.sync.dma_start(out=w2l_sb[:, 4 * q:4 * q + 4, :],
                          in_=dram["w2l"][:, 4 * q:4 * q + 4, :])
    c_b1 = consts.tile([128, NHC], F32, name="c_b1")
    nc.sync.dma_start(out=c_b1, in_=dram["b1p"][:, :])
    c_bo = cdma("boS", [128, NCH], F32)
    c_bb2 = cdma("bb2p", [128, NCH], F32)
    c_g1 = cdma("g1p", [128, NCH], F32)
    c_g2 = cdma("g2p", [128, NCH], F32)
    c_be2 = cdma("be2p", [128, NCH], F32)
    for (t, name) in pending_cdma:
        nc.sync.dma_start(out=t, in_=dram[name][:, :])
    pending_cdma.clear()

    pb = popen("pb", bufs=2)
    psb = popen("psb", bufs=1, space="PSUM")

    def half_ps(nm):
        return psb.tile([128, 4, SB], F32, tag="half_ps", bufs=3, name=nm)

    def emit_requant(s):
        tsl = ts(s, SB)
        xst = pb.tile([128, NCH, SB], F16, tag="xst", bufs=2)
        nc.scalar.dma_start(out=xst, in_=dram["xs"][:, :, tsl])
        os8 = pb.tile([128, NCH, SB], FP8, tag="os8", bufs=2)
        nc.scalar.activation(out=os8, in_=os16[:, :, tsl], func=AF.Identity,
                             scale=SO)
        os8l = pb.tile([128, NCH, SB], FP8, tag="os8l", bufs=2)
        nc.vector.scalar_tensor_tensor(out=os8l, in0=os16[:, :, tsl],
                                       scalar=SO, in1=os8,
                                       op0=OP.mult, op1=OP.subtract)
        return {"xst": xst, "os8": os8, "os8l": os8l}

    def emit_oproj(s, st):
        tsl = ts(s, SB)
        xst, os8, os8l = st["xst"], st["os8"], st["os8l"]

        hpre = pb.tile([128, NCH, SB], F16, tag="hpre", bufs=1)
        for half in range(2):
            po = half_ps("po")
            for dq in range(4):
                dc = 4 * half + dq
                first = dq % 2 == 0
                for xa in (os8, os8l):
                    for i in range(NCH // 2):
                        nc.tensor.matmul(
                            po[:, dq, :],
                            wo_sb[:, 2 * i:2 * i + 2, ts(dc, 128)],
                            xa[:, 2 * i:2 * i + 2, :],
                            start=first,
                            stop=(dq % 2 == 1 and xa is os8l
                                  and i == NCH // 2 - 1),
                            perf_mode=DR)
                        first = False
            sl = slice(4 * half, 4 * half + 4)
            if zbo:
                nc.vector.scalar_tensor_tensor(
                    out=hpre[:, sl, :], in0=po, scalar=1.0 / (SO * SW),
                    in1=xst[:, sl, :], op0=OP.mult, op1=OP.add)
            else:
                for dq in range(4):
                    dc = 4 * half + dq
                    yo = pb.tile([128, SB], F16, tag="yg", bufs=2, name="yo")
                    nc.vector.tensor_scalar(out=yo, in0=xst[:, dc, :],
                                            scalar1=1.0,
                                            scalar2=c_bo[:, dc:dc + 1],
                                            op0=OP.mult, op1=OP.add)
                    nc.vector.scalar_tensor_tensor(
                        out=hpre[:, dc, :], in0=po[:, dq, :],
                        scalar=1.0 / (SO * SW), in1=yo,
                        op0=OP.mult, op1=OP.add)
        return {"hpre": hpre}

    def emit_ln1(s, st):
        hpre = st["hpre"]
        sq = pb.tile([128, NCH, SB], F16, tag="sq", bufs=1)
        nc.scalar.activation(out=sq, in_=hpre, func=AF.Square)
        st_ps = psb.tile([1, 2, SB], F32, tag="st_ps", bufs=1, name="st_ps")
        for dc in range(NCH):
            nc.tensor.matmul(st_ps[:, 0, :], c_oc, hpre[:, dc, :],
                             start=dc == 0, stop=False)
            nc.tensor.matmul(st_ps[:, 1, :], c_oc, sq[:, dc, :],
                             start=False, stop=dc == NCH - 1)
        rbm = _ln_stats(nc, pb, st_ps, c_eps, SB, "")
        bc_ps = psb.tile([128, 2, SB], F32, tag="bc_ps", bufs=1, name="bc_ps")
        nc.tensor.matmul(bc_ps[:, :, :], c_or, rbm, start=True, stop=True)
        bcs = pb.tile([128, 2, SB], F16, tag="bcs", bufs=2)
        nc.scalar.copy(out=bcs, in_=bc_ps)
        ys = pb.tile([128, NCH, SB], F16, tag="ys", bufs=2, name="ys")
        for dc in range(NCH):
            nc.vector.tensor_mul(out=ys[:, dc, :], in0=hpre[:, dc, :],
                                 in1=bcs[:, 0, :])
        for dc in range(NCH):
            nc.vector.tensor_sub(out=ys[:, dc, :], in0=ys[:, dc, :],
                                 in1=bcs[:, 1, :])
        y8 = pb.tile([128, NCH, SB], FP8, tag="y8", bufs=1)
        nc.scalar.activation(out=y8, in_=ys, func=AF.Identity, scale=SH)
        y8l = pb.tile([128, NCH, SB], FP8, tag="y8l", bufs=1)
        nc.vector.scalar_tensor_tensor(out=y8l, in0=ys, scalar=SH, in1=y8,
                                       op0=OP.mult, op1=OP.subtract)
        st.update(ys=ys, y8=y8, y8l=y8l)

    def emit_w1(s, st):
        y8, y8l = st["y8"], st["y8l"]
        h8 = pb.tile([128, NHC, SB], FP8, tag="h8", bufs=1)
        h8l = pb.tile([128, NHC, SB], FP8, tag="h8l", bufs=1)
        for g in range(NHC // 4):
            w1ps = half_ps("w1ps")
            for hh in range(4):
                hc = 4 * g + hh
                first = hh % 2 == 0
                for xa in (y8, y8l):
                    for i in range(NCH // 2):
                        nc.tensor.matmul(
                            w1ps[:, hh, :],
                            w1h_sb[:, 2 * i:2 * i + 2, ts(hc, 128)],
                            xa[:, 2 * i:2 * i + 2, :],
                            start=first,
                            stop=(hh % 2 == 1 and xa is y8l
                                  and i == NCH // 2 - 1),
                            perf_mode=DR)
                        first = False
            gsl = slice(4 * g, 4 * g + 4)
            g16 = pb.tile([128, 4, SB], F16, tag="g16", bufs=2)
            if zb1:
                nc.scalar.activation(out=g16, in_=w1ps, func=AF.Gelu,
                                     scale=1.0 / (SH * SW))
            else:
                for hh in range(4):
                    hc = 4 * g + hh
                    nc.scalar.activation(out=g16[:, hh, :],
                                         in_=w1ps[:, hh, :], func=AF.Gelu,
                                         bias=c_b1[:, hc:hc + 1],
                                         scale=1.0 / (SH * SW))
            nc.vector.tensor_scalar(out=h8[:, gsl, :], in0=g16,
                                    scalar1=SH, scalar2=None, op0=OP.mult)
            nc.vector.scalar_tensor_tensor(out=h8l[:, gsl, :], in0=g16,
                                           scalar=SH, in1=h8[:, gsl, :],
                                           op0=OP.mult, op1=OP.subtract)
        st["h8"] = h8
        st["h8l"] = h8l

    def emit_w2_half(s, st, half):
        h8, h8l, ys = st["h8"], st["h8l"], st["ys"]
        if half == 0:
            st["h2"] = pb.tile([128, NCH, SB], F16, tag="h2", bufs=2,
                               name="h2")
        h2 = st["h2"]
        w2ps = half_ps("w2ps")
        for dp in range(2):
            first = True
            for (xa, wa) in ((h8, w2h_sb), (h8l, w2h_sb), (h8, w2l_sb)):
                for dq in (2 * dp, 2 * dp + 1):
                    dc = 4 * half + dq
                    for i in range(NHC // 2):
                        nc.tensor.matmul(
                            w2ps[:, dq, :],
                            wa[:, 2 * i:2 * i + 2, ts(dc, 128)],
                            xa[:, 2 * i:2 * i + 2, :],
                            start=first,
                            stop=(dq == 2 * dp + 1 and wa is w2l_sb
                                  and i == NHC // 2 - 1),
                            perf_mode=DR)
                        first = False
        sl = slice(4 * half, 4 * half + 4)
        if zb2a1:
            nc.vector.scalar_tensor_tensor(
                out=h2[:, sl, :], in0=w2ps, scalar=1.0 / (SH * SW),
                in1=ys[:, sl, :], op0=OP.mult, op1=OP.add)
        else:
            for dq in range(4):
                dc = 4 * half + dq
                yg = pb.tile([128, SB], F16, tag="yg", bufs=2)
                nc.vector.tensor_scalar(out=yg, in0=ys[:, dc, :],
                                        scalar1=c_g1[:, dc:dc + 1],
                                        scalar2=c_bb2[:, dc:dc + 1],
                                        op0=OP.mult, op1=OP.add)
                nc.vector.scalar_tensor_tensor(
                    out=h2[:, dc, :], in0=w2ps[:, dq, :],
                    scalar=1.0 / (SH * SW), in1=yg,
                    op0=OP.mult, op1=OP.add)


    def emit_ln2(s, st):
        h2 = st["h2"]
        sq2 = pb.tile([128, NCH, SB], F16, tag="sq", bufs=1, name="sq2")
        nc.scalar.activation(out=sq2, in_=h2, func=AF.Square)
        tsl = ts(s, SB)
        st2_ps = psb.tile([1, 2, SB], F32, tag="st_ps", bufs=1, name="st2_ps")
        for dc in range(NCH):
            nc.tensor.matmul(st2_ps[:, 0, :], c_oc, h2[:, dc, :],
                             start=dc == 0, stop=False)
            nc.tensor.matmul(st2_ps[:, 1, :], c_oc, sq2[:, dc, :],
                             start=False, stop=dc == NCH - 1)
        rbm2 = _ln_stats(nc, pb, st2_ps, c_eps, SB, "")
        bc2_ps = psb.tile([128, 2, SB], F32, tag="bc_ps", bufs=1,
                          name="bc2_ps")
        nc.tensor.matmul(bc2_ps[:, :, :], c_or, rbm2, start=True, stop=True)
        bc2s = pb.tile([128, 2, SB], F16, tag="bcs", bufs=2, name="bc2s")
        nc.scalar.copy(out=bc2s, in_=bc2_ps)
        yout = pb.tile([128, NCH, SB], F16, tag="xst", bufs=2, name="yout")
        for dc in range(NCH):
            nc.vector.tensor_mul(out=yout[:, dc, :], in0=h2[:, dc, :],
                                 in1=bc2s[:, 0, :])
        for dc in range(NCH):
            nc.vector.tensor_sub(out=yout[:, dc, :], in0=yout[:, dc, :],
                                 in1=bc2s[:, 1, :])
            if not za2:
                nc.vector.tensor_scalar(out=yout[:, dc, :],
                                        in0=yout[:, dc, :],
                                        scalar1=c_g2[:, dc:dc + 1],
                                        scalar2=c_be2[:, dc:dc + 1],
                                        op0=OP.mult, op1=OP.add)
        nc.scalar.dma_start(out=dram["outb"][:, :, tsl], in_=yout)

    rq = {0: emit_requant(0)}
    states = {0: emit_oproj(0, rq.pop(0))}
    emit_ln1(0, states[0])
    rq[1] = emit_requant(1)
    for s in range(NSB):
        st = states[s]
        if s + 1 < NSB:
            states[s + 1] = emit_oproj(s + 1, rq.pop(s + 1))
        if s + 2 < NSB:
            rq[s + 2] = emit_requant(s + 2)
        emit_w1(s, st)
        if s + 1 < NSB:
            emit_ln1(s + 1, states[s + 1])
        emit_w2_half(s, st, 0)
        emit_w2_half(s, st, 1)
        if s >= 1:
            emit_ln2(s - 1, states.pop(s - 1))
    emit_ln2(NSB - 1, states.pop(NSB - 1))

    pclose("psb")
    pclose("pb")
    pclose("w2p")
    pclose("w1p")
    pclose("osp")
    pclose("consts")


def _ln_stats(nc, pool, st_ps, eps_t, TW, tag):
    """stat psum [1, 2, TW] (sum, sumsq) -> rstd, bm rows (fp16)."""
    mean = pool.tile([1, TW], F16, tag=f"mean{tag}", bufs=1)
    nc.scalar.activation(out=mean, in_=st_ps[:, 0, :], func=AF.Identity,
                         scale=1.0 / D)
    msq = pool.tile([1, TW], F16, tag=f"msq{tag}", bufs=1)
    nc.vector.tensor_mul(out=msq, in0=mean, in1=mean)
    var = pool.tile([1, TW], F16, tag=f"var{tag}", bufs=1)
    nc.vector.scalar_tensor_tensor(out=var, in0=st_ps[:, 1, :],
                                   scalar=1.0 / D, in1=msq,
                                   op0=OP.mult, op1=OP.subtract)
    sd = pool.tile([1, TW], F16, tag=f"msq{tag}", bufs=1, name="sd")
    nc.scalar.activation(out=sd, in_=var, func=AF.Sqrt, bias=eps_t, scale=1.0)
    rbm = pool.tile([1, 2, TW], F16, tag=f"rbm{tag}", bufs=1, name="rbm")
    with nc.allow_low_precision(reason="LN broadcast rows in fp16"):
        nc.vector.reciprocal(out=rbm[:, 0, :], in_=sd)
    nc.vector.tensor_mul(out=rbm[:, 1, :], in0=mean, in1=rbm[:, 0, :])
    return rbm


# ======================= host side =======================

def _prep_shared(w_qkv, b_qkv, w_out, b_out, w1, b1, w2, b2,
                 g1, beta1, g2, beta2):
    wq, wk, wv = w_qkv[0:D], w_qkv[D:2 * D], w_qkv[2 * D:3 * D]
    bq, bk, bv = b_qkv[0:D], b_qkv[D:2 * D], b_qkv[2 * D:3 * D]

    def pmaj(v, n):
        return np.ascontiguousarray(
            np.asarray(v, np.float32).reshape(n, 128).T)

    def chunk8(wT, nk):
        # [K, M] -> [128, nk, M] (K = nk*128, chunk-major along K)
        return np.ascontiguousarray(
            wT.reshape(nk, 128, wT.shape[1]).transpose(1, 0, 2))

    wqT = np.ascontiguousarray(np.asarray(wq, np.float32).T)
    wkT = np.ascontiguousarray(np.asarray(wk, np.float32).T)
    wvT = np.ascontiguousarray(np.asarray(wv, np.float32).T)
    woT = np.ascontiguousarray(np.asarray(w_out, np.float32).T)
    w1g = np.asarray(w1, np.float32) * np.asarray(g1, np.float32)[None, :]
    w1T = np.ascontiguousarray(w1g.T)          # [D, FF]
    w2T = np.ascontiguousarray(np.asarray(w2, np.float32).T)  # [FF, D]

    w2s = w2T * SW
    w2hT = w2s.astype(E4)
    w2lT = (w2s - w2hT.astype(np.float32)).astype(E4)

    b1f = np.asarray(b1, np.float32) + w1g @ np.asarray(beta1, np.float32)
    bb2 = np.asarray(b2, np.float32) + np.asarray(beta1, np.float32)

    mu = np.zeros((9, 128), np.float32)
    mw = np.zeros((9, 128), np.float32)
    for w in range(8):
        mu[w, w * 16:(w + 1) * 16] = MASKC
        mw[w, w * 16:(w + 1) * 16] = 1.0
    mu[8, :] = -MASKC
    mw[8, :] = 1.0
    mwr = np.tile(mw, (1, 4))

    shared = {
        "wq8": chunk8(wqT * SW, NCH).astype(E4),
        "wk8": chunk8(wkT * SW, NCH).astype(E4),
        "wv8": chunk8(wvT * SW, NCH).astype(E4),
        "wo8": chunk8(woT * SW, NCH).astype(E4),
        "w1h": chunk8(w1T * SW, NCH).astype(E4),
        "w2h": chunk8(w2hT.astype(np.float32), NHC).astype(E4),
        "w2l": chunk8(w2lT.astype(np.float32), NHC).astype(E4),
        "bqp": pmaj(bq, NCH), "bkp": pmaj(bk, NCH),
        "bvS": (np.asarray(bv, np.float32) * SX * SW).reshape(1, D).astype(NF),
        "b1p": pmaj(b1f, NHC),
        "boS": pmaj(np.asarray(b_out, np.float32), NCH),
        "bb2p": pmaj(bb2, NCH),
        "g1p": pmaj(g1, NCH),
        "g2p": pmaj(g2, NCH), "be2p": pmaj(beta2, NCH),
        "masku": mu.astype(NF), "maskw": mwr.astype(NF),
        "onesrow": np.ones((1, 128), np.float32).astype(NF),
        "onescol": np.ones((128, 1), np.float32).astype(NF),
    }
    flags = (
        bool(np.all(np.asarray(bv) == 0)),                       # zbv
        bool(np.all(b1f == 0)),                                  # zb1
        bool(np.all(np.asarray(b_out) == 0)),                    # zbo
        bool(np.all(bb2 == 0)
             and np.all(np.asarray(g1, np.float32) == 1.0)),     # zb2a1
        bool(np.all(np.asarray(beta2) == 0)
             and np.all(np.asarray(g2, np.float32) == 1.0)),     # za2
    )
    return shared, flags


def make_in_maps(inputs):
    ff = np.asarray(inputs["frame_features"], np.float32)
    fi = np.asarray(inputs["frame_indices"])
    shared, flags = _prep_shared(
        np.asarray(inputs["w_qkv"]), np.asarray(inputs["b_qkv"]),
        np.asarray(inputs["w_out"]), np.asarray(inputs["b_out"]),
        np.asarray(inputs["w1"]), np.asarray(inputs["b1"]),
        np.asarray(inputs["w2"]), np.asarray(inputs["b2"]),
        np.asarray(inputs["g1"]), np.asarray(inputs["beta1"]),
        np.asarray(inputs["g2"]), np.asarray(inputs["beta2"]))

    div = np.exp(np.float32(-np.log(10000.0))
                 * np.arange(0, D, 2, dtype=np.float32) / np.float32(D))
    in_maps = []
    for b in range(B):
        pos = np.asarray(fi[b], np.float32)[:, None]
        ang = pos * div[None, :]
        pe = np.empty((T, D), np.float32)
        pe[:, 0::2] = np.sin(ang)
        pe[:, 1::2] = np.cos(ang)
        xpe = ff[b] + pe                       # [T, D]
        xpeT = np.ascontiguousarray(xpe.T)     # [D, T]
        x8 = np.ascontiguousarray(
            (xpeT * SX).reshape(NCH, 128, T).transpose(1, 0, 2)).astype(E4)
        xsc = np.ascontiguousarray(
            xpeT.reshape(NCH, 128, T).transpose(1, 0, 2)).astype(NF)
        m = dict(shared)
        m["x8"] = x8
        m["xs"] = xsc
        in_maps.append(m)
    return in_maps, flags


def get_nc(flags=(True, True, True, True, True)):
    if flags not in _NC_CACHE:
        _NC_CACHE[flags] = build_nc(flags)
    return _NC_CACHE[flags]


def kernel(**inputs) -> np.ndarray:
    in_maps, flags = make_in_maps(inputs)
    nc = get_nc(flags)
    res = run_bass_kernel_spmd(nc, in_maps, core_ids=list(range(B)))
    outs = []
    for r in res.results:
        ob = np.asarray(r["outb"])             # [128, NCH, T] fp16
        oT = ob.transpose(1, 0, 2).reshape(D, T)
        outs.append(oT.T.astype(np.float32))
    return np.ascontiguousarray(np.stack(outs))


# revision 37
# speedup vs baseline: 1.2629x; 1.0070x over previous
"""Trainium2 Bass kernel for a local-window-attention transformer block (v4).

Sharding: data-parallel over batch (one batch element per NeuronCore).

v4 vs v2 (604us -> 478us TimelineSim per core):
- fp16 replaces bf16 on the whole residual/attention path (same engine
  cost, ~8x finer mantissa), buying accuracy budget to cut PE work:
  * w1 runs as fp8 "x2b": y8/y8l hi-lo activations against a single fp8
    w1h (w1l is gone entirely, freeing 32KB/partition of SBUF),
  * w2 runs as fp8 "x3": h8/h8l hi-lo against w2h + h8 against w2l,
  * the out-projection moving operand is the fp16 attention output
    requantized per-slab to fp8 hi/lo (os8/os8l) against fp8 wo,
  all DoubleRow at 0.5 cycles/row instead of 1.0 bf16.
- scale-free residual chain (no CS1 prescaling of xs/eps).
- attention runs per (128-token block, 4-head group) with every PSUM stage
  (scores, zsum/broadcast, PV) on its own 1-bank double-buffered tag, so
  the exp -> zsum -> reciprocal -> broadcast -> PV chains of consecutive
  groups overlap.
- macro34 is software-pipelined as: oproj(s+1) | requant(s+2) | w1(s) |
  ln1(s+1) | w2(s) | ln2(s-1), which gives each LayerNorm's long
  Act/DVE chain a full w2 phase to hide behind; sq/sq2 are computed at
  their LN sites so one buffer suffices.
- w2h streams during the attention phase (space freed by dropping w1l);
  only w2l loads at the phase boundary, and w2 accumulation orders the
  w2l-reads last to cover its DMA.
- QKV/attention PSUM drains alternate between Act and DVE per head so
  neither engine is the macro12 bottleneck, and attention-group emission is
  interleaved with the next slab's QKV pieces so the PE always has
  projection matmuls to run while softmax chains resolve.
"""
import numpy as np
import ml_dtypes

import concourse.bass as bass
import concourse.bacc as bacc
import concourse.mybir as mybir
import concourse.tile as tile
from concourse.bass import ts
from concourse.bass_utils import run_bass_kernel_spmd

F32 = mybir.dt.float32
F16 = mybir.dt.float16
FP8 = mybir.dt.float8e4
AF = mybir.ActivationFunctionType
OP = mybir.AluOpType
DR = mybir.MatmulPerfMode.DoubleRow
NF = np.float16
E4 = ml_dtypes.float8_e4m3

B, T, D, W, H = 8, 2048, 1024, 16, 8
HD = D // H            # 128 = head dim = one partition chunk
FF = 4 * D             # 4096
NCH = D // 128         # 8 feature chunks
NHC = FF // 128        # 32 hidden chunks
EPS = 1e-5
ISQ = float(1.0 / np.sqrt(128.0))
MASKC = 340.0

SX = 16.0              # x+pe fp8 scale
SW = 64.0              # weight fp8 scale
SH = 32.0              # LN1-out / gelu-out fp8 scale
SO = 32.0              # attention-out fp8 requant scale

SA = 512               # macro12 token slab
NSA = T // SA
SB = 256               # macro34 token slab
NSB = T // SB

_NC_CACHE = {}


def build_nc(flags):
    (zbv, zb1, zbo, zb2a1, za2) = flags
    nc = bacc.Bacc(None, target_bir_lowering=False)

    dram = {}
    # ---- per-core inputs ----
    dram["x8"] = nc.declare_dram_parameter("x8", [128, NCH, T], FP8,
                                           isOutput=False)
    dram["xs"] = nc.declare_dram_parameter("xs", [128, NCH, T], F16,
                                           isOutput=False)
    # ---- shared weights ----
    for nm, sh, dt in (
            ("wq8", [128, NCH, D], FP8), ("wk8", [128, NCH, D], FP8),
            ("wv8", [128, NCH, D], FP8), ("wo8", [128, NCH, D], FP8),
            ("w1h", [128, NCH, FF], FP8),
            ("w2h", [128, NHC, D], FP8), ("w2l", [128, NHC, D], FP8),
            ("bqp", [128, NCH], F32), ("bkp", [128, NCH], F32),
            ("bvS", [1, D], F16), ("b1p", [128, NHC], F32),
            ("boS", [128, NCH], F32), ("bb2p", [128, NCH], F32),
            ("g1p", [128, NCH], F32), ("g2p", [128, NCH], F32),
            ("be2p", [128, NCH], F32),
            ("masku", [9, 128], F16), ("maskw", [9, 512], F16),
            ("onesrow", [1, 128], F16), ("onescol", [128, 1], F16)):
        dram[nm] = nc.declare_dram_parameter(nm, sh, dt, isOutput=False)

    dram["outb"] = nc.declare_dram_parameter("outb", [128, NCH, T], F16,
                                             isOutput=True)

    with tile.TileContext(nc) as tc:
        _emit(nc, tc, flags, dram)
    nc.compile()
    return nc


def _emit(nc, tc, flags, dram):
    (zbv, zb1, zbo, zb2a1, za2) = flags
    open_pools = {}

    def popen(name, **kw):
        cm = tc.tile_pool(name=name, **kw)
        pool = cm.__enter__()
        open_pools[name] = cm
        return pool

    def pclose(name):
        open_pools.pop(name).__exit__(None, None, None)

    # ---- constants (live whole kernel) ----
    consts = popen("consts", bufs=1)

    pending_cdma = []

    def cdma(name, shape, dt):
        t = consts.tile(shape, dt, tag=f"c_{name}", name=f"c_{name}")
        pending_cdma.append((t, name))
        return t

    c_bq = cdma("bqp", [128, NCH], F32)
    c_bk = cdma("bkp", [128, NCH], F32)
    c_bv = None if zbv else cdma("bvS", [1, D], F16)
    c_mu = cdma("masku", [9, 128], F16)
    c_mw = cdma("maskw", [9, 512], F16)
    c_or = cdma("onesrow", [1, 128], F16)
    c_oc = cdma("onescol", [128, 1], F16)
    c_eps = consts.tile([1, 1], F32, name="c_eps")
    nc.vector.memset(c_eps, EPS)

    # attention output (fp16, full residency)
    osp = popen("osp", bufs=1)
    os16 = osp.tile([128, NCH, T], F16, name="os16")

    # macro34 weights minus w2l: space reserved up-front so their DMAs can
    # stream during macro12 instead of waiting for its pools to die.
    w1p = popen("w1p", bufs=1)
    wo_sb = w1p.tile([128, NCH, D], FP8, name="wo_sb")
    w1h_sb = w1p.tile([128, NCH, FF], FP8, name="w1h_sb")
    w2h_sb = w1p.tile([128, NHC, D], FP8, name="w2h_sb")

    wqp = popen("wqp", bufs=1)
    wq_sb = wqp.tile([128, NCH, D], FP8, name="wq_sb")
    nc.sync.dma_start(out=wq_sb, in_=dram["wq8"][:, :, :])
    early = [p for p in pending_cdma if p[1] in ("bqp", "bkp")]
    for (t, name) in early:
        nc.sync.dma_start(out=t, in_=dram[name][:, :])
        pending_cdma.remove((t, name))
    wk_sb = wqp.tile([128, NCH, D], FP8, name="wk_sb")
    nc.sync.dma_start(out=wk_sb, in_=dram["wk8"][:, :, :])
    wv_sb = wqp.tile([128, NCH, D], FP8, name="wv_sb")
    nc.sync.dma_start(out=wv_sb, in_=dram["wv8"][:, :, :])
    for (t, name) in pending_cdma:
        nc.sync.dma_start(out=t, in_=dram[name][:, :])
    pending_cdma.clear()
    nc.sync.dma_start(out=wo_sb, in_=dram["wo8"][:, :, :])
    for q in range(4):
        nc.sync.dma_start(out=w1h_sb[:, 2 * q:2 * q + 2, :],
                          in_=dram["w1h"][:, 2 * q:2 * q + 2, :])
    # stream w2h during macro12 (fills DMA idle; ready before macro34)
    for q in range(8):
        nc.sync.dma_start(out=w2h_sb[:, 4 * q:4 * q + 4, :],
                          in_=dram["w2h"][:, 4 * q:4 * q + 4, :])

    # ================= macro 1+2: QKV + attention =================
    pa = popen("pa", bufs=2)
    psa = popen("psa", bufs=1, space="PSUM")

    def emit_qkv_pieces(s):
        tsl = ts(s, SA)
        x8t = pa.tile([128, NCH, SA], FP8, tag="x8t")
        nc.scalar.dma_start(out=x8t, in_=dram["x8"][:, :, tsl])

        qst = pa.tile([128, NCH, SA], F16, tag="qst")
        kst = pa.tile([128, NCH, SA], F16, tag="kst")
        vst = pa.tile([128, SA // 128, D], F16, tag="vst")
        pieces = []

        def qk_head(w_sb, cbias, dst, h):
            ps = psa.tile([128, SA], F32, tag="sps", bufs=2, name="sps")
            for i in range(NCH // 2):
                nc.tensor.matmul(ps, w_sb[:, 2 * i:2 * i + 2, ts(h, 128)],
                                 x8t[:, 2 * i:2 * i + 2, :],
                                 start=i == 0, stop=i == NCH // 2 - 1,
                                 perf_mode=DR)
            if h % 2 == 0:
                nc.scalar.activation(out=dst[:, h, :], in_=ps,
                                     func=AF.Identity,
                                     bias=cbias[:, h:h + 1],
                                     scale=1.0 / (SX * SW))
            else:
                nc.vector.tensor_scalar(out=dst[:, h, :], in0=ps,
                                        scalar1=1.0 / (SX * SW),
                                        scalar2=cbias[:, h:h + 1],
                                        op0=OP.mult, op1=OP.add)

        def v_piece(tb, nb):
            ps = psa.tile([128, 512], F32, tag="sps", bufs=2, name="spsv")
            nkp = NCH // 2
            for i in range(nkp):
                last = (i == nkp - 1) and zbv
                nc.tensor.matmul(ps, x8t[:, 2 * i:2 * i + 2, ts(tb, 128)],
                                 wv_sb[:, 2 * i:2 * i + 2, ts(nb, 512)],
                                 start=i == 0, stop=last, perf_mode=DR)
            if not zbv:
                nc.tensor.matmul(ps, c_or, c_bv[:, ts(nb, 512)],
                                 start=False, stop=True)
            if nb == 0:
                nc.scalar.activation(out=vst[:, tb, ts(nb, 512)],
                                     in_=ps, func=AF.Identity,
                                     scale=1.0 / (SX * SW))
            else:
                nc.vector.tensor_scalar(out=vst[:, tb, ts(nb, 512)],
                                        in0=ps, scalar1=1.0 / (SX * SW),
                                        scalar2=None, op0=OP.mult)

        for (w_sb, cbias, dst) in ((wq_sb, c_bq, qst), (wk_sb, c_bk, kst)):
            for h in range(H):
                pieces.append(lambda w=w_sb, c=cbias, d=dst, hh=h:
                              qk_head(w, c, d, hh))
        for tb in range(SA // 128):
            for nb in range(2):
                pieces.append(lambda t=tb, n=nb: v_piece(t, n))
        return (qst, kst, vst), pieces

    def emit_attn_groups(s, qkv):
        qst, kst, vst = qkv

        def group(tb, hf):
            hs = slice(4 * hf, 4 * hf + 4)
            et_ps = psa.tile([128, 4, 128], F32, tag="et_ps", bufs=2,
                             name="et_ps")
            ets = pa.tile([128, 4, 128], F16, tag="ets", bufs=4)
            nc.tensor.matmul(et_ps, c_mu, c_mw, start=True, stop=False,
                             skip_group_check=True)
            for hh in range(4):
                h = 4 * hf + hh
                nc.tensor.matmul(
                    et_ps[:, hh, :],
                    kst[:, h, ts(tb, 128)], qst[:, h, ts(tb, 128)],
                    start=False, stop=hh == 3, skip_group_check=True)
            nc.scalar.activation(out=ets, in_=et_ps, func=AF.Exp,
                                 scale=ISQ)
            rb_ps = psa.tile([128, 4, 128], F32, tag="rb_ps", bufs=2,
                             name="rb_ps")
            # z row lives on partition 0 of rb_ps until the broadcast
            # matmul overwrites it (tile deps serialize the reciprocal
            # before it).
            nc.tensor.matmul(rb_ps[0:1, :, :], c_oc, ets,
                             start=True, stop=True)
            rz = pa.tile([1, 4, 128], F16, tag="rz", bufs=4)
            with nc.allow_low_precision(reason="softmax renorm in fp16"):
                nc.vector.reciprocal(out=rz, in_=rb_ps[0:1, :, :])
            nc.tensor.matmul(rb_ps, c_or, rz, start=True, stop=True)
            rbs = pa.tile([128, 4, 128], F16, tag="rbs", bufs=4)
            nc.scalar.copy(out=rbs, in_=rb_ps)
            o_ps = psa.tile([128, 4, 128], F32, tag="o_ps", bufs=2,
                            name="o_ps")
            for hh in range(4):
                h = 4 * hf + hh
                nc.tensor.matmul(o_ps[:, hh, :], vst[:, tb, ts(h, 128)],
                                 ets[:, hh, :], start=hh == 0,
                                 stop=hh == 3)
            tok = ts(s * (SA // 128) + tb, 128)
            nc.vector.tensor_mul(out=os16[:, hs, tok], in0=o_ps,
                                 in1=rbs)

        return [lambda t=tb, h=hf: group(t, h)
                for tb in range(SA // 128) for hf in range(2)]

    # interleave attn(s-1) groups between qkv(s) pieces so the PE always
    # has projection matmuls to run while the softmax chains resolve.
    qkv_live = None
    for s in range(NSA):
        qkv_now, pieces = emit_qkv_pieces(s)
        groups = emit_attn_groups(s - 1, qkv_live) if qkv_live else []
        gi = iter(groups)
        for i, pc in enumerate(pieces):
            pc()
            if i % 3 == 2:
                g = next(gi, None)
                if g:
                    g()
        for g in gi:
            g()
        qkv_live = qkv_now
    for g in emit_attn_groups(NSA - 1, qkv_live):
        g()

    pclose("psa")
    pclose("pa")
    pclose("wqp")

    # ================= macro 3+4 (w2l streams at the boundary) ====
    w2p = popen("w2p", bufs=1)
    w2l_sb = w2p.tile([128, NHC, D], FP8, name="w2l_sb")
    for q in range(8):
        nc.sync.dma_start(out=w2l_sb[:, 4 * q:4 * q + 4, :],
                          in_=dram["w2l"][:, 4 * q:4 * q + 4, :])
    c_b1 = consts.tile([128, NHC], F32, name="c_b1")
    nc.sync.dma_start(out=c_b1, in_=dram["b1p"][:, :])
    c_bo = cdma("boS", [128, NCH], F32)
    c_bb2 = cdma("bb2p", [128, NCH], F32)
    c_g1 = cdma("g1p", [128, NCH], F32)
    c_g2 = cdma("g2p", [128, NCH], F32)
    c_be2 = cdma("be2p", [128, NCH], F32)
    for (t, name) in pending_cdma:
        nc.sync.dma_start(out=t, in_=dram[name][:, :])
    pending_cdma.clear()

    pb = popen("pb", bufs=2)
    psb = popen("psb", bufs=1, space="PSUM")

    def half_ps(nm):
        return psb.tile([128, 4, SB], F32, tag="half_ps", bufs=3, name=nm)

    def emit_requant(s):
        tsl = ts(s, SB)
        xst = pb.tile([128, NCH, SB], F16, tag="xst", bufs=2)
        nc.scalar.dma_start(out=xst, in_=dram["xs"][:, :, tsl])
        os8 = pb.tile([128, NCH, SB], FP8, tag="os8", bufs=2)
        nc.scalar.activation(out=os8, in_=os16[:, :, tsl], func=AF.Identity,
                             scale=SO)
        os8l = pb.tile([128, NCH, SB], FP8, tag="os8l", bufs=2)
        nc.vector.scalar_tensor_tensor(out=os8l, in0=os16[:, :, tsl],
                                       scalar=SO, in1=os8,
                                       op0=OP.mult, op1=OP.subtract)
        return {"xst": xst, "os8": os8, "os8l": os8l}

    def emit_oproj(s, st):
        tsl = ts(s, SB)
        xst, os8, os8l = st["xst"], st["os8"], st["os8l"]

        hpre = pb.tile([128, NCH, SB], F16, tag="hpre", bufs=1)
        for half in range(2):
            po = half_ps("po")
            for dq in range(4):
                dc = 4 * half + dq
                first = dq % 2 == 0
                for xa in (os8, os8l):
                    for i in range(NCH // 2):
                        nc.tensor.matmul(
                            po[:, dq, :],
                            wo_sb[:, 2 * i:2 * i + 2, ts(dc, 128)],
                            xa[:, 2 * i:2 * i + 2, :],
                            start=first,
                            stop=(dq % 2 == 1 and xa is os8l
                                  and i == NCH // 2 - 1),
                            perf_mode=DR)
                        first = False
            sl = slice(4 * half, 4 * half + 4)
            if zbo:
                nc.vector.scalar_tensor_tensor(
                    out=hpre[:, sl, :], in0=po, scalar=1.0 / (SO * SW),
                    in1=xst[:, sl, :], op0=OP.mult, op1=OP.add)
            else:
                for dq in range(4):
                    dc = 4 * half + dq
                    yo = pb.tile([128, SB], F16, tag="yg", bufs=2, name="yo")
                    nc.vector.tensor_scalar(out=yo, in0=xst[:, dc, :],
                                            scalar1=1.0,
                                            scalar2=c_bo[:, dc:dc + 1],
                                            op0=OP.mult, op1=OP.add)
                    nc.vector.scalar_tensor_tensor(
                        out=hpre[:, dc, :], in0=po[:, dq, :],
                        scalar=1.0 / (SO * SW), in1=yo,
                        op0=OP.mult, op1=OP.add)
        return {"hpre": hpre}

    def emit_ln1(s, st):
        hpre = st["hpre"]
        sq = pb.tile([128, NCH, SB], F16, tag="sq", bufs=1)
        nc.scalar.activation(out=sq, in_=hpre, func=AF.Square)
        st_ps = psb.tile([1, 2, SB], F32, tag="st_ps", bufs=1, name="st_ps")
        for dc in range(NCH):
            nc.tensor.matmul(st_ps[:, 0, :], c_oc, hpre[:, dc, :],
                             start=dc == 0, stop=False)
            nc.tensor.matmul(st_ps[:, 1, :], c_oc, sq[:, dc, :],
                             start=False, stop=dc == NCH - 1)
        rbm = _ln_stats(nc, pb, st_ps, c_eps, SB, "")
        bc_ps = psb.tile([128, 2, SB], F32, tag="bc_ps", bufs=1, name="bc_ps")
        nc.tensor.matmul(bc_ps[:, :, :], c_or, rbm, start=True, stop=True)
        bcs = pb.tile([128, 2, SB], F16, tag="bcs", bufs=1)
        nc.scalar.copy(out=bcs, in_=bc_ps)
        ys = pb.tile([128, NCH, SB], F16, tag="ys", bufs=2, name="ys")
        for dc in range(NCH):
            nc.vector.tensor_mul(out=ys[:, dc, :], in0=hpre[:, dc, :],
                                 in1=bcs[:, 0, :])
        for dc in range(NCH):
            nc.vector.tensor_sub(out=ys[:, dc, :], in0=ys[:, dc, :],
                                 in1=bcs[:, 1, :])
        y8 = pb.tile([128, NCH, SB], FP8, tag="y8", bufs=1)
        nc.scalar.activation(out=y8, in_=ys, func=AF.Identity, scale=SH)
        y8l = pb.tile([128, NCH, SB], FP8, tag="y8l", bufs=1)
        nc.vector.scalar_tensor_tensor(out=y8l, in0=ys, scalar=SH, in1=y8,
                                       op0=OP.mult, op1=OP.subtract)
        st.update(ys=ys, y8=y8, y8l=y8l)

    def emit_w1(s, st):
        y8, y8l = st["y8"], st["y8l"]
        h8 = pb.tile([128, NHC, SB], FP8, tag="h8", bufs=1)
        h8l = pb.tile([128, NHC, SB], FP8, tag="h8l", bufs=1)
        for g in range(NHC // 4):
            w1ps = half_ps("w1ps")
            for hh in range(4):
                hc = 4 * g + hh
                first = hh % 2 == 0
                for xa in (y8, y8l):
                    for i in range(NCH // 2):
                        nc.tensor.matmul(
                            w1ps[:, hh, :],
                            w1h_sb[:, 2 * i:2 * i + 2, ts(hc, 128)],
                            xa[:, 2 * i:2 * i + 2, :],
                            start=first,
                            stop=(hh % 2 == 1 and xa is y8l
                                  and i == NCH // 2 - 1),
                            perf_mode=DR)
                        first = False
            gsl = slice(4 * g, 4 * g + 4)
            g16 = pb.tile([128, 4, SB], F16, tag="g16", bufs=3)
            if zb1:
                nc.scalar.activation(out=g16, in_=w1ps, func=AF.Gelu,
                                     scale=1.0 / (SH * SW))
            else:
                for hh in range(4):
                    hc = 4 * g + hh
                    nc.scalar.activation(out=g16[:, hh, :],
                                         in_=w1ps[:, hh, :], func=AF.Gelu,
                                         bias=c_b1[:, hc:hc + 1],
                                         scale=1.0 / (SH * SW))
            nc.vector.tensor_scalar(out=h8[:, gsl, :], in0=g16,
                                    scalar1=SH, scalar2=None, op0=OP.mult)
            nc.vector.scalar_tensor_tensor(out=h8l[:, gsl, :], in0=g16,
                                           scalar=SH, in1=h8[:, gsl, :],
                                           op0=OP.mult, op1=OP.subtract)
        st["h8"] = h8
        st["h8l"] = h8l

    def emit_w2_half(s, st, half):
        h8, h8l, ys = st["h8"], st["h8l"], st["ys"]
        if half == 0:
            st["h2"] = pb.tile([128, NCH, SB], F16, tag="h2", bufs=2,
                               name="h2")
        h2 = st["h2"]
        w2ps = half_ps("w2ps")
        for dp in range(2):
            first = True
            for (xa, wa) in ((h8, w2h_sb), (h8l, w2h_sb), (h8, w2l_sb)):
                for dq in (2 * dp, 2 * dp + 1):
                    dc = 4 * half + dq
                    for i in range(NHC // 2):
                        nc.tensor.matmul(
                            w2ps[:, dq, :],
                            wa[:, 2 * i:2 * i + 2, ts(dc, 128)],
                            xa[:, 2 * i:2 * i + 2, :],
                            start=first,
                            stop=(dq == 2 * dp + 1 and wa is w2l_sb
                                  and i == NHC // 2 - 1),
                            perf_mode=DR)
                        first = False
        sl = slice(4 * half, 4 * half + 4)
        if zb2a1:
            nc.vector.scalar_tensor_tensor(
                out=h2[:, sl, :], in0=w2ps, scalar=1.0 / (SH * SW),
                in1=ys[:, sl, :], op0=OP.mult, op1=OP.add)
        else:
            for dq in range(4):
                dc = 4 * half + dq
                yg = pb.tile([128, SB], F16, tag="yg", bufs=2)
                nc.vector.tensor_scalar(out=yg, in0=ys[:, dc, :],
                                        scalar1=c_g1[:, dc:dc + 1],
                                        scalar2=c_bb2[:, dc:dc + 1],
                                        op0=OP.mult, op1=OP.add)
                nc.vector.scalar_tensor_tensor(
                    out=h2[:, dc, :], in0=w2ps[:, dq, :],
                    scalar=1.0 / (SH * SW), in1=yg,
                    op0=OP.mult, op1=OP.add)


    def emit_ln2(s, st):
        h2 = st["h2"]
        sq2 = pb.tile([128, NCH, SB], F16, tag="sq", bufs=1, name="sq2")
        nc.scalar.activation(out=sq2, in_=h2, func=AF.Square)
        tsl = ts(s, SB)
        st2_ps = psb.tile([1, 2, SB], F32, tag="st_ps", bufs=1, name="st2_ps")
        for dc in range(NCH):
            nc.tensor.matmul(st2_ps[:, 0, :], c_oc, h2[:, dc, :],
                             start=dc == 0, stop=False)
            nc.tensor.matmul(st2_ps[:, 1, :], c_oc, sq2[:, dc, :],
                             start=False, stop=dc == NCH - 1)
        rbm2 = _ln_stats(nc, pb, st2_ps, c_eps, SB, "")
        bc2_ps = psb.tile([128, 2, SB], F32, tag="bc_ps", bufs=1,
                          name="bc2_ps")
        nc.tensor.matmul(bc2_ps[:, :, :], c_or, rbm2, start=True, stop=True)
        bc2s = pb.tile([128, 2, SB], F16, tag="bcs", bufs=1, name="bc2s")
        nc.scalar.copy(out=bc2s, in_=bc2_ps)
        yout = pb.tile([128, NCH, SB], F16, tag="xst", bufs=2, name="yout")
        for dc in range(NCH):
            nc.vector.tensor_mul(out=yout[:, dc, :], in0=h2[:, dc, :],
                                 in1=bc2s[:, 0, :])
        for dc in range(NCH):
            nc.vector.tensor_sub(out=yout[:, dc, :], in0=yout[:, dc, :],
                                 in1=bc2s[:, 1, :])
            if not za2:
                nc.vector.tensor_scalar(out=yout[:, dc, :],
                                        in0=yout[:, dc, :],
                                        scalar1=c_g2[:, dc:dc + 1],
                                        scalar2=c_be2[:, dc:dc + 1],
                                        op0=OP.mult, op1=OP.add)
        nc.scalar.dma_start(out=dram["outb"][:, :, tsl], in_=yout)

    rq = {0: emit_requant(0)}
    states = {0: emit_oproj(0, rq.pop(0))}
    emit_ln1(0, states[0])
    rq[1] = emit_requant(1)
    for s in range(NSB):
        st = states[s]
        if s + 1 < NSB:
            states[s + 1] = emit_oproj(s + 1, rq.pop(s + 1))
        if s + 2 < NSB:
            rq[s + 2] = emit_requant(s + 2)
        emit_w1(s, st)
        if s + 1 < NSB:
            emit_ln1(s + 1, states[s + 1])
        emit_w2_half(s, st, 0)
        emit_w2_half(s, st, 1)
        if s >= 1:
            emit_ln2(s - 1, states.pop(s - 1))
    emit_ln2(NSB - 1, states.pop(NSB - 1))

    pclose("psb")
    pclose("pb")
    pclose("w2p")
    pclose("w1p")
    pclose("osp")
    pclose("consts")


def _ln_stats(nc, pool, st_ps, eps_t, TW, tag):
    """stat psum [1, 2, TW] (sum, sumsq) -> rstd, bm rows (fp16)."""
    mean = pool.tile([1, TW], F16, tag=f"mean{tag}", bufs=1)
    nc.scalar.activation(out=mean, in_=st_ps[:, 0, :], func=AF.Identity,
                         scale=1.0 / D)
    msq = pool.tile([1, TW], F16, tag=f"msq{tag}", bufs=1)
    nc.vector.tensor_mul(out=msq, in0=mean, in1=mean)
    var = pool.tile([1, TW], F16, tag=f"var{tag}", bufs=1)
    nc.vector.scalar_tensor_tensor(out=var, in0=st_ps[:, 1, :],
                                   scalar=1.0 / D, in1=msq,
                                   op0=OP.mult, op1=OP.subtract)
    sd = pool.tile([1, TW], F16, tag=f"msq{tag}", bufs=1, name="sd")
    nc.scalar.activation(out=sd, in_=var, func=AF.Sqrt, bias=eps_t, scale=1.0)
    rbm = pool.tile([1, 2, TW], F16, tag=f"rbm{tag}", bufs=1, name="rbm")
    with nc.allow_low_precision(reason="LN broadcast rows in fp16"):
        nc.vector.reciprocal(out=rbm[:, 0, :], in_=sd)
    nc.vector.tensor_mul(out=rbm[:, 1, :], in0=mean, in1=rbm[:, 0, :])
    return rbm


# ======================= host side =======================

def _prep_shared(w_qkv, b_qkv, w_out, b_out, w1, b1, w2, b2,
                 g1, beta1, g2, beta2):
    wq, wk, wv = w_qkv[0:D], w_qkv[D:2 * D], w_qkv[2 * D:3 * D]
    bq, bk, bv = b_qkv[0:D], b_qkv[D:2 * D], b_qkv[2 * D:3 * D]

    def pmaj(v, n):
        return np.ascontiguousarray(
            np.asarray(v, np.float32).reshape(n, 128).T)

    def chunk8(wT, nk):
        # [K, M] -> [128, nk, M] (K = nk*128, chunk-major along K)
        return np.ascontiguousarray(
            wT.reshape(nk, 128, wT.shape[1]).transpose(1, 0, 2))

    wqT = np.ascontiguousarray(np.asarray(wq, np.float32).T)
    wkT = np.ascontiguousarray(np.asarray(wk, np.float32).T)
    wvT = np.ascontiguousarray(np.asarray(wv, np.float32).T)
    woT = np.ascontiguousarray(np.asarray(w_out, np.float32).T)
    w1g = np.asarray(w1, np.float32) * np.asarray(g1, np.float32)[None, :]
    w1T = np.ascontiguousarray(w1g.T)          # [D, FF]
    w2T = np.ascontiguousarray(np.asarray(w2, np.float32).T)  # [FF, D]

    w2s = w2T * SW
    w2hT = w2s.astype(E4)
    w2lT = (w2s - w2hT.astype(np.float32)).astype(E4)

    b1f = np.asarray(b1, np.float32) + w1g @ np.asarray(beta1, np.float32)
    bb2 = np.asarray(b2, np.float32) + np.asarray(beta1, np.float32)

    mu = np.zeros((9, 128), np.float32)
    mw = np.zeros((9, 128), np.float32)
    for w in range(8):
        mu[w, w * 16:(w + 1) * 16] = MASKC
        mw[w, w * 16:(w + 1) * 16] = 1.0
    mu[8, :] = -MASKC
    mw[8, :] = 1.0
    mwr = np.tile(mw, (1, 4))

    shared = {
        "wq8": chunk8(wqT * SW, NCH).astype(E4),
        "wk8": chunk8(wkT * SW, NCH).astype(E4),
        "wv8": chunk8(wvT * SW, NCH).astype(E4),
        "wo8": chunk8(woT * SW, NCH).astype(E4),
        "w1h": chunk8(w1T * SW, NCH).astype(E4),
        "w2h": chunk8(w2hT.astype(np.float32), NHC).astype(E4),
        "w2l": chunk8(w2lT.astype(np.float32), NHC).astype(E4),
        "bqp": pmaj(bq, NCH), "bkp": pmaj(bk, NCH),
        "bvS": (np.asarray(bv, np.float32) * SX * SW).reshape(1, D).astype(NF),
        "b1p": pmaj(b1f, NHC),
        "boS": pmaj(np.asarray(b_out, np.float32), NCH),
        "bb2p": pmaj(bb2, NCH),
        "g1p": pmaj(g1, NCH),
        "g2p": pmaj(g2, NCH), "be2p": pmaj(beta2, NCH),
        "masku": mu.astype(NF), "maskw": mwr.astype(NF),
        "onesrow": np.ones((1, 128), np.float32).astype(NF),
        "onescol": np.ones((128, 1), np.float32).astype(NF),
    }
    flags = (
        bool(np.all(np.asarray(bv) == 0)),                       # zbv
        bool(np.all(b1f == 0)),                                  # zb1
        bool(np.all(np.asarray(b_out) == 0)),                    # zbo
        bool(np.all(bb2 == 0)
             and np.all(np.asarray(g1, np.float32) == 1.0)),     # zb2a1
        bool(np.all(np.asarray(beta2) == 0)
             and np.all(np.asarray(g2, np.float32) == 1.0)),     # za2
    )
    return shared, flags


def make_in_maps(inputs):
    ff = np.asarray(inputs["frame_features"], np.float32)
    fi = np.asarray(inputs["frame_indices"])
    shared, flags = _prep_shared(
        np.asarray(inputs["w_qkv"]), np.asarray(inputs["b_qkv"]),
        np.asarray(inputs["w_out"]), np.asarray(inputs["b_out"]),
        np.asarray(inputs["w1"]), np.asarray(inputs["b1"]),
        np.asarray(inputs["w2"]), np.asarray(inputs["b2"]),
        np.asarray(inputs["g1"]), np.asarray(inputs["beta1"]),
        np.asarray(inputs["g2"]), np.asarray(inputs["beta2"]))

    div = np.exp(np.float32(-np.log(10000.0))
                 * np.arange(0, D, 2, dtype=np.float32) / np.float32(D))
    in_maps = []
    for b in range(B):
        pos = np.asarray(fi[b], np.float32)[:, None]
        ang = pos * div[None, :]
        pe = np.empty((T, D), np.float32)
        pe[:, 0::2] = np.sin(ang)
        pe[:, 1::2] = np.cos(ang)
        xpe = ff[b] + pe                       # [T, D]
        xpeT = np.ascontiguousarray(xpe.T)     # [D, T]
        x8 = np.ascontiguousarray(
            (xpeT * SX).reshape(NCH, 128, T).transpose(1, 0, 2)).astype(E4)
        xsc = np.ascontiguousarray(
            xpeT.reshape(NCH, 128, T).transpose(1, 0, 2)).astype(NF)
        m = dict(shared)
        m["x8"] = x8
        m["xs"] = xsc
        in_maps.append(m)
    return in_maps, flags


def get_nc(flags=(True, True, True, True, True)):
    if flags not in _NC_CACHE:
        _NC_CACHE[flags] = build_nc(flags)
    return _NC_CACHE[flags]


def kernel(**inputs) -> np.ndarray:
    in_maps, flags = make_in_maps(inputs)
    nc = get_nc(flags)
    res = run_bass_kernel_spmd(nc, in_maps, core_ids=list(range(B)))
    outs = []
    for r in res.results:
        ob = np.asarray(r["outb"])             # [128, NCH, T] fp16
        oT = ob.transpose(1, 0, 2).reshape(D, T)
        outs.append(oT.T.astype(np.float32))
    return np.ascontiguousarray(np.stack(outs))


# revision 40
# speedup vs baseline: 1.2764x; 1.0107x over previous
"""Trainium2 Bass kernel for a local-window-attention transformer block (v4).

Sharding: data-parallel over batch (one batch element per NeuronCore).

v4 vs v2 (604us -> 473us TimelineSim per core):
- fp16 replaces bf16 on the whole residual/attention path (same engine
  cost, ~8x finer mantissa), buying accuracy budget to cut PE work:
  * w1 runs as fp8 "x2b": y8/y8l hi-lo activations against a single fp8
    w1h (w1l is gone entirely, freeing 32KB/partition of SBUF),
  * w2 runs as fp8 "x3": h8/h8l hi-lo against w2h + h8 against w2l,
  * the out-projection moving operand is the fp16 attention output
    requantized per-slab to fp8 hi/lo (os8/os8l) against fp8 wo,
  all DoubleRow at 0.5 cycles/row instead of 1.0 bf16.
- scale-free residual chain (no CS1 prescaling of xs/eps); LN2's sumsq
  reduction sums fp8 squares via DoubleRow (0.2% on var after averaging
  1024 terms — verified no max-err change).
- attention runs per (128-token block, 4-head group) with every PSUM stage
  (scores, zsum/broadcast, PV) on its own 1-bank double-buffered tag, so
  the exp -> zsum -> reciprocal -> broadcast -> PV chains of consecutive
  groups overlap.
- macro34 is software-pipelined as: oproj(s+1) | requant(s+2) | w1(s) |
  ln1(s+1) | w2(s) | ln2(s-1), which gives each LayerNorm's long
  Act/DVE chain a full w2 phase to hide behind; sq/sq2 are computed at
  their LN sites so one buffer suffices.
- w2h streams during the attention phase (space freed by dropping w1l);
  only w2l loads at the phase boundary, and w2 accumulation orders the
  w2l-reads last to cover its DMA.
- QKV/attention PSUM drains alternate between Act and DVE per head so
  neither engine is the macro12 bottleneck, and attention-group emission is
  interleaved with the next slab's QKV pieces so the PE always has
  projection matmuls to run while softmax chains resolve.
"""
import numpy as np
import ml_dtypes

import concourse.bass as bass
import concourse.bacc as bacc
import concourse.mybir as mybir
import concourse.tile as tile
from concourse.bass import ts
from concourse.bass_utils import run_bass_kernel_spmd

F32 = mybir.dt.float32
F16 = mybir.dt.float16
FP8 = mybir.dt.float8e4
AF = mybir.ActivationFunctionType
OP = mybir.AluOpType
DR = mybir.MatmulPerfMode.DoubleRow
NF = np.float16
E4 = ml_dtypes.float8_e4m3

B, T, D, W, H = 8, 2048, 1024, 16, 8
HD = D // H            # 128 = head dim = one partition chunk
FF = 4 * D             # 4096
NCH = D // 128         # 8 feature chunks
NHC = FF // 128        # 32 hidden chunks
EPS = 1e-5
ISQ = float(1.0 / np.sqrt(128.0))
MASKC = 340.0

SX = 16.0              # x+pe fp8 scale
SW = 64.0              # weight fp8 scale
SH = 32.0              # LN1-out / gelu-out fp8 scale
SO = 32.0              # attention-out fp8 requant scale

SA = 512               # macro12 token slab
NSA = T // SA
SB = 256               # macro34 token slab
NSB = T // SB

_NC_CACHE = {}


def build_nc(flags):
    (zbv, zb1, zbo, zb2a1, za2) = flags
    nc = bacc.Bacc(None, target_bir_lowering=False)

    dram = {}
    # ---- per-core inputs ----
    dram["x8"] = nc.declare_dram_parameter("x8", [128, NCH, T], FP8,
                                           isOutput=False)
    dram["xs"] = nc.declare_dram_parameter("xs", [128, NCH, T], F16,
                                           isOutput=False)
    # ---- shared weights ----
    for nm, sh, dt in (
            ("wq8", [128, NCH, D], FP8), ("wk8", [128, NCH, D], FP8),
            ("wv8", [128, NCH, D], FP8), ("wo8", [128, NCH, D], FP8),
            ("w1h", [128, NCH, FF], FP8),
            ("w2h", [128, NHC, D], FP8), ("w2l", [128, NHC, D], FP8),
            ("bqp", [128, NCH], F32), ("bkp", [128, NCH], F32),
            ("bvS", [1, D], F16), ("b1p", [128, NHC], F32),
            ("boS", [128, NCH], F32), ("bb2p", [128, NCH], F32),
            ("g1p", [128, NCH], F32), ("g2p", [128, NCH], F32),
            ("be2p", [128, NCH], F32),
            ("masku", [9, 128], F16), ("maskw", [9, 512], F16),
            ("onesrow", [1, 128], F16), ("onescol", [128, 1], F16)):
        dram[nm] = nc.declare_dram_parameter(nm, sh, dt, isOutput=False)

    dram["outb"] = nc.declare_dram_parameter("outb", [128, NCH, T], F16,
                                             isOutput=True)

    with tile.TileContext(nc) as tc:
        _emit(nc, tc, flags, dram)
    nc.compile()
    return nc


def _emit(nc, tc, flags, dram):
    (zbv, zb1, zbo, zb2a1, za2) = flags
    open_pools = {}

    def popen(name, **kw):
        cm = tc.tile_pool(name=name, **kw)
        pool = cm.__enter__()
        open_pools[name] = cm
        return pool

    def pclose(name):
        open_pools.pop(name).__exit__(None, None, None)

    # ---- constants (live whole kernel) ----
    consts = popen("consts", bufs=1)

    pending_cdma = []

    def cdma(name, shape, dt):
        t = consts.tile(shape, dt, tag=f"c_{name}", name=f"c_{name}")
        pending_cdma.append((t, name))
        return t

    c_bq = cdma("bqp", [128, NCH], F32)
    c_bk = cdma("bkp", [128, NCH], F32)
    c_bv = None if zbv else cdma("bvS", [1, D], F16)
    c_mu = cdma("masku", [9, 128], F16)
    c_mw = cdma("maskw", [9, 512], F16)
    c_or = cdma("onesrow", [1, 128], F16)
    c_oc = cdma("onescol", [128, 1], F16)
    c_eps = consts.tile([1, 1], F32, name="c_eps")
    nc.vector.memset(c_eps, EPS)
    # dual-fp8 LdWeights requires a full-width stationary (walrus
    # s3_lw_dual_fp8_restrictions rejects 1-column); use a 128-wide ones
    # block and read row 0 of the (all-equal) output.
    c_oc8 = consts.tile([128, 2, 128], FP8, name="c_oc8")
    nc.vector.memset(c_oc8, 1.0)

    # attention output (fp16, full residency)
    osp = popen("osp", bufs=1)
    os16 = osp.tile([128, NCH, T], F16, name="os16")

    # macro34 weights minus w2l: space reserved up-front so their DMAs can
    # stream during macro12 instead of waiting for its pools to die.
    w1p = popen("w1p", bufs=1)
    wo_sb = w1p.tile([128, NCH, D], FP8, name="wo_sb")
    w1h_sb = w1p.tile([128, NCH, FF], FP8, name="w1h_sb")
    w2h_sb = w1p.tile([128, NHC, D], FP8, name="w2h_sb")

    wqp = popen("wqp", bufs=1)
    wq_sb = wqp.tile([128, NCH, D], FP8, name="wq_sb")
    nc.sync.dma_start(out=wq_sb, in_=dram["wq8"][:, :, :])
    early = [p for p in pending_cdma if p[1] in ("bqp", "bkp")]
    for (t, name) in early:
        nc.sync.dma_start(out=t, in_=dram[name][:, :])
        pending_cdma.remove((t, name))
    wk_sb = wqp.tile([128, NCH, D], FP8, name="wk_sb")
    nc.sync.dma_start(out=wk_sb, in_=dram["wk8"][:, :, :])
    wv_sb = wqp.tile([128, NCH, D], FP8, name="wv_sb")
    nc.sync.dma_start(out=wv_sb, in_=dram["wv8"][:, :, :])
    for (t, name) in pending_cdma:
        nc.sync.dma_start(out=t, in_=dram[name][:, :])
    pending_cdma.clear()
    nc.sync.dma_start(out=wo_sb, in_=dram["wo8"][:, :, :])
    for q in range(4):
        nc.sync.dma_start(out=w1h_sb[:, 2 * q:2 * q + 2, :],
                          in_=dram["w1h"][:, 2 * q:2 * q + 2, :])
    # stream w2h during macro12 (fills DMA idle; ready before macro34)
    for q in range(8):
        nc.sync.dma_start(out=w2h_sb[:, 4 * q:4 * q + 4, :],
                          in_=dram["w2h"][:, 4 * q:4 * q + 4, :])

    # ================= macro 1+2: QKV + attention =================
    pa = popen("pa", bufs=2)
    psa = popen("psa", bufs=1, space="PSUM")

    def emit_qkv_pieces(s):
        tsl = ts(s, SA)
        x8t = pa.tile([128, NCH, SA], FP8, tag="x8t")
        nc.scalar.dma_start(out=x8t, in_=dram["x8"][:, :, tsl])

        qst = pa.tile([128, NCH, SA], F16, tag="qst")
        kst = pa.tile([128, NCH, SA], F16, tag="kst")
        vst = pa.tile([128, SA // 128, D], F16, tag="vst")
        pieces = []

        def qk_head(w_sb, cbias, dst, h):
            ps = psa.tile([128, SA], F32, tag="sps", bufs=2, name="sps")
            for i in range(NCH // 2):
                nc.tensor.matmul(ps, w_sb[:, 2 * i:2 * i + 2, ts(h, 128)],
                                 x8t[:, 2 * i:2 * i + 2, :],
                                 start=i == 0, stop=i == NCH // 2 - 1,
                                 perf_mode=DR)
            if h % 2 == 0:
                nc.scalar.activation(out=dst[:, h, :], in_=ps,
                                     func=AF.Identity,
                                     bias=cbias[:, h:h + 1],
                                     scale=1.0 / (SX * SW))
            else:
                nc.vector.tensor_scalar(out=dst[:, h, :], in0=ps,
                                        scalar1=1.0 / (SX * SW),
                                        scalar2=cbias[:, h:h + 1],
                                        op0=OP.mult, op1=OP.add)

        def v_piece(tb, nb):
            ps = psa.tile([128, 512], F32, tag="sps", bufs=2, name="spsv")
            nkp = NCH // 2
            for i in range(nkp):
                last = (i == nkp - 1) and zbv
                nc.tensor.matmul(ps, x8t[:, 2 * i:2 * i + 2, ts(tb, 128)],
                                 wv_sb[:, 2 * i:2 * i + 2, ts(nb, 512)],
                                 start=i == 0, stop=last, perf_mode=DR)
            if not zbv:
                nc.tensor.matmul(ps, c_or, c_bv[:, ts(nb, 512)],
                                 start=False, stop=True)
            if nb == 0:
                nc.scalar.activation(out=vst[:, tb, ts(nb, 512)],
                                     in_=ps, func=AF.Identity,
                                     scale=1.0 / (SX * SW))
            else:
                nc.vector.tensor_scalar(out=vst[:, tb, ts(nb, 512)],
                                        in0=ps, scalar1=1.0 / (SX * SW),
                                        scalar2=None, op0=OP.mult)

        for (w_sb, cbias, dst) in ((wq_sb, c_bq, qst), (wk_sb, c_bk, kst)):
            for h in range(H):
                pieces.append(lambda w=w_sb, c=cbias, d=dst, hh=h:
                              qk_head(w, c, d, hh))
        for tb in range(SA // 128):
            for nb in range(2):
                pieces.append(lambda t=tb, n=nb: v_piece(t, n))
        return (qst, kst, vst), pieces

    def emit_attn_groups(s, qkv):
        qst, kst, vst = qkv

        def group(tb, hf):
            hs = slice(4 * hf, 4 * hf + 4)
            et_ps = psa.tile([128, 4, 128], F32, tag="et_ps", bufs=2,
                             name="et_ps")
            ets = pa.tile([128, 4, 128], F16, tag="ets", bufs=4)
            nc.tensor.matmul(et_ps, c_mu, c_mw, start=True, stop=False,
                             skip_group_check=True)
            for hh in range(4):
                h = 4 * hf + hh
                nc.tensor.matmul(
                    et_ps[:, hh, :],
                    kst[:, h, ts(tb, 128)], qst[:, h, ts(tb, 128)],
                    start=False, stop=hh == 3, skip_group_check=True)
            nc.scalar.activation(out=ets, in_=et_ps, func=AF.Exp,
                                 scale=ISQ)
            rb_ps = psa.tile([128, 4, 128], F32, tag="rb_ps", bufs=2,
                             name="rb_ps")
            # z row lives on partition 0 of rb_ps until the broadcast
            # matmul overwrites it (tile deps serialize the reciprocal
            # before it).
            nc.tensor.matmul(rb_ps[0:1, :, :], c_oc, ets,
                             start=True, stop=True)
            rz = pa.tile([1, 4, 128], F16, tag="rz", bufs=4)
            with nc.allow_low_precision(reason="softmax renorm in fp16"):
                nc.vector.reciprocal(out=rz, in_=rb_ps[0:1, :, :])
            nc.tensor.matmul(rb_ps, c_or, rz, start=True, stop=True)
            rbs = pa.tile([128, 4, 128], F16, tag="rbs", bufs=4)
            nc.scalar.copy(out=rbs, in_=rb_ps)
            o_ps = psa.tile([128, 4, 128], F32, tag="o_ps", bufs=2,
                            name="o_ps")
            for hh in range(4):
                h = 4 * hf + hh
                nc.tensor.matmul(o_ps[:, hh, :], vst[:, tb, ts(h, 128)],
                                 ets[:, hh, :], start=hh == 0,
                                 stop=hh == 3)
            tok = ts(s * (SA // 128) + tb, 128)
            nc.vector.tensor_mul(out=os16[:, hs, tok], in0=o_ps,
                                 in1=rbs)

        return [lambda t=tb, h=hf: group(t, h)
                for tb in range(SA // 128) for hf in range(2)]

    # interleave attn(s-1) groups between qkv(s) pieces so the PE always
    # has projection matmuls to run while the softmax chains resolve.
    qkv_live = None
    for s in range(NSA):
        qkv_now, pieces = emit_qkv_pieces(s)
        groups = emit_attn_groups(s - 1, qkv_live) if qkv_live else []
        gi = iter(groups)
        for i, pc in enumerate(pieces):
            pc()
            if i % 3 == 2:
                g = next(gi, None)
                if g:
                    g()
        for g in gi:
            g()
        qkv_live = qkv_now
    for g in emit_attn_groups(NSA - 1, qkv_live):
        g()

    pclose("psa")
    pclose("pa")
    pclose("wqp")

    # ================= macro 3+4 (w2l streams at the boundary) ====
    w2p = popen("w2p", bufs=1)
    w2l_sb = w2p.tile([128, NHC, D], FP8, name="w2l_sb")
    for q in range(8):
        nc.sync.dma_start(out=w2l_sb[:, 4 * q:4 * q + 4, :],
                          in_=dram["w2l"][:, 4 * q:4 * q + 4, :])
    c_b1 = consts.tile([128, NHC], F32, name="c_b1")
    nc.sync.dma_start(out=c_b1, in_=dram["b1p"][:, :])
    c_bo = cdma("boS", [128, NCH], F32)
    c_bb2 = cdma("bb2p", [128, NCH], F32)
    c_g1 = cdma("g1p", [128, NCH], F32)
    c_g2 = cdma("g2p", [128, NCH], F32)
    c_be2 = cdma("be2p", [128, NCH], F32)
    for (t, name) in pending_cdma:
        nc.sync.dma_start(out=t, in_=dram[name][:, :])
    pending_cdma.clear()

    pb = popen("pb", bufs=2)
    psb = popen("psb", bufs=1, space="PSUM")

    def half_ps(nm):
        return psb.tile([128, 4, SB], F32, tag="half_ps", bufs=3, name=nm)

    def emit_requant(s):
        tsl = ts(s, SB)
        xst = pb.tile([128, NCH, SB], F16, tag="xst", bufs=2)
        nc.scalar.dma_start(out=xst, in_=dram["xs"][:, :, tsl])
        os8 = pb.tile([128, NCH, SB], FP8, tag="os8", bufs=2)
        nc.scalar.activation(out=os8, in_=os16[:, :, tsl], func=AF.Identity,
                             scale=SO)
        os8l = pb.tile([128, NCH, SB], FP8, tag="os8l", bufs=2)
        nc.vector.scalar_tensor_tensor(out=os8l, in0=os16[:, :, tsl],
                                       scalar=SO, in1=os8,
                                       op0=OP.mult, op1=OP.subtract)
        return {"xst": xst, "os8": os8, "os8l": os8l}

    def emit_oproj(s, st):
        tsl = ts(s, SB)
        xst, os8, os8l = st["xst"], st["os8"], st["os8l"]

        hpre = pb.tile([128, NCH, SB], F16, tag="hpre", bufs=1)
        for half in range(2):
            po = half_ps("po")
            for dq in range(4):
                dc = 4 * half + dq
                first = dq % 2 == 0
                for xa in (os8, os8l):
                    for i in range(NCH // 2):
                        nc.tensor.matmul(
                            po[:, dq, :],
                            wo_sb[:, 2 * i:2 * i + 2, ts(dc, 128)],
                            xa[:, 2 * i:2 * i + 2, :],
                            start=first,
                            stop=(dq % 2 == 1 and xa is os8l
                                  and i == NCH // 2 - 1),
                            perf_mode=DR)
                        first = False
            sl = slice(4 * half, 4 * half + 4)
            if zbo:
                nc.vector.scalar_tensor_tensor(
                    out=hpre[:, sl, :], in0=po, scalar=1.0 / (SO * SW),
                    in1=xst[:, sl, :], op0=OP.mult, op1=OP.add)
            else:
                for dq in range(4):
                    dc = 4 * half + dq
                    yo = pb.tile([128, SB], F16, tag="yg", bufs=2, name="yo")
                    nc.vector.tensor_scalar(out=yo, in0=xst[:, dc, :],
                                            scalar1=1.0,
                                            scalar2=c_bo[:, dc:dc + 1],
                                            op0=OP.mult, op1=OP.add)
                    nc.vector.scalar_tensor_tensor(
                        out=hpre[:, dc, :], in0=po[:, dq, :],
                        scalar=1.0 / (SO * SW), in1=yo,
                        op0=OP.mult, op1=OP.add)
        return {"hpre": hpre}

    def emit_ln1(s, st):
        hpre = st["hpre"]
        sq = pb.tile([128, NCH, SB], F16, tag="sq", bufs=1)
        nc.scalar.activation(out=sq, in_=hpre, func=AF.Square)
        st_ps = psb.tile([1, 2, SB], F32, tag="st_ps", bufs=1, name="st_ps")
        for dc in range(NCH):
            nc.tensor.matmul(st_ps[:, 0, :], c_oc, hpre[:, dc, :],
                             start=dc == 0, stop=False)
            nc.tensor.matmul(st_ps[:, 1, :], c_oc, sq[:, dc, :],
                             start=False, stop=dc == NCH - 1)
        rbm = _ln_stats(nc, pb, st_ps, c_eps, SB, "")
        bc_ps = psb.tile([128, 2, SB], F32, tag="bc_ps", bufs=1, name="bc_ps")
        nc.tensor.matmul(bc_ps[:, :, :], c_or, rbm, start=True, stop=True)
        bcs = pb.tile([128, 2, SB], F16, tag="bcs", bufs=1)
        nc.scalar.copy(out=bcs, in_=bc_ps)
        ys = pb.tile([128, NCH, SB], F16, tag="ys", bufs=2, name="ys")
        for dc in range(NCH):
            nc.vector.tensor_mul(out=ys[:, dc, :], in0=hpre[:, dc, :],
                                 in1=bcs[:, 0, :])
        for dc in range(NCH):
            nc.vector.tensor_sub(out=ys[:, dc, :], in0=ys[:, dc, :],
                                 in1=bcs[:, 1, :])
        y8 = pb.tile([128, NCH, SB], FP8, tag="y8", bufs=1)
        nc.scalar.activation(out=y8, in_=ys, func=AF.Identity, scale=SH)
        y8l = pb.tile([128, NCH, SB], FP8, tag="y8l", bufs=1)
        nc.vector.scalar_tensor_tensor(out=y8l, in0=ys, scalar=SH, in1=y8,
                                       op0=OP.mult, op1=OP.subtract)
        st.update(ys=ys, y8=y8, y8l=y8l)

    def emit_w1(s, st):
        y8, y8l = st["y8"], st["y8l"]
        h8 = pb.tile([128, NHC, SB], FP8, tag="h8", bufs=1)
        h8l = pb.tile([128, NHC, SB], FP8, tag="h8l", bufs=1)
        for g in range(NHC // 4):
            w1ps = half_ps("w1ps")
            for hh in range(4):
                hc = 4 * g + hh
                first = hh % 2 == 0
                for xa in (y8, y8l):
                    for i in range(NCH // 2):
                        nc.tensor.matmul(
                            w1ps[:, hh, :],
                            w1h_sb[:, 2 * i:2 * i + 2, ts(hc, 128)],
                            xa[:, 2 * i:2 * i + 2, :],
                            start=first,
                            stop=(hh % 2 == 1 and xa is y8l
                                  and i == NCH // 2 - 1),
                            perf_mode=DR)
                        first = False
            gsl = slice(4 * g, 4 * g + 4)
            g16 = pb.tile([128, 4, SB], F16, tag="g16", bufs=3)
            if zb1:
                nc.scalar.activation(out=g16, in_=w1ps, func=AF.Gelu,
                                     scale=1.0 / (SH * SW))
            else:
                for hh in range(4):
                    hc = 4 * g + hh
                    nc.scalar.activation(out=g16[:, hh, :],
                                         in_=w1ps[:, hh, :], func=AF.Gelu,
                                         bias=c_b1[:, hc:hc + 1],
                                         scale=1.0 / (SH * SW))
            nc.vector.tensor_scalar(out=h8[:, gsl, :], in0=g16,
                                    scalar1=SH, scalar2=None, op0=OP.mult)
            nc.vector.scalar_tensor_tensor(out=h8l[:, gsl, :], in0=g16,
                                           scalar=SH, in1=h8[:, gsl, :],
                                           op0=OP.mult, op1=OP.subtract)
        st["h8"] = h8
        st["h8l"] = h8l

    def emit_w2_half(s, st, half):
        h8, h8l, ys = st["h8"], st["h8l"], st["ys"]
        if half == 0:
            st["h2"] = pb.tile([128, NCH, SB], F16, tag="h2", bufs=2,
                               name="h2")
        h2 = st["h2"]
        w2ps = half_ps("w2ps")
        for dp in range(2):
            first = True
            for (xa, wa) in ((h8, w2h_sb), (h8l, w2h_sb), (h8, w2l_sb)):
                for dq in (2 * dp, 2 * dp + 1):
                    dc = 4 * half + dq
                    for i in range(NHC // 2):
                        nc.tensor.matmul(
                            w2ps[:, dq, :],
                            wa[:, 2 * i:2 * i + 2, ts(dc, 128)],
                            xa[:, 2 * i:2 * i + 2, :],
                            start=first,
                            stop=(dq == 2 * dp + 1 and wa is w2l_sb
                                  and i == NHC // 2 - 1),
                            perf_mode=DR)
                        first = False
        sl = slice(4 * half, 4 * half + 4)
        if zb2a1:
            nc.vector.scalar_tensor_tensor(
                out=h2[:, sl, :], in0=w2ps, scalar=1.0 / (SH * SW),
                in1=ys[:, sl, :], op0=OP.mult, op1=OP.add)
        else:
            for dq in range(4):
                dc = 4 * half + dq
                yg = pb.tile([128, SB], F16, tag="yg", bufs=2)
                nc.vector.tensor_scalar(out=yg, in0=ys[:, dc, :],
                                        scalar1=c_g1[:, dc:dc + 1],
                                        scalar2=c_bb2[:, dc:dc + 1],
                                        op0=OP.mult, op1=OP.add)
                nc.vector.scalar_tensor_tensor(
                    out=h2[:, dc, :], in0=w2ps[:, dq, :],
                    scalar=1.0 / (SH * SW), in1=yg,
                    op0=OP.mult, op1=OP.add)


    def emit_ln2(s, st):
        h2 = st["h2"]
        # sumsq tolerates fp8 squares (0.2% on var after averaging 1024
        # terms; verified no max-err change in the numpy model), which
        # makes the sumsq reduction a DoubleRow matmul at half the passes
        # and half the cycles/row.
        sq2 = pb.tile([128, NCH, SB], FP8, tag="sq", bufs=1, name="sq2")
        nc.scalar.activation(out=sq2, in_=h2, func=AF.Square)
        tsl = ts(s, SB)
        st2_ps = psb.tile([128, 2, SB], F32, tag="st_ps", bufs=1,
                          name="st2_ps")
        for dc in range(NCH):
            nc.tensor.matmul(st2_ps[0:1, 0, :], c_oc, h2[:, dc, :],
                             start=dc == 0, stop=False)
        for i in range(NCH // 2):
            nc.tensor.matmul(st2_ps[:, 1, :], c_oc8,
                             sq2[:, 2 * i:2 * i + 2, :],
                             start=False, stop=i == NCH // 2 - 1,
                             perf_mode=DR)
        rbm2 = _ln_stats(nc, pb, st2_ps[0:1], c_eps, SB, "")
        bc2_ps = psb.tile([128, 2, SB], F32, tag="bc_ps", bufs=1,
                          name="bc2_ps")
        nc.tensor.matmul(bc2_ps[:, :, :], c_or, rbm2, start=True, stop=True)
        bc2s = pb.tile([128, 2, SB], F16, tag="bcs", bufs=1, name="bc2s")
        nc.scalar.copy(out=bc2s, in_=bc2_ps)
        yout = pb.tile([128, NCH, SB], F16, tag="xst", bufs=2, name="yout")
        for dc in range(NCH):
            nc.vector.tensor_mul(out=yout[:, dc, :], in0=h2[:, dc, :],
                                 in1=bc2s[:, 0, :])
        for dc in range(NCH):
            nc.vector.tensor_sub(out=yout[:, dc, :], in0=yout[:, dc, :],
                                 in1=bc2s[:, 1, :])
            if not za2:
                nc.vector.tensor_scalar(out=yout[:, dc, :],
                                        in0=yout[:, dc, :],
                                        scalar1=c_g2[:, dc:dc + 1],
                                        scalar2=c_be2[:, dc:dc + 1],
                                        op0=OP.mult, op1=OP.add)
        nc.scalar.dma_start(out=dram["outb"][:, :, tsl], in_=yout)

    rq = {0: emit_requant(0)}
    states = {0: emit_oproj(0, rq.pop(0))}
    emit_ln1(0, states[0])
    rq[1] = emit_requant(1)
    for s in range(NSB):
        st = states[s]
        if s + 1 < NSB:
            states[s + 1] = emit_oproj(s + 1, rq.pop(s + 1))
        if s + 2 < NSB:
            rq[s + 2] = emit_requant(s + 2)
        emit_w1(s, st)
        if s + 1 < NSB:
            emit_ln1(s + 1, states[s + 1])
        emit_w2_half(s, st, 0)
        emit_w2_half(s, st, 1)
        if s >= 1:
            emit_ln2(s - 1, states.pop(s - 1))
    emit_ln2(NSB - 1, states.pop(NSB - 1))

    pclose("psb")
    pclose("pb")
    pclose("w2p")
    pclose("w1p")
    pclose("osp")
    pclose("consts")


def _ln_stats(nc, pool, st_ps, eps_t, TW, tag):
    """stat psum [1, 2, TW] (sum, sumsq) -> rstd, bm rows (fp16)."""
    mean = pool.tile([1, TW], F16, tag=f"mean{tag}", bufs=1)
    nc.scalar.activation(out=mean, in_=st_ps[:, 0, :], func=AF.Identity,
                         scale=1.0 / D)
    msq = pool.tile([1, TW], F16, tag=f"msq{tag}", bufs=1)
    nc.vector.tensor_mul(out=msq, in0=mean, in1=mean)
    var = pool.tile([1, TW], F16, tag=f"var{tag}", bufs=1)
    nc.vector.scalar_tensor_tensor(out=var, in0=st_ps[:, 1, :],
                                   scalar=1.0 / D, in1=msq,
                                   op0=OP.mult, op1=OP.subtract)
    sd = pool.tile([1, TW], F16, tag=f"msq{tag}", bufs=1, name="sd")
    nc.scalar.activation(out=sd, in_=var, func=AF.Sqrt, bias=eps_t, scale=1.0)
    rbm = pool.tile([1, 2, TW], F16, tag=f"rbm{tag}", bufs=1, name="rbm")
    with nc.allow_low_precision(reason="LN broadcast rows in fp16"):
        nc.vector.reciprocal(out=rbm[:, 0, :], in_=sd)
    nc.vector.tensor_mul(out=rbm[:, 1, :], in0=mean, in1=rbm[:, 0, :])
    return rbm


# ======================= host side =======================

def _prep_shared(w_qkv, b_qkv, w_out, b_out, w1, b1, w2, b2,
                 g1, beta1, g2, beta2):
    wq, wk, wv = w_qkv[0:D], w_qkv[D:2 * D], w_qkv[2 * D:3 * D]
    bq, bk, bv = b_qkv[0:D], b_qkv[D:2 * D], b_qkv[2 * D:3 * D]

    def pmaj(v, n):
        return np.ascontiguousarray(
            np.asarray(v, np.float32).reshape(n, 128).T)

    def chunk8(wT, nk):
        # [K, M] -> [128, nk, M] (K = nk*128, chunk-major along K)
        return np.ascontiguousarray(
            wT.reshape(nk, 128, wT.shape[1]).transpose(1, 0, 2))

    wqT = np.ascontiguousarray(np.asarray(wq, np.float32).T)
    wkT = np.ascontiguousarray(np.asarray(wk, np.float32).T)
    wvT = np.ascontiguousarray(np.asarray(wv, np.float32).T)
    woT = np.ascontiguousarray(np.asarray(w_out, np.float32).T)
    w1g = np.asarray(w1, np.float32) * np.asarray(g1, np.float32)[None, :]
    w1T = np.ascontiguousarray(w1g.T)          # [D, FF]
    w2T = np.ascontiguousarray(np.asarray(w2, np.float32).T)  # [FF, D]

    w2s = w2T * SW
    w2hT = w2s.astype(E4)
    w2lT = (w2s - w2hT.astype(np.float32)).astype(E4)

    b1f = np.asarray(b1, np.float32) + w1g @ np.asarray(beta1, np.float32)
    bb2 = np.asarray(b2, np.float32) + np.asarray(beta1, np.float32)

    mu = np.zeros((9, 128), np.float32)
    mw = np.zeros((9, 128), np.float32)
    for w in range(8):
        mu[w, w * 16:(w + 1) * 16] = MASKC
        mw[w, w * 16:(w + 1) * 16] = 1.0
    mu[8, :] = -MASKC
    mw[8, :] = 1.0
    mwr = np.tile(mw, (1, 4))

    shared = {
        "wq8": chunk8(wqT * SW, NCH).astype(E4),
        "wk8": chunk8(wkT * SW, NCH).astype(E4),
        "wv8": chunk8(wvT * SW, NCH).astype(E4),
        "wo8": chunk8(woT * SW, NCH).astype(E4),
        "w1h": chunk8(w1T * SW, NCH).astype(E4),
        "w2h": chunk8(w2hT.astype(np.float32), NHC).astype(E4),
        "w2l": chunk8(w2lT.astype(np.float32), NHC).astype(E4),
        "bqp": pmaj(bq, NCH), "bkp": pmaj(bk, NCH),
        "bvS": (np.asarray(bv, np.float32) * SX * SW).reshape(1, D).astype(NF),
        "b1p": pmaj(b1f, NHC),
        "boS": pmaj(np.asarray(b_out, np.float32), NCH),
        "bb2p": pmaj(bb2, NCH),
        "g1p": pmaj(g1, NCH),
        "g2p": pmaj(g2, NCH), "be2p": pmaj(beta2, NCH),
        "masku": mu.astype(NF), "maskw": mwr.astype(NF),
        "onesrow": np.ones((1, 128), np.float32).astype(NF),
        "onescol": np.ones((128, 1), np.float32).astype(NF),
    }
    flags = (
        bool(np.all(np.asarray(bv) == 0)),                       # zbv
        bool(np.all(b1f == 0)),                                  # zb1
        bool(np.all(np.asarray(b_out) == 0)),                    # zbo
        bool(np.all(bb2 == 0)
             and np.all(np.asarray(g1, np.float32) == 1.0)),     # zb2a1
        bool(np.all(np.asarray(beta2) == 0)
             and np.all(np.asarray(g2, np.float32) == 1.0)),     # za2
    )
    return shared, flags


def make_in_maps(inputs):
    ff = np.asarray(inputs["frame_features"], np.float32)
    fi = np.asarray(inputs["frame_indices"])
    shared, flags = _prep_shared(
        np.asarray(inputs["w_qkv"]), np.asarray(inputs["b_qkv"]),
        np.asarray(inputs["w_out"]), np.asarray(inputs["b_out"]),
        np.asarray(inputs["w1"]), np.asarray(inputs["b1"]),
        np.asarray(inputs["w2"]), np.asarray(inputs["b2"]),
        np.asarray(inputs["g1"]), np.asarray(inputs["beta1"]),
        np.asarray(inputs["g2"]), np.asarray(inputs["beta2"]))

    div = np.exp(np.float32(-np.log(10000.0))
                 * np.arange(0, D, 2, dtype=np.float32) / np.float32(D))
    in_maps = []
    for b in range(B):
        pos = np.asarray(fi[b], np.float32)[:, None]
        ang = pos * div[None, :]
        pe = np.empty((T, D), np.float32)
        pe[:, 0::2] = np.sin(ang)
        pe[:, 1::2] = np.cos(ang)
        xpe = ff[b] + pe                       # [T, D]
        xpeT = np.ascontiguousarray(xpe.T)     # [D, T]
        x8 = np.ascontiguousarray(
            (xpeT * SX).reshape(NCH, 128, T).transpose(1, 0, 2)).astype(E4)
        xsc = np.ascontiguousarray(
            xpeT.reshape(NCH, 128, T).transpose(1, 0, 2)).astype(NF)
        m = dict(shared)
        m["x8"] = x8
        m["xs"] = xsc
        in_maps.append(m)
    return in_maps, flags


def get_nc(flags=(True, True, True, True, True)):
    if flags not in _NC_CACHE:
        _NC_CACHE[flags] = build_nc(flags)
    return _NC_CACHE[flags]


def kernel(**inputs) -> np.ndarray:
    in_maps, flags = make_in_maps(inputs)
    nc = get_nc(flags)
    res = run_bass_kernel_spmd(nc, in_maps, core_ids=list(range(B)))
    outs = []
    for r in res.results:
        ob = np.asarray(r["outb"])             # [128, NCH, T] fp16
        oT = ob.transpose(1, 0, 2).reshape(D, T)
        outs.append(oT.T.astype(np.float32))
    return np.ascontiguousarray(np.stack(outs))


# revision 41
# speedup vs baseline: 1.2788x; 1.0019x over previous
"""Trainium2 Bass kernel for a local-window-attention transformer block (v4).

Sharding: data-parallel over batch (one batch element per NeuronCore).

v4 vs v2 (604us -> 473us TimelineSim per core):
- fp16 replaces bf16 on the whole residual/attention path (same engine
  cost, ~8x finer mantissa), buying accuracy budget to cut PE work:
  * w1 runs as fp8 "x2b": y8/y8l hi-lo activations against a single fp8
    w1h (w1l is gone entirely, freeing 32KB/partition of SBUF),
  * w2 runs as fp8 "x3": h8/h8l hi-lo against w2h + h8 against w2l,
  * the out-projection moving operand is the fp16 attention output
    requantized per-slab to fp8 hi/lo (os8/os8l) against fp8 wo,
  all DoubleRow at 0.5 cycles/row instead of 1.0 bf16.
- scale-free residual chain (no CS1 prescaling of xs/eps); LN2's sumsq
  reduction sums fp8 squares via DoubleRow (0.2% on var after averaging
  1024 terms — verified no max-err change).
- attention runs per (128-token block, 4-head group) with every PSUM stage
  (scores, zsum/broadcast, PV) on its own 1-bank double-buffered tag, so
  the exp -> zsum -> reciprocal -> broadcast -> PV chains of consecutive
  groups overlap.
- macro34 is software-pipelined as: oproj(s+1) | requant(s+2) | w1(s) |
  ln1(s+1) | w2(s) | ln2(s-1), which gives each LayerNorm's long
  Act/DVE chain a full w2 phase to hide behind; sq/sq2 are computed at
  their LN sites so one buffer suffices.
- w2h streams during the attention phase (space freed by dropping w1l);
  only w2l loads at the phase boundary, and w2 accumulation orders the
  w2l-reads last to cover its DMA.
- QKV/attention PSUM drains alternate between Act and DVE per head so
  neither engine is the macro12 bottleneck, and attention-group emission is
  interleaved with the next slab's QKV pieces so the PE always has
  projection matmuls to run while softmax chains resolve.
"""
import numpy as np
import ml_dtypes

import concourse.bass as bass
import concourse.bacc as bacc
import concourse.mybir as mybir
import concourse.tile as tile
from concourse.bass import ts
from concourse.bass_utils import run_bass_kernel_spmd

F32 = mybir.dt.float32
F16 = mybir.dt.float16
FP8 = mybir.dt.float8e4
AF = mybir.ActivationFunctionType
OP = mybir.AluOpType
DR = mybir.MatmulPerfMode.DoubleRow
NF = np.float16
E4 = ml_dtypes.float8_e4m3

B, T, D, W, H = 8, 2048, 1024, 16, 8
HD = D // H            # 128 = head dim = one partition chunk
FF = 4 * D             # 4096
NCH = D // 128         # 8 feature chunks
NHC = FF // 128        # 32 hidden chunks
EPS = 1e-5
ISQ = float(1.0 / np.sqrt(128.0))
MASKC = 340.0

SX = 16.0              # x+pe fp8 scale
SW = 64.0              # weight fp8 scale
SH = 32.0              # LN1-out / gelu-out fp8 scale
SO = 32.0              # attention-out fp8 requant scale

SA = 512               # macro12 token slab
NSA = T // SA
SB = 256               # macro34 token slab
NSB = T // SB

_NC_CACHE = {}


def build_nc(flags):
    (zbv, zb1, zbo, zb2a1, za2) = flags
    nc = bacc.Bacc(None, target_bir_lowering=False)

    dram = {}
    # ---- per-core inputs ----
    dram["x8"] = nc.declare_dram_parameter("x8", [128, NCH, T], FP8,
                                           isOutput=False)
    dram["xs"] = nc.declare_dram_parameter("xs", [128, NCH, T], F16,
                                           isOutput=False)
    # ---- shared weights ----
    for nm, sh, dt in (
            ("wq8", [128, NCH, D], FP8), ("wk8", [128, NCH, D], FP8),
            ("wv8", [128, NCH, D], FP8), ("wo8", [128, NCH, D], FP8),
            ("w1h", [128, NCH, FF], FP8),
            ("w2h", [128, NHC, D], FP8), ("w2l", [128, NHC, D], FP8),
            ("bqp", [128, NCH], F32), ("bkp", [128, NCH], F32),
            ("bvS", [1, D], F16), ("b1p", [128, NHC], F32),
            ("boS", [128, NCH], F32), ("bb2p", [128, NCH], F32),
            ("g1p", [128, NCH], F32), ("g2p", [128, NCH], F32),
            ("be2p", [128, NCH], F32),
            ("masku", [9, 128], F16), ("maskw", [9, 512], F16),
            ("onesrow", [1, 128], F16), ("onescol", [128, 1], F16)):
        dram[nm] = nc.declare_dram_parameter(nm, sh, dt, isOutput=False)

    dram["outb"] = nc.declare_dram_parameter("outb", [128, NCH, T], F16,
                                             isOutput=True)

    with tile.TileContext(nc) as tc:
        _emit(nc, tc, flags, dram)
    nc.compile()
    return nc


def _emit(nc, tc, flags, dram):
    (zbv, zb1, zbo, zb2a1, za2) = flags
    open_pools = {}

    def popen(name, **kw):
        cm = tc.tile_pool(name=name, **kw)
        pool = cm.__enter__()
        open_pools[name] = cm
        return pool

    def pclose(name):
        open_pools.pop(name).__exit__(None, None, None)

    # ---- constants (live whole kernel) ----
    consts = popen("consts", bufs=1)

    pending_cdma = []

    def cdma(name, shape, dt):
        t = consts.tile(shape, dt, tag=f"c_{name}", name=f"c_{name}")
        pending_cdma.append((t, name))
        return t

    c_bq = cdma("bqp", [128, NCH], F32)
    c_bk = cdma("bkp", [128, NCH], F32)
    c_bv = None if zbv else cdma("bvS", [1, D], F16)
    c_mu = cdma("masku", [9, 128], F16)
    c_mw = cdma("maskw", [9, 512], F16)
    c_or = cdma("onesrow", [1, 128], F16)
    c_oc = cdma("onescol", [128, 1], F16)
    c_eps = consts.tile([1, 1], F32, name="c_eps")
    nc.vector.memset(c_eps, EPS)
    # dual-fp8 LdWeights requires a full-width stationary (walrus
    # s3_lw_dual_fp8_restrictions rejects 1-column); use a 128-wide ones
    # block and read row 0 of the (all-equal) output.
    c_oc8 = consts.tile([128, 2, 128], FP8, name="c_oc8")
    nc.vector.memset(c_oc8, 1.0)

    # attention output (fp16, full residency)
    osp = popen("osp", bufs=1)
    os16 = osp.tile([128, NCH, T], F16, name="os16")

    # macro34 weights minus w2l: space reserved up-front so their DMAs can
    # stream during macro12 instead of waiting for its pools to die.
    w1p = popen("w1p", bufs=1)
    wo_sb = w1p.tile([128, NCH, D], FP8, name="wo_sb")
    w1h_sb = w1p.tile([128, NCH, FF], FP8, name="w1h_sb")
    w2h_sb = w1p.tile([128, NHC, D], FP8, name="w2h_sb")

    wqp = popen("wqp", bufs=1)
    wq_sb = wqp.tile([128, NCH, D], FP8, name="wq_sb")
    nc.sync.dma_start(out=wq_sb, in_=dram["wq8"][:, :, :])
    early = [p for p in pending_cdma if p[1] in ("bqp", "bkp")]
    for (t, name) in early:
        nc.sync.dma_start(out=t, in_=dram[name][:, :])
        pending_cdma.remove((t, name))
    wk_sb = wqp.tile([128, NCH, D], FP8, name="wk_sb")
    nc.sync.dma_start(out=wk_sb, in_=dram["wk8"][:, :, :])
    wv_sb = wqp.tile([128, NCH, D], FP8, name="wv_sb")
    nc.sync.dma_start(out=wv_sb, in_=dram["wv8"][:, :, :])
    for (t, name) in pending_cdma:
        nc.sync.dma_start(out=t, in_=dram[name][:, :])
    pending_cdma.clear()
    nc.sync.dma_start(out=wo_sb, in_=dram["wo8"][:, :, :])
    for q in range(4):
        nc.sync.dma_start(out=w1h_sb[:, 2 * q:2 * q + 2, :],
                          in_=dram["w1h"][:, 2 * q:2 * q + 2, :])
    # stream w2h during macro12 (fills DMA idle; ready before macro34)
    for q in range(8):
        nc.sync.dma_start(out=w2h_sb[:, 4 * q:4 * q + 4, :],
                          in_=dram["w2h"][:, 4 * q:4 * q + 4, :])

    # ================= macro 1+2: QKV + attention =================
    pa = popen("pa", bufs=2)
    psa = popen("psa", bufs=1, space="PSUM")

    def emit_qkv_pieces(s):
        tsl = ts(s, SA)
        x8t = pa.tile([128, NCH, SA], FP8, tag="x8t")
        nc.scalar.dma_start(out=x8t, in_=dram["x8"][:, :, tsl])

        qst = pa.tile([128, NCH, SA], F16, tag="qst")
        kst = pa.tile([128, NCH, SA], F16, tag="kst")
        vst = pa.tile([128, SA // 128, D], F16, tag="vst")
        pieces = []

        def qk_head(w_sb, cbias, dst, h):
            ps = psa.tile([128, SA], F32, tag="sps", bufs=2, name="sps")
            for i in range(NCH // 2):
                nc.tensor.matmul(ps, w_sb[:, 2 * i:2 * i + 2, ts(h, 128)],
                                 x8t[:, 2 * i:2 * i + 2, :],
                                 start=i == 0, stop=i == NCH // 2 - 1,
                                 perf_mode=DR)
            if h % 2 == 0:
                nc.scalar.activation(out=dst[:, h, :], in_=ps,
                                     func=AF.Identity,
                                     bias=cbias[:, h:h + 1],
                                     scale=1.0 / (SX * SW))
            else:
                nc.vector.tensor_scalar(out=dst[:, h, :], in0=ps,
                                        scalar1=1.0 / (SX * SW),
                                        scalar2=cbias[:, h:h + 1],
                                        op0=OP.mult, op1=OP.add)

        def v_piece(tb, nb):
            ps = psa.tile([128, 512], F32, tag="sps", bufs=2, name="spsv")
            nkp = NCH // 2
            for i in range(nkp):
                last = (i == nkp - 1) and zbv
                nc.tensor.matmul(ps, x8t[:, 2 * i:2 * i + 2, ts(tb, 128)],
                                 wv_sb[:, 2 * i:2 * i + 2, ts(nb, 512)],
                                 start=i == 0, stop=last, perf_mode=DR)
            if not zbv:
                nc.tensor.matmul(ps, c_or, c_bv[:, ts(nb, 512)],
                                 start=False, stop=True)
            if nb == 0:
                nc.scalar.activation(out=vst[:, tb, ts(nb, 512)],
                                     in_=ps, func=AF.Identity,
                                     scale=1.0 / (SX * SW))
            else:
                nc.vector.tensor_scalar(out=vst[:, tb, ts(nb, 512)],
                                        in0=ps, scalar1=1.0 / (SX * SW),
                                        scalar2=None, op0=OP.mult)

        for (w_sb, cbias, dst) in ((wq_sb, c_bq, qst), (wk_sb, c_bk, kst)):
            for h in range(H):
                pieces.append(lambda w=w_sb, c=cbias, d=dst, hh=h:
                              qk_head(w, c, d, hh))
        for tb in range(SA // 128):
            for nb in range(2):
                pieces.append(lambda t=tb, n=nb: v_piece(t, n))
        return (qst, kst, vst), pieces

    def emit_attn_groups(s, qkv):
        qst, kst, vst = qkv

        def group(tb, hf):
            hs = slice(4 * hf, 4 * hf + 4)
            et_ps = psa.tile([128, 4, 128], F32, tag="et_ps", bufs=2,
                             name="et_ps")
            ets = pa.tile([128, 4, 128], F16, tag="ets", bufs=4)
            nc.tensor.matmul(et_ps, c_mu, c_mw, start=True, stop=False,
                             skip_group_check=True)
            for hh in range(4):
                h = 4 * hf + hh
                nc.tensor.matmul(
                    et_ps[:, hh, :],
                    kst[:, h, ts(tb, 128)], qst[:, h, ts(tb, 128)],
                    start=False, stop=hh == 3, skip_group_check=True)
            nc.scalar.activation(out=ets, in_=et_ps, func=AF.Exp,
                                 scale=ISQ)
            rb_ps = psa.tile([128, 4, 128], F32, tag="rb_ps", bufs=2,
                             name="rb_ps")
            # z row lives on partition 0 of rb_ps until the broadcast
            # matmul overwrites it (tile deps serialize the reciprocal
            # before it).
            nc.tensor.matmul(rb_ps[0:1, :, :], c_oc, ets,
                             start=True, stop=True)
            rz = pa.tile([1, 4, 128], F16, tag="rz", bufs=4)
            with nc.allow_low_precision(reason="softmax renorm in fp16"):
                nc.vector.reciprocal(out=rz, in_=rb_ps[0:1, :, :])
            nc.tensor.matmul(rb_ps, c_or, rz, start=True, stop=True)
            rbs = pa.tile([128, 4, 128], F16, tag="rbs", bufs=4)
            nc.scalar.copy(out=rbs, in_=rb_ps)
            o_ps = psa.tile([128, 4, 128], F32, tag="o_ps", bufs=2,
                            name="o_ps")
            for hh in range(4):
                h = 4 * hf + hh
                nc.tensor.matmul(o_ps[:, hh, :], vst[:, tb, ts(h, 128)],
                                 ets[:, hh, :], start=hh == 0,
                                 stop=hh == 3)
            tok = ts(s * (SA // 128) + tb, 128)
            nc.vector.tensor_mul(out=os16[:, hs, tok], in0=o_ps,
                                 in1=rbs)

        return [lambda t=tb, h=hf: group(t, h)
                for tb in range(SA // 128) for hf in range(2)]

    # interleave attn(s-1) groups between qkv(s) pieces so the PE always
    # has projection matmuls to run while the softmax chains resolve.
    qkv_live = None
    for s in range(NSA):
        qkv_now, pieces = emit_qkv_pieces(s)
        groups = emit_attn_groups(s - 1, qkv_live) if qkv_live else []
        gi = iter(groups)
        for i, pc in enumerate(pieces):
            pc()
            if i % 3 == 2:
                g = next(gi, None)
                if g:
                    g()
        for g in gi:
            g()
        qkv_live = qkv_now
    for g in emit_attn_groups(NSA - 1, qkv_live):
        g()

    pclose("psa")
    pclose("pa")
    pclose("wqp")

    # ================= macro 3+4 (w2l streams at the boundary) ====
    w2p = popen("w2p", bufs=1)
    w2l_sb = w2p.tile([128, NHC, D], FP8, name="w2l_sb")
    pb = popen("pb", bufs=2)
    psb = popen("psb", bufs=1, space="PSUM")

    # xs slabs and outputs ride the SP DGE queue (idle through macro34);
    # the first two xs slabs are queued ahead of w2l's 4MB so oproj(0/1)
    # aren't starved, and w2l still lands before the first w2l-read.
    xst01 = []
    for s0 in range(2):
        xt = pb.tile([128, NCH, SB], F16, tag="xst", bufs=2,
                     name=f"xst{s0}")
        nc.sync.dma_start(out=xt, in_=dram["xs"][:, :, ts(s0, SB)])
        xst01.append(xt)
    for q in range(8):
        nc.sync.dma_start(out=w2l_sb[:, 4 * q:4 * q + 4, :],
                          in_=dram["w2l"][:, 4 * q:4 * q + 4, :])
    c_b1 = consts.tile([128, NHC], F32, name="c_b1")
    nc.sync.dma_start(out=c_b1, in_=dram["b1p"][:, :])
    c_bo = cdma("boS", [128, NCH], F32)
    c_bb2 = cdma("bb2p", [128, NCH], F32)
    c_g1 = cdma("g1p", [128, NCH], F32)
    c_g2 = cdma("g2p", [128, NCH], F32)
    c_be2 = cdma("be2p", [128, NCH], F32)
    for (t, name) in pending_cdma:
        nc.sync.dma_start(out=t, in_=dram[name][:, :])
    pending_cdma.clear()

    def half_ps(nm):
        return psb.tile([128, 4, SB], F32, tag="half_ps", bufs=3, name=nm)

    def load_xst(s):
        xst = pb.tile([128, NCH, SB], F16, tag="xst", bufs=2)
        nc.sync.dma_start(out=xst, in_=dram["xs"][:, :, ts(s, SB)])
        return xst

    def emit_requant(s, xst=None):
        tsl = ts(s, SB)
        xst = xst if xst is not None else load_xst(s)
        os8 = pb.tile([128, NCH, SB], FP8, tag="os8", bufs=2)
        nc.scalar.activation(out=os8, in_=os16[:, :, tsl], func=AF.Identity,
                             scale=SO)
        os8l = pb.tile([128, NCH, SB], FP8, tag="os8l", bufs=2)
        nc.vector.scalar_tensor_tensor(out=os8l, in0=os16[:, :, tsl],
                                       scalar=SO, in1=os8,
                                       op0=OP.mult, op1=OP.subtract)
        return {"xst": xst, "os8": os8, "os8l": os8l}

    def emit_oproj(s, st):
        tsl = ts(s, SB)
        xst, os8, os8l = st["xst"], st["os8"], st["os8l"]

        hpre = pb.tile([128, NCH, SB], F16, tag="hpre", bufs=1)
        for half in range(2):
            po = half_ps("po")
            for dq in range(4):
                dc = 4 * half + dq
                first = dq % 2 == 0
                for xa in (os8, os8l):
                    for i in range(NCH // 2):
                        nc.tensor.matmul(
                            po[:, dq, :],
                            wo_sb[:, 2 * i:2 * i + 2, ts(dc, 128)],
                            xa[:, 2 * i:2 * i + 2, :],
                            start=first,
                            stop=(dq % 2 == 1 and xa is os8l
                                  and i == NCH // 2 - 1),
                            perf_mode=DR)
                        first = False
            sl = slice(4 * half, 4 * half + 4)
            if zbo:
                nc.vector.scalar_tensor_tensor(
                    out=hpre[:, sl, :], in0=po, scalar=1.0 / (SO * SW),
                    in1=xst[:, sl, :], op0=OP.mult, op1=OP.add)
            else:
                for dq in range(4):
                    dc = 4 * half + dq
                    yo = pb.tile([128, SB], F16, tag="yg", bufs=2, name="yo")
                    nc.vector.tensor_scalar(out=yo, in0=xst[:, dc, :],
                                            scalar1=1.0,
                                            scalar2=c_bo[:, dc:dc + 1],
                                            op0=OP.mult, op1=OP.add)
                    nc.vector.scalar_tensor_tensor(
                        out=hpre[:, dc, :], in0=po[:, dq, :],
                        scalar=1.0 / (SO * SW), in1=yo,
                        op0=OP.mult, op1=OP.add)
        return {"hpre": hpre}

    def emit_ln1(s, st):
        hpre = st["hpre"]
        sq = pb.tile([128, NCH, SB], F16, tag="sq", bufs=1)
        nc.scalar.activation(out=sq, in_=hpre, func=AF.Square)
        st_ps = psb.tile([1, 2, SB], F32, tag="st_ps", bufs=1, name="st_ps")
        for dc in range(NCH):
            nc.tensor.matmul(st_ps[:, 0, :], c_oc, hpre[:, dc, :],
                             start=dc == 0, stop=False)
            nc.tensor.matmul(st_ps[:, 1, :], c_oc, sq[:, dc, :],
                             start=False, stop=dc == NCH - 1)
        rbm = _ln_stats(nc, pb, st_ps, c_eps, SB, "")
        bc_ps = psb.tile([128, 2, SB], F32, tag="bc_ps", bufs=1, name="bc_ps")
        nc.tensor.matmul(bc_ps[:, :, :], c_or, rbm, start=True, stop=True)
        bcs = pb.tile([128, 2, SB], F16, tag="bcs", bufs=1)
        nc.scalar.copy(out=bcs, in_=bc_ps)
        ys = pb.tile([128, NCH, SB], F16, tag="ys", bufs=2, name="ys")
        for dc in range(NCH):
            nc.vector.tensor_mul(out=ys[:, dc, :], in0=hpre[:, dc, :],
                                 in1=bcs[:, 0, :])
        for dc in range(NCH):
            nc.vector.tensor_sub(out=ys[:, dc, :], in0=ys[:, dc, :],
                                 in1=bcs[:, 1, :])
        y8 = pb.tile([128, NCH, SB], FP8, tag="y8", bufs=1)
        nc.scalar.activation(out=y8, in_=ys, func=AF.Identity, scale=SH)
        y8l = pb.tile([128, NCH, SB], FP8, tag="y8l", bufs=1)
        nc.vector.scalar_tensor_tensor(out=y8l, in0=ys, scalar=SH, in1=y8,
                                       op0=OP.mult, op1=OP.subtract)
        st.update(ys=ys, y8=y8, y8l=y8l)

    def emit_w1(s, st):
        y8, y8l = st["y8"], st["y8l"]
        h8 = pb.tile([128, NHC, SB], FP8, tag="h8", bufs=1)
        h8l = pb.tile([128, NHC, SB], FP8, tag="h8l", bufs=1)
        for g in range(NHC // 4):
            w1ps = half_ps("w1ps")
            for hh in range(4):
                hc = 4 * g + hh
                first = hh % 2 == 0
                for xa in (y8, y8l):
                    for i in range(NCH // 2):
                        nc.tensor.matmul(
                            w1ps[:, hh, :],
                            w1h_sb[:, 2 * i:2 * i + 2, ts(hc, 128)],
                            xa[:, 2 * i:2 * i + 2, :],
                            start=first,
                            stop=(hh % 2 == 1 and xa is y8l
                                  and i == NCH // 2 - 1),
                            perf_mode=DR)
                        first = False
            gsl = slice(4 * g, 4 * g + 4)
            g16 = pb.tile([128, 4, SB], F16, tag="g16", bufs=3)
            if zb1:
                nc.scalar.activation(out=g16, in_=w1ps, func=AF.Gelu,
                                     scale=1.0 / (SH * SW))
            else:
                for hh in range(4):
                    hc = 4 * g + hh
                    nc.scalar.activation(out=g16[:, hh, :],
                                         in_=w1ps[:, hh, :], func=AF.Gelu,
                                         bias=c_b1[:, hc:hc + 1],
                                         scale=1.0 / (SH * SW))
            nc.vector.tensor_scalar(out=h8[:, gsl, :], in0=g16,
                                    scalar1=SH, scalar2=None, op0=OP.mult)
            nc.vector.scalar_tensor_tensor(out=h8l[:, gsl, :], in0=g16,
                                           scalar=SH, in1=h8[:, gsl, :],
                                           op0=OP.mult, op1=OP.subtract)
        st["h8"] = h8
        st["h8l"] = h8l

    def emit_w2_half(s, st, half):
        h8, h8l, ys = st["h8"], st["h8l"], st["ys"]
        if half == 0:
            st["h2"] = pb.tile([128, NCH, SB], F16, tag="h2", bufs=2,
                               name="h2")
        h2 = st["h2"]
        w2ps = half_ps("w2ps")
        for dp in range(2):
            first = True
            for (xa, wa) in ((h8, w2h_sb), (h8l, w2h_sb), (h8, w2l_sb)):
                for dq in (2 * dp, 2 * dp + 1):
                    dc = 4 * half + dq
                    for i in range(NHC // 2):
                        nc.tensor.matmul(
                            w2ps[:, dq, :],
                            wa[:, 2 * i:2 * i + 2, ts(dc, 128)],
                            xa[:, 2 * i:2 * i + 2, :],
                            start=first,
                            stop=(dq == 2 * dp + 1 and wa is w2l_sb
                                  and i == NHC // 2 - 1),
                            perf_mode=DR)
                        first = False
        sl = slice(4 * half, 4 * half + 4)
        if zb2a1:
            nc.vector.scalar_tensor_tensor(
                out=h2[:, sl, :], in0=w2ps, scalar=1.0 / (SH * SW),
                in1=ys[:, sl, :], op0=OP.mult, op1=OP.add)
        else:
            for dq in range(4):
                dc = 4 * half + dq
                yg = pb.tile([128, SB], F16, tag="yg", bufs=2)
                nc.vector.tensor_scalar(out=yg, in0=ys[:, dc, :],
                                        scalar1=c_g1[:, dc:dc + 1],
                                        scalar2=c_bb2[:, dc:dc + 1],
                                        op0=OP.mult, op1=OP.add)
                nc.vector.scalar_tensor_tensor(
                    out=h2[:, dc, :], in0=w2ps[:, dq, :],
                    scalar=1.0 / (SH * SW), in1=yg,
                    op0=OP.mult, op1=OP.add)


    def emit_ln2(s, st):
        h2 = st["h2"]
        # sumsq tolerates fp8 squares (0.2% on var after averaging 1024
        # terms; verified no max-err change in the numpy model), which
        # makes the sumsq reduction a DoubleRow matmul at half the passes
        # and half the cycles/row.
        sq2 = pb.tile([128, NCH, SB], FP8, tag="sq", bufs=1, name="sq2")
        nc.scalar.activation(out=sq2, in_=h2, func=AF.Square)
        tsl = ts(s, SB)
        st2_ps = psb.tile([128, 2, SB], F32, tag="st_ps", bufs=1,
                          name="st2_ps")
        for dc in range(NCH):
            nc.tensor.matmul(st2_ps[0:1, 0, :], c_oc, h2[:, dc, :],
                             start=dc == 0, stop=False)
        for i in range(NCH // 2):
            nc.tensor.matmul(st2_ps[:, 1, :], c_oc8,
                             sq2[:, 2 * i:2 * i + 2, :],
                             start=False, stop=i == NCH // 2 - 1,
                             perf_mode=DR)
        rbm2 = _ln_stats(nc, pb, st2_ps[0:1], c_eps, SB, "")
        bc2_ps = psb.tile([128, 2, SB], F32, tag="bc_ps", bufs=1,
                          name="bc2_ps")
        nc.tensor.matmul(bc2_ps[:, :, :], c_or, rbm2, start=True, stop=True)
        bc2s = pb.tile([128, 2, SB], F16, tag="bcs", bufs=1, name="bc2s")
        nc.scalar.copy(out=bc2s, in_=bc2_ps)
        yout = pb.tile([128, NCH, SB], F16, tag="xst", bufs=2, name="yout")
        for dc in range(NCH):
            nc.vector.tensor_mul(out=yout[:, dc, :], in0=h2[:, dc, :],
                                 in1=bc2s[:, 0, :])
        for dc in range(NCH):
            nc.vector.tensor_sub(out=yout[:, dc, :], in0=yout[:, dc, :],
                                 in1=bc2s[:, 1, :])
            if not za2:
                nc.vector.tensor_scalar(out=yout[:, dc, :],
                                        in0=yout[:, dc, :],
                                        scalar1=c_g2[:, dc:dc + 1],
                                        scalar2=c_be2[:, dc:dc + 1],
                                        op0=OP.mult, op1=OP.add)
        nc.sync.dma_start(out=dram["outb"][:, :, tsl], in_=yout)

    rq = {0: emit_requant(0, xst01[0])}
    states = {0: emit_oproj(0, rq.pop(0))}
    emit_ln1(0, states[0])
    rq[1] = emit_requant(1, xst01[1])
    for s in range(NSB):
        st = states[s]
        if s + 1 < NSB:
            states[s + 1] = emit_oproj(s + 1, rq.pop(s + 1))
        if s + 2 < NSB:
            rq[s + 2] = emit_requant(s + 2)
        emit_w1(s, st)
        if s + 1 < NSB:
            emit_ln1(s + 1, states[s + 1])
        emit_w2_half(s, st, 0)
        emit_w2_half(s, st, 1)
        if s >= 1:
            emit_ln2(s - 1, states.pop(s - 1))
    emit_ln2(NSB - 1, states.pop(NSB - 1))

    pclose("psb")
    pclose("pb")
    pclose("w2p")
    pclose("w1p")
    pclose("osp")
    pclose("consts")


def _ln_stats(nc, pool, st_ps, eps_t, TW, tag):
    """stat psum [1, 2, TW] (sum, sumsq) -> rstd, bm rows (fp16)."""
    mean = pool.tile([1, TW], F16, tag=f"mean{tag}", bufs=1)
    nc.scalar.activation(out=mean, in_=st_ps[:, 0, :], func=AF.Identity,
                         scale=1.0 / D)
    msq = pool.tile([1, TW], F16, tag=f"msq{tag}", bufs=1)
    nc.vector.tensor_mul(out=msq, in0=mean, in1=mean)
    var = pool.tile([1, TW], F16, tag=f"var{tag}", bufs=1)
    nc.vector.scalar_tensor_tensor(out=var, in0=st_ps[:, 1, :],
                                   scalar=1.0 / D, in1=msq,
                                   op0=OP.mult, op1=OP.subtract)
    sd = pool.tile([1, TW], F16, tag=f"msq{tag}", bufs=1, name="sd")
    nc.scalar.activation(out=sd, in_=var, func=AF.Sqrt, bias=eps_t, scale=1.0)
    rbm = pool.tile([1, 2, TW], F16, tag=f"rbm{tag}", bufs=1, name="rbm")
    with nc.allow_low_precision(reason="LN broadcast rows in fp16"):
        nc.vector.reciprocal(out=rbm[:, 0, :], in_=sd)
    nc.vector.tensor_mul(out=rbm[:, 1, :], in0=mean, in1=rbm[:, 0, :])
    return rbm


# ======================= host side =======================

def _prep_shared(w_qkv, b_qkv, w_out, b_out, w1, b1, w2, b2,
                 g1, beta1, g2, beta2):
    wq, wk, wv = w_qkv[0:D], w_qkv[D:2 * D], w_qkv[2 * D:3 * D]
    bq, bk, bv = b_qkv[0:D], b_qkv[D:2 * D], b_qkv[2 * D:3 * D]

    def pmaj(v, n):
        return np.ascontiguousarray(
            np.asarray(v, np.float32).reshape(n, 128).T)

    def chunk8(wT, nk):
        # [K, M] -> [128, nk, M] (K = nk*128, chunk-major along K)
        return np.ascontiguousarray(
            wT.reshape(nk, 128, wT.shape[1]).transpose(1, 0, 2))

    wqT = np.ascontiguousarray(np.asarray(wq, np.float32).T)
    wkT = np.ascontiguousarray(np.asarray(wk, np.float32).T)
    wvT = np.ascontiguousarray(np.asarray(wv, np.float32).T)
    woT = np.ascontiguousarray(np.asarray(w_out, np.float32).T)
    w1g = np.asarray(w1, np.float32) * np.asarray(g1, np.float32)[None, :]
    w1T = np.ascontiguousarray(w1g.T)          # [D, FF]
    w2T = np.ascontiguousarray(np.asarray(w2, np.float32).T)  # [FF, D]

    w2s = w2T * SW
    w2hT = w2s.astype(E4)
    w2lT = (w2s - w2hT.astype(np.float32)).astype(E4)

    b1f = np.asarray(b1, np.float32) + w1g @ np.asarray(beta1, np.float32)
    bb2 = np.asarray(b2, np.float32) + np.asarray(beta1, np.float32)

    mu = np.zeros((9, 128), np.float32)
    mw = np.zeros((9, 128), np.float32)
    for w in range(8):
        mu[w, w * 16:(w + 1) * 16] = MASKC
        mw[w, w * 16:(w + 1) * 16] = 1.0
    mu[8, :] = -MASKC
    mw[8, :] = 1.0
    mwr = np.tile(mw, (1, 4))

    shared = {
        "wq8": chunk8(wqT * SW, NCH).astype(E4),
        "wk8": chunk8(wkT * SW, NCH).astype(E4),
        "wv8": chunk8(wvT * SW, NCH).astype(E4),
        "wo8": chunk8(woT * SW, NCH).astype(E4),
        "w1h": chunk8(w1T * SW, NCH).astype(E4),
        "w2h": chunk8(w2hT.astype(np.float32), NHC).astype(E4),
        "w2l": chunk8(w2lT.astype(np.float32), NHC).astype(E4),
        "bqp": pmaj(bq, NCH), "bkp": pmaj(bk, NCH),
        "bvS": (np.asarray(bv, np.float32) * SX * SW).reshape(1, D).astype(NF),
        "b1p": pmaj(b1f, NHC),
        "boS": pmaj(np.asarray(b_out, np.float32), NCH),
        "bb2p": pmaj(bb2, NCH),
        "g1p": pmaj(g1, NCH),
        "g2p": pmaj(g2, NCH), "be2p": pmaj(beta2, NCH),
        "masku": mu.astype(NF), "maskw": mwr.astype(NF),
        "onesrow": np.ones((1, 128), np.float32).astype(NF),
        "onescol": np.ones((128, 1), np.float32).astype(NF),
    }
    flags = (
        bool(np.all(np.asarray(bv) == 0)),                       # zbv
        bool(np.all(b1f == 0)),                                  # zb1
        bool(np.all(np.asarray(b_out) == 0)),                    # zbo
        bool(np.all(bb2 == 0)
             and np.all(np.asarray(g1, np.float32) == 1.0)),     # zb2a1
        bool(np.all(np.asarray(beta2) == 0)
             and np.all(np.asarray(g2, np.float32) == 1.0)),     # za2
    )
    return shared, flags


def make_in_maps(inputs):
    ff = np.asarray(inputs["frame_features"], np.float32)
    fi = np.asarray(inputs["frame_indices"])
    shared, flags = _prep_shared(
        np.asarray(inputs["w_qkv"]), np.asarray(inputs["b_qkv"]),
        np.asarray(inputs["w_out"]), np.asarray(inputs["b_out"]),
        np.asarray(inputs["w1"]), np.asarray(inputs["b1"]),
        np.asarray(inputs["w2"]), np.asarray(inputs["b2"]),
        np.asarray(inputs["g1"]), np.asarray(inputs["beta1"]),
        np.asarray(inputs["g2"]), np.asarray(inputs["beta2"]))

    div = np.exp(np.float32(-np.log(10000.0))
                 * np.arange(0, D, 2, dtype=np.float32) / np.float32(D))
    in_maps = []
    for b in range(B):
        pos = np.asarray(fi[b], np.float32)[:, None]
        ang = pos * div[None, :]
        pe = np.empty((T, D), np.float32)
        pe[:, 0::2] = np.sin(ang)
        pe[:, 1::2] = np.cos(ang)
        xpe = ff[b] + pe                       # [T, D]
        xpeT = np.ascontiguousarray(xpe.T)     # [D, T]
        x8 = np.ascontiguousarray(
            (xpeT * SX).reshape(NCH, 128, T).transpose(1, 0, 2)).astype(E4)
        xsc = np.ascontiguousarray(
            xpeT.reshape(NCH, 128, T).transpose(1, 0, 2)).astype(NF)
        m = dict(shared)
        m["x8"] = x8
        m["xs"] = xsc
        in_maps.append(m)
    return in_maps, flags


def get_nc(flags=(True, True, True, True, True)):
    if flags not in _NC_CACHE:
        _NC_CACHE[flags] = build_nc(flags)
    return _NC_CACHE[flags]


def kernel(**inputs) -> np.ndarray:
    in_maps, flags = make_in_maps(inputs)
    nc = get_nc(flags)
    res = run_bass_kernel_spmd(nc, in_maps, core_ids=list(range(B)))
    outs = []
    for r in res.results:
        ob = np.asarray(r["outb"])             # [128, NCH, T] fp16
        oT = ob.transpose(1, 0, 2).reshape(D, T)
        outs.append(oT.T.astype(np.float32))
    return np.ascontiguousarray(np.stack(outs))
